# revision 1
# baseline (speedup 1.0000x reference)
"""Trainium2 Bass kernel: AutoregressiveSelfAttention (sparse_attention).

Sharding: 8 cores, token-parallel with zigzag causal load balancing.
  core i -> batch b = i//4, j = i%4, query chunks cA = j, cB = 7-j (256 tokens each).
  Each core computes the full per-batch KV locally, runs attention for its 512
  query tokens, and the output projection for them. Host reassembles the 8
  disjoint output slices.

Wire format (dominates wall time through the axon tunnel): ONE u8 blob input
per core + ONE int8 output. x/side and ALL weights (q/k/v and proj) travel
int8 with per-channel f32 scales, dequantized to bf16 on device (int8+scale
beats fp8 e4m3 ~3x in accuracy; fp8 wire fails the 2e-2 gate); biases f32.
The x/side shards (1/4 per core, both batches) and weight shards (1/8) are
AllGathered on device over 8-core groups with Shared outputs (the 4-group
collective's forced-Local path is ~4x slower), so replicated bytes never
cross the host link; each core selects its batch half of xs with one
partition-id-offset DMA. Causal masks are generated on device (iota + a
per-core q-offset row), the q token columns are sliced out of the gathered
xs at runtime via partition-id, and the output is quantized per token to
int8 (PE transpose + abs-max), with the f32 scales embedded in the output
tensor's tail rows.

Device layouts (per core):
  scores as sT[kv, q] (kv on partitions) so softmax needs no transpose; the
  denominator is folded into the AV matmul via an augmented V (97th channel);
  exp needs no max-subtraction (scores are O(1)).
  k^T/q^T are head-padded to 32-row strips so score matmuls address them in
  place via tile_position. Compute instructions here may carry only ONE
  semaphore wait, so DMA-loaded tiles get same-engine pre-touches before
  their consumers (with _legalize_waits as the generic backstop).
"""

import sys

sys.path.insert(0, "/opt/trn_rl_repo")

import numpy as np
import ml_dtypes

import jax

# Persistent XLA compilation cache: run_bass_via_pjrt re-jits a fresh closure
# every call, so without this the whole BIR->NEFF pipeline reruns per call
# (~200ms). With it, repeat dispatches deserialize the cached executable.
jax.config.update("jax_compilation_cache_dir", "/tmp/jax_cc_cache")
jax.config.update("jax_persistent_cache_min_compile_time_secs", 0)
jax.config.update("jax_persistent_cache_min_entry_size_bytes", 0)

import concourse.bass as bass
import concourse.mybir as mybir
from concourse.tile import TileContext
from concourse.bass_utils import run_bass_kernel_spmd

BF16 = mybir.dt.bfloat16
F32 = mybir.dt.float32
NP_BF16 = ml_dtypes.bfloat16
I8 = mybir.dt.int8
AF = mybir.ActivationFunctionType

N_HEAD = 12
N_KQ = 192
N_OUT = 1152
HD_K = 16
HD_V = 96
HD_VA = 97             # v head channels + denominator column
N_VA = N_HEAD * HD_VA  # 1164
N_KP = N_HEAD * 32     # 384: head-padded k/q channel count
B, L = 2, 2048
CH = 256

# ---- blob layout (byte offsets) ----
XS_SH_ROWS = N_OUT // 4          # 288 rows of xsT per core (4-way gather)
W_ROWS = 944                     # packed q/k/v weight rows (2048 int8 cols)
W_SH_ROWS = W_ROWS // 8          # 118
WPH_SH_ROWS = N_OUT // 8         # 144

O_XS = 0
O_W = O_XS + XS_SH_ROWS * 2048                  # 589824 (int8 xs shard)
O_WPH = O_W + W_SH_ROWS * 2048                  # 831488 (int8 w shard)
O_BIAS = O_WPH + WPH_SH_ROWS * N_OUT            # 997376 (int8 wph shard)
N_BIAS = N_KP + N_KP + N_VA + N_OUT             # 3084 f32
O_QOFF = O_BIAS + N_BIAS * 4                    # 1009712
O_XSC = O_QOFF + 2 * CH * 4                     # 1011760
O_WSC = O_XSC + N_OUT * 4                       # 1016368
N_WSC = N_KP + N_OUT + N_OUT                    # 2688 f32 w row scales
O_WPSC = O_WSC + N_WSC * 4                      # 1027120
BLOB_BYTES = O_WPSC + N_OUT * 4                 # 1031728

WQ_ELS = N_KP * N_KP            # 147456 (padded wq is [384, 384])
WK_ELS = N_OUT * N_KP           # 442368
WV_ELS = N_OUT * N_VA           # 1340928

_NC_CACHE = None


MAGIC = 12582912.0  # 1.5 * 2**23: f32 add/sub rounds to nearest integer
OUT_ROWS = 2 * CH + 2  # 512 token rows int8 + 2 rows carrying 512 f32 scales


def _build_graph():
    nc = bass.Bass(num_devices=8)
    blob = nc.declare_dram_parameter("blob", [BLOB_BYTES], mybir.dt.uint8,
                                     isOutput=False)
    out_d = nc.declare_dram_parameter("out", [OUT_ROWS, N_OUT], mybir.dt.int8,
                                      isOutput=True)

    bap = blob.ap()
    xs_sh_ap = bap[O_XS:O_W].bitcast(I8).rearrange("(p n) -> p n",
                                                   p=XS_SH_ROWS)
    w_sh_ap = bap[O_W:O_WPH].bitcast(I8).rearrange("(p n) -> p n",
                                                   p=W_SH_ROWS)
    wph_sh_ap = (bap[O_WPH:O_BIAS].bitcast(I8)
                 .rearrange("(p n) -> p n", p=WPH_SH_ROWS))
    bias_ap = bap[O_BIAS:O_QOFF].bitcast(F32)
    qoff_ap = (bap[O_QOFF:O_XSC].bitcast(F32)
               .rearrange("(o n) -> o n", o=1))
    xsc_ap = (bap[O_XSC:O_WSC].bitcast(F32)
              .rearrange("(e p) -> p e", e=9))
    wsc_ap = bap[O_WSC:O_WPSC].bitcast(F32)
    wpsc_ap = (bap[O_WPSC:BLOB_BYTES].bitcast(F32)
               .rearrange("(h p) -> p h", p=96))
    wqsc_ap = wsc_ap[0:N_KP].rearrange("(m p) -> p m", p=128)
    wksc_ap = wsc_ap[N_KP:N_KP + N_OUT].rearrange("(e p) -> p e", p=128)
    wvsc_ap = (wsc_ap[N_KP + N_OUT:N_WSC]
               .rearrange("(e p) -> p e", p=128))
    bq_ap = bias_ap[0:N_KP].rearrange("(m p) -> p m", p=128)
    bk_ap = bias_ap[N_KP:2 * N_KP].rearrange("(m p) -> p m", p=128)
    bv_ap = bias_ap[2 * N_KP:2 * N_KP + N_VA].rearrange("(o n) -> o n", o=1)
    bp_ap = (bias_ap[2 * N_KP + N_VA:N_BIAS]
             .rearrange("(m p) -> p m", p=128))

    # gather staging + outputs (collectives cannot read IO tensors)
    xs_stage = nc.dram_tensor("xs_stage", [XS_SH_ROWS, 2048], I8)
    w_stage = nc.dram_tensor("w_stage", [W_SH_ROWS, 2048], I8)
    wph_stage = nc.dram_tensor("wph_stage", [WPH_SH_ROWS, N_OUT], I8)
    # 8-group gather of BOTH batches (Shared output is much faster than the
    # 4-group/Local path); this core's batch half is then selected by one
    # partition-id-offset dram->dram DMA.
    xs_all = nc.dram_tensor("xs_all", [2 * N_OUT, 2048], I8,
                            addr_space="Shared")
    xs_full = nc.dram_tensor("xs_full", [N_OUT, 2048], I8)
    w_full = nc.dram_tensor("w_full", [W_ROWS, 2048], I8, addr_space="Shared")
    wph_full = nc.dram_tensor("wph_full", [N_OUT, N_OUT], I8,
                              addr_space="Shared")

    with TileContext(nc) as tc, tc.tile_pool(name="resident", bufs=1) as pr:
        # ---- resident tiles ----
        kpad = pr.tile([128, 3, L], BF16)        # k^T head-padded (32 rows/head)
        qpad = pr.tile([128, 3, 2 * CH], BF16)
        v_t = pr.tile([128, L // 128, N_VA], BF16)   # augmented v, token-major
        mC_t = pr.tile([128, 8, 2 * CH], BF16)
        mD_t = pr.tile([128, 8, CH], BF16)
        wph_t = pr.tile([96, 12, N_OUT], BF16)
        bp_t = pr.tile([128, 9], F32)
        id_t = pr.tile([128, 128], F32)      # identity for PE transposes
        yts = [pr.tile([HD_V, 2 * CH], BF16, name=f"yt{h}", tag=f"yt{h}")
               for h in range(N_HEAD)]

        with (
            tc.tile_pool(name="loads", bufs=1) as pw,
            tc.tile_pool(name="xsp", bufs=1) as pxs,
            tc.tile_pool(name="xstage", bufs=2) as pst,
            tc.tile_pool(name="scratch", bufs=1) as psc,
            tc.tile_pool(name="ps_small", bufs=2, space="PSUM") as psp,
            tc.tile_pool(name="ps_v", bufs=2, space="PSUM") as psv,
        ):
            # ---- stage shards, all-gather on device ----
            pid = nc.partition_id()
            jv = pid % 4
            nc.sync.dma_start(out=xs_stage.ap(), in_=xs_sh_ap)
            nc.sync.dma_start(out=w_stage.ap(), in_=w_sh_ap)
            nc.sync.dma_start(out=wph_stage.ap(), in_=wph_sh_ap)
            nc.gpsimd.collective_compute(
                "AllGather", mybir.AluOpType.bypass,
                replica_groups=[[0, 1, 2, 3, 4, 5, 6, 7]],
                ins=[xs_stage.ap()], outs=[xs_all.ap()],
            )
            boff = nc.s_assert_within((pid - jv) * (N_OUT // 4), 0, N_OUT,
                                      skip_runtime_assert=True)
            nc.gpsimd.dma_start(
                out=xs_full.ap(),
                in_=xs_all.ap()[bass.ds(boff, N_OUT), :],
            )
            nc.gpsimd.collective_compute(
                "AllGather", mybir.AluOpType.bypass,
                replica_groups=[[0, 1, 2, 3, 4, 5, 6, 7]],
                ins=[w_stage.ap()], outs=[w_full.ap()],
            )
            nc.gpsimd.collective_compute(
                "AllGather", mybir.AluOpType.bypass,
                replica_groups=[[0, 1, 2, 3, 4, 5, 6, 7]],
                ins=[wph_stage.ap()], outs=[wph_full.ap()],
            )

            # ---- SBUF loads; xs dequantized per 128-channel slab ----
            xs_t = pxs.tile([128, 9, L], BF16)
            xsc_t = pw.tile([128, 9], F32)
            nc.sync.dma_start(out=xsc_t, in_=xsc_ap)
            scv0 = pxs.tile([1, 16], F32, tag="scv0")
            nc.vector.tensor_copy(scv0[0:1, 0:1], xsc_t[0:1, 0:1])  # pre-touch
            xsf_r = xs_full.ap().rearrange("(e p) n -> p e n", p=128)
            for e in range(9):
                st8 = pst.tile([128, L], I8, tag="st8")
                nc.sync.dma_start(out=st8, in_=xsf_r[:, e, :])
                nc.vector.tensor_scalar(xs_t[:, e, :], st8,
                                        xsc_t[:, e:e + 1], None,
                                        mybir.AluOpType.mult)
            wfl = w_full.ap().flatten()
            wqsc_t = pw.tile([128, 3], F32, tag="wqsc")
            nc.sync.dma_start(out=wqsc_t, in_=wqsc_ap)
            wksc_t = pw.tile([128, 9], F32, tag="wksc")
            nc.sync.dma_start(out=wksc_t, in_=wksc_ap)
            wvsc_t = pw.tile([128, 9], F32, tag="wvsc")
            nc.sync.dma_start(out=wvsc_t, in_=wvsc_ap)
            wq_t = pw.tile([128, 3, N_KP], BF16)
            wq8_t = pw.tile([128, 3, N_KP], I8, tag="wq8")
            nc.sync.dma_start(
                out=wq8_t,
                in_=wfl[0:WQ_ELS].rearrange("(m p n) -> p m n", m=3, p=128))
            for m in range(3):
                nc.vector.tensor_scalar(wq_t[:, m, :], wq8_t[:, m, :],
                                        wqsc_t[:, m:m + 1], None,
                                        mybir.AluOpType.mult)
            wk_t = pw.tile([128, 9, N_KP], BF16)
            wv_t = pw.tile([128, 9, N_VA], BF16)
            for e in range(9):
                st = pst.tile([128, N_KP], I8, tag="wk8")
                nc.sync.dma_start(
                    out=st,
                    in_=wfl[WQ_ELS + e * 128 * N_KP:
                            WQ_ELS + (e + 1) * 128 * N_KP]
                    .rearrange("(p n) -> p n", p=128))
                nc.vector.tensor_scalar(wk_t[:, e, :], st,
                                        wksc_t[:, e:e + 1], None,
                                        mybir.AluOpType.mult)
            for e in range(9):
                st = pst.tile([128, N_VA], I8, tag="wv8")
                nc.sync.dma_start(
                    out=st,
                    in_=wfl[WQ_ELS + WK_ELS + e * 128 * N_VA:
                            WQ_ELS + WK_ELS + (e + 1) * 128 * N_VA]
                    .rearrange("(p n) -> p n", p=128))
                nc.vector.tensor_scalar(wv_t[:, e, :], st,
                                        wvsc_t[:, e:e + 1], None,
                                        mybir.AluOpType.mult)
            wpsc_t = pw.tile([96, 12], F32, tag="wpsc")
            nc.sync.dma_start(out=wpsc_t, in_=wpsc_ap)
            wphf_r = wph_full.ap().rearrange("(h p) n -> p h n", p=96)
            for h in range(N_HEAD):
                st = pst.tile([96, N_OUT], I8, tag="wph8")
                nc.sync.dma_start(out=st, in_=wphf_r[:, h, :])
                nc.vector.tensor_scalar(wph_t[:, h, :], st,
                                        wpsc_t[:, h:h + 1], None,
                                        mybir.AluOpType.mult)
            qb_t = pw.tile([128, 2 * CH], F32)
            nc.sync.dma_start(out=qb_t, in_=qoff_ap.to_broadcast([128, 2 * CH]))
            bq_t = pw.tile([128, 3], F32)
            nc.sync.dma_start(out=bq_t, in_=bq_ap)
            bk_t = pw.tile([128, 3], F32)
            nc.sync.dma_start(out=bk_t, in_=bk_ap)
            bv_t = pw.tile([128, N_VA], F32)
            nc.sync.dma_start(out=bv_t, in_=bv_ap.to_broadcast([128, N_VA]))
            nc.sync.dma_start(out=bp_t, in_=bp_ap)

            # ---- pre-touches: give each engine 1-wait visibility of loads ----
            dps = psp.tile([128, 512], F32, tag="ps")
            for i, t in enumerate(
                [xs_t[0:1, 0, 0:1], wq_t[0:1, 0, 0:1],
                 wk_t[0:1, 0, 0:1], wv_t[0:1, 0, 0:1], wph_t[0:1, 0, 0:1]]
            ):
                nc.tensor.matmul(dps[0:1, i:i + 1], lhsT=t, rhs=t,
                                 start=True, stop=True)
            sc = psc.tile([1, 16], F32)
            nc.scalar.activation(sc[0:1, 0:1], bq_t[0:1, 0:1], AF.Copy)
            nc.scalar.activation(sc[0:1, 1:2], bk_t[0:1, 0:1], AF.Copy)
            nc.scalar.activation(sc[0:1, 2:3], bp_t[0:1, 0:1], AF.Copy)
            scv = psc.tile([1, 16], F32, tag="scv")
            nc.vector.tensor_copy(scv[0:1, 0:1], bv_t[0:1, 0:1])
            nc.vector.tensor_copy(scv[0:1, 1:2], qb_t[0:1, 0:1])
            # ACT warm-up of Exp's implicit const-bias AP
            sce = psc.tile([1, 16], F32, tag="sce")
            nc.scalar.activation(sce[0:1, 0:1], scv[0:1, 0:1], AF.Exp)

            # ---- mask gen: m[p, f] = (qidx[f] - (128*kt + p) > 0) ----
            ci_t = pw.tile([128, 2 * CH], mybir.dt.int32)
            nc.gpsimd.iota(ci_t, pattern=[[1, 2 * CH]], base=0,
                           channel_multiplier=-1)
            cif_t = pw.tile([128, 2 * CH], F32)
            nc.vector.tensor_copy(cif_t, ci_t)
            mb_t = pw.tile([128, 2 * CH], F32)
            nc.vector.tensor_add(mb_t, cif_t, qb_t)
            for kt in range(8):
                nc.vector.tensor_scalar(
                    mC_t[:, kt, :], mb_t, float(128 * kt), None,
                    mybir.AluOpType.is_gt)
            for kt in range(8, 16):
                nc.vector.tensor_scalar(
                    mD_t[:, kt - 8, :], mb_t[:, CH:], float(128 * kt), None,
                    mybir.AluOpType.is_gt)
            nc.vector.tensor_scalar(id_t, cif_t[:, :128], 0.0, None,
                                    mybir.AluOpType.is_equal)

            # ---- q projection: qpad[384, 512]; q-token columns are sliced
            #      out of the gathered xs at runtime via partition-id ----
            offA = jv * CH
            offB = (7 - jv) * CH
            sq_t = pw.tile([128, 3, 2 * CH], BF16)
            for e in range(3):
                nc.vector.tensor_copy(sq_t[:, e, :CH],
                                      xs_t[:, 6 + e, bass.ds(offA, CH)])
                nc.vector.tensor_copy(sq_t[:, e, CH:],
                                      xs_t[:, 6 + e, bass.ds(offB, CH)])
            for m in range(3):
                ps = psp.tile([128, 2 * CH], F32, tag="ps")
                for e in range(3):
                    nc.tensor.matmul(
                        ps, lhsT=wq_t[:, e, m * 128:(m + 1) * 128],
                        rhs=sq_t[:, e, :],
                        start=(e == 0), stop=(e == 2),
                    )
                nc.scalar.activation(qpad[:, m, :], ps, AF.Identity,
                                     bias=bq_t[:, m:m + 1])

            # ---- k projection: kpad[384, 2048], 512-token slabs ----
            for m in range(3):
                for nt in range(L // 512):
                    ps = psp.tile([128, 512], F32, tag="ps")
                    for e in range(9):
                        nc.tensor.matmul(
                            ps,
                            lhsT=wk_t[:, e, m * 128:(m + 1) * 128],
                            rhs=xs_t[:, e, nt * 512:(nt + 1) * 512],
                            start=(e == 0), stop=(e == 8),
                        )
                    nc.scalar.activation(
                        kpad[:, m, nt * 512:(nt + 1) * 512], ps, AF.Identity,
                        bias=bk_t[:, m:m + 1],
                    )

            # ---- v projection: v[2048, 1164] (token-major, augmented) ----
            for c in range(L // 128):
                ps = psv.tile([128, N_VA], F32, tag="vps")
                for e in range(9):
                    for n0, nn in [(0, 512), (512, 512), (1024, N_VA - 1024)]:
                        nc.tensor.matmul(
                            ps[:, n0:n0 + nn],
                            lhsT=xs_t[:, e, c * 128:(c + 1) * 128],
                            rhs=wv_t[:, e, n0:n0 + nn],
                            start=(e == 0), stop=(e == 8),
                        )
                nc.vector.tensor_add(v_t[:, c, :], ps, bv_t)

        # ---- attention ----
        with (
            tc.tile_pool(name="ps_s", bufs=4, space="PSUM") as pss,
            tc.tile_pool(name="ps_y", bufs=3, space="PSUM") as psy,
            tc.tile_pool(name="exps", bufs=20) as pe,
            tc.tile_pool(name="norm", bufs=2) as pn,
            tc.tile_pool(name="rdram", bufs=6, space="DRAM") as pdram,
        ):
            for h in range(N_HEAD):
                t, a = h // 4, 32 * (h % 4)
                ems = []
                for kt in range(8):
                    s_ps = pss.tile([128, 2 * CH], F32, tag="sps")
                    nc.tensor.matmul(
                        s_ps,
                        lhsT=kpad[a:a + HD_K, t, kt * 128:(kt + 1) * 128],
                        rhs=qpad[a:a + HD_K, t, :],
                        start=True, stop=True,
                        tile_position=(a, 0),
                    )
                    e_sb = pe.tile([128, 2 * CH], BF16, tag="esb")
                    nc.scalar.activation(e_sb, s_ps, AF.Exp, scale=0.25)
                    em_sb = pe.tile([128, 2 * CH], BF16, tag="emsb")
                    nc.vector.tensor_mul(em_sb, e_sb, mC_t[:, kt, :])
                    ems.append(em_sb)
                for kt in range(8, 16):
                    s_ps = pss.tile([128, 2 * CH], F32, tag="sps")
                    nc.tensor.matmul(
                        s_ps[:, :CH],
                        lhsT=kpad[a:a + HD_K, t, kt * 128:(kt + 1) * 128],
                        rhs=qpad[a:a + HD_K, t, CH:],
                        start=True, stop=True,
                        tile_position=(a, 0),
                    )
                    e_sb = pe.tile([128, 2 * CH], BF16, tag="esb")
                    nc.scalar.activation(e_sb[:, :CH], s_ps[:, :CH], AF.Exp,
                                         scale=0.25)
                    em_sb = pe.tile([128, 2 * CH], BF16, tag="emsb")
                    nc.vector.tensor_mul(em_sb[:, :CH], e_sb[:, :CH],
                                         mD_t[:, kt - 8, :])
                    ems.append(em_sb)
                y_ps = psy.tile([HD_VA, 2 * CH], F32, tag="yps")
                for kt in range(8):
                    nc.tensor.matmul(
                        y_ps,
                        lhsT=v_t[:, kt, h * HD_VA:(h + 1) * HD_VA],
                        rhs=ems[kt],
                        start=(kt == 0), stop=False,
                    )
                for kt in range(8, 16):
                    nc.tensor.matmul(
                        y_ps[:, CH:],
                        lhsT=v_t[:, kt, h * HD_VA:(h + 1) * HD_VA],
                        rhs=ems[kt][:, :CH],
                        start=False, stop=(kt == 15),
                    )
                # normalize: row 96 of y_ps is the softmax denominator
                # (clamped away from 0 so the dead q=0 column yields 0, not NaN)
                r_sb = pn.tile([128, 2 * CH], F32, tag="rsb")
                rmx = pn.tile([128, 2 * CH], F32, tag="rmx")
                nc.vector.tensor_scalar_max(rmx[96:97, :], y_ps[96:97, :],
                                            1e-30)
                nc.vector.reciprocal(r_sb[96:97, :], rmx[96:97, :])
                rd = pdram.tile([1, 2 * CH], F32, tag="rd")
                nc.sync.dma_start(out=rd, in_=r_sb[96:97, :])
                rb_t = pn.tile([HD_V, 2 * CH], F32, tag="rbt")
                nc.sync.dma_start(
                    out=rb_t, in_=rd[0:1, :].to_broadcast([HD_V, 2 * CH])
                )
                rtc = pn.tile([1, 1], F32, tag="rtc")
                nc.vector.tensor_copy(rtc, rb_t[0:1, 0:1])  # pre-touch
                nc.vector.tensor_mul(yts[h], y_ps[:HD_V, :], rb_t)

        # ---- output projection: outT[1152, 512] = sum_h Wp_h^T @ y_h,
        #      then per-token int8 quantization: transpose, abs-max, scale ----
        with (
            tc.tile_pool(name="ps_o", bufs=2, space="PSUM") as pso,
            tc.tile_pool(name="ps_q", bufs=2, space="PSUM") as psq,
            tc.tile_pool(name="qsb", bufs=3) as pq,
            tc.tile_pool(name="qsc", bufs=1) as pqs,
        ):
            outb = pqs.tile([128, 9, 2 * CH], F32, tag="outb")
            for mo in range(9):
                ps = pso.tile([128, 2 * CH], F32)
                for h in range(N_HEAD):
                    nc.tensor.matmul(
                        ps,
                        lhsT=wph_t[:, h, mo * 128:(mo + 1) * 128],
                        rhs=yts[h],
                        start=(h == 0), stop=(h == N_HEAD - 1),
                    )
                nc.scalar.activation(outb[:, mo, :], ps, AF.Identity,
                                     bias=bp_t[:, mo:mo + 1])
            sc_all = pqs.tile([128, 4], F32)
            rcp = pqs.tile([128, 4], F32, tag="rcp")
            mxs = pqs.tile([128, 4], F32, tag="mxs")
            for tcn in range(4):
                psT = psq.tile([128, N_OUT], F32, tag="psT")
                for mo in range(9):
                    nc.tensor.matmul(
                        psT[:, mo * 128:(mo + 1) * 128],
                        lhsT=outb[:, mo, tcn * 128:(tcn + 1) * 128],
                        rhs=id_t, is_transpose=True,
                        start=True, stop=True,
                    )
                nc.vector.tensor_reduce(
                    mxs[:, tcn:tcn + 1], psT, axis=mybir.AxisListType.X,
                    op=mybir.AluOpType.max, apply_absolute_value=True)
                nc.vector.tensor_scalar_mul(sc_all[:, tcn:tcn + 1],
                                            mxs[:, tcn:tcn + 1], 1.0 / 127.0)
                nc.vector.reciprocal(rcp[:, tcn:tcn + 1],
                                     sc_all[:, tcn:tcn + 1])
                qf = pq.tile([128, N_OUT], F32, tag="qf")
                nc.vector.tensor_scalar(qf, psT, rcp[:, tcn:tcn + 1], MAGIC,
                                        mybir.AluOpType.mult,
                                        mybir.AluOpType.add)
                qg = pq.tile([128, N_OUT], F32, tag="qg")
                nc.vector.tensor_scalar(qg, qf, MAGIC, None,
                                        mybir.AluOpType.subtract)
                qi = pq.tile([128, N_OUT], mybir.dt.int8, tag="qi")
                nc.vector.tensor_copy(qi, qg)
                nc.sync.dma_start(
                    out=out_d.ap()[tcn * 128:(tcn + 1) * 128, :], in_=qi)
            sc_dst = (out_d.ap()[2 * CH:OUT_ROWS, :].flatten()[0:2 * CH * 4]
                      .bitcast(F32).rearrange("(p n) -> p n", p=128))
            nc.sync.dma_start(out=sc_dst, in_=sc_all)
    return nc


def _legalize_waits(nc):
    """This walrus build accepts only ONE sync-wait per regular instruction;
    move overflow waits onto injected same-engine NoOps (like raw-bass
    wait_ge)."""
    keep = ("InstEventSemaphore",)
    cnt = 0
    for bbh in nc.bb_map.values():
        bb = bbh.bb
        new_list = []
        for inst in bb.instructions:
            si = inst.sync_info
            if (si is not None and len(si.on_wait) > 1
                    and type(inst).__name__ not in keep):
                waits = list(si.on_wait)
                for w in waits[:-1]:
                    cnt += 1
                    n = mybir.InstNoOp(name=f"legwait_{cnt}", ins=[], outs=[])
                    n.engine = inst.engine
                    n.sync_info = mybir.SyncInfo(on_wait=[w], on_update=[])
                    try:
                        nc.register_instruction(n)
                    except Exception:
                        pass
                    new_list.append(n)
                inst.sync_info = mybir.SyncInfo(
                    on_wait=[waits[-1]], on_update=list(si.on_update))
            new_list.append(inst)
        bb.instructions = new_list
    return cnt


def _get_nc():
    global _NC_CACHE
    if _NC_CACHE is None:
        nc = _build_graph()
        _legalize_waits(nc)
        # The pjrt lowering re-serializes the (frozen, never-mutated) graph
        # on every dispatch (~25ms for this BIR); memoize the identical bytes.
        raw = nc.to_json_bytes()
        nc.to_json_bytes = lambda: raw
        _NC_CACHE = nc
    return _NC_CACHE


def _head_pad_kq(W, b):
    """[in, 192] -> [in, 384] with head h cols at 128*(h//4)+32*(h%4)."""
    Wp = np.zeros((W.shape[0], N_KP), np.float32)
    bp = np.zeros((N_KP,), np.float32)
    for h in range(N_HEAD):
        c = 128 * (h // 4) + 32 * (h % 4)
        Wp[:, c:c + HD_K] = W[:, h * HD_K:(h + 1) * HD_K]
        bp[c:c + HD_K] = b[h * HD_K:(h + 1) * HD_K]
    return Wp, bp


def _bf(a):
    return np.ascontiguousarray(a.astype(NP_BF16))


def _prep_inputs(x, side, Wq, bq, Wkv, bkv, Wproj, bproj):
    Wk = Wkv[:, :N_KQ]
    Wv = Wkv[:, N_KQ:]
    bk = bkv[:N_KQ]
    bv = bkv[N_KQ:]
    Wq_p, bq_p = _head_pad_kq(Wq, bq)
    Wk_p, bk_p = _head_pad_kq(Wk, bk)
    # augmented V: per head 96 channels + a zero-weight/one-bias denom channel
    Wv_a = np.zeros((N_OUT, N_VA), np.float32)
    bv_a = np.zeros((N_VA,), np.float32)
    for h in range(N_HEAD):
        Wv_a[:, h * HD_VA:h * HD_VA + HD_V] = Wv[:, h * HD_V:(h + 1) * HD_V]
        bv_a[h * HD_VA:h * HD_VA + HD_V] = bv[h * HD_V:(h + 1) * HD_V]
        bv_a[h * HD_VA + HD_V] = 1.0

    # packed int8 q/k/v weights (per input-channel-row scales), [944, 2048]
    def q8_rows(W):
        sc = np.maximum(np.abs(W).max(axis=1), 1e-30) / 127.0
        q = np.clip(np.round(W / sc[:, None]), -127, 127).astype(np.int8)
        return q, sc.astype(np.float32)

    wq8, wqsc = q8_rows(Wq_p)
    wk8, wksc = q8_rows(Wk_p)
    wv8, wvsc = q8_rows(Wv_a)
    wpack = np.zeros((W_ROWS * 2048,), np.int8)
    wpack[0:WQ_ELS] = wq8.ravel()
    wpack[WQ_ELS:WQ_ELS + WK_ELS] = wk8.ravel()
    wpack[WQ_ELS + WK_ELS:WQ_ELS + WK_ELS + WV_ELS] = wv8.ravel()
    wpack = wpack.reshape(W_ROWS, 2048)
    wscales = np.concatenate([wqsc, wksc, wvsc]).astype(np.float32)

    # Wproj rows per head, int8 per-row [1152, 1152]
    wph_all, wphsc = q8_rows(Wproj.reshape(N_HEAD * HD_V, N_OUT))

    biases = np.concatenate([bq_p, bk_p, bv_a, bproj]).astype(np.float32)

    # per-channel int8 scales for [x|side]^T, shared by the 4 cores of a batch
    xscs, xsqs = [], []
    for b in range(B):
        xsT = np.ascontiguousarray(np.concatenate([x[b], side[b]], axis=1).T)
        xsc = np.maximum(np.abs(xsT).max(axis=1), 1e-30) / 127.0
        xsq = np.clip(np.round(xsT / xsc[:, None]), -127, 127).astype(np.int8)
        xscs.append(xsc.astype(np.float32))
        xsqs.append(xsq)

    in_maps = []
    for i in range(8):
        b, j = i // 4, i % 4
        xs_shard = xsqs[b][XS_SH_ROWS * j:XS_SH_ROWS * (j + 1), :]
        # qidx[f] - f for the mask generator: q token of em column f
        qoff = np.empty((2 * CH,), np.float32)
        qoff[:CH] = 256 * j
        qoff[CH:] = 256 * (7 - j) - CH

        blob = np.empty((BLOB_BYTES,), np.uint8)
        blob[O_XS:O_W] = xs_shard.reshape(-1).view(np.uint8)
        blob[O_W:O_WPH] = (wpack[W_SH_ROWS * i:W_SH_ROWS * (i + 1), :]
                           .reshape(-1).view(np.uint8))
        blob[O_WPH:O_BIAS] = (wph_all[WPH_SH_ROWS * i:WPH_SH_ROWS * (i + 1), :]
                              .reshape(-1).view(np.uint8))
        blob[O_BIAS:O_QOFF] = biases.view(np.uint8)
        blob[O_QOFF:O_XSC] = qoff.view(np.uint8)
        blob[O_XSC:O_WSC] = xscs[b].view(np.uint8)
        blob[O_WSC:O_WPSC] = wscales.view(np.uint8)
        blob[O_WPSC:BLOB_BYTES] = wphsc.view(np.uint8)
        in_maps.append({"blob": blob})
    return in_maps


def kernel(x, side, Wq, bq, Wkv, bkv, Wproj, bproj, Wemb, bemb, **_unused):
    x = np.asarray(x, np.float32)
    side = np.asarray(side, np.float32)
    Wq = np.asarray(Wq, np.float32)
    bq = np.asarray(bq, np.float32)
    Wkv = np.asarray(Wkv, np.float32)
    bkv = np.asarray(bkv, np.float32)
    Wproj = np.asarray(Wproj, np.float32)
    bproj = np.asarray(bproj, np.float32)
    Wemb = np.asarray(Wemb, np.float32)
    bemb = np.asarray(bemb, np.float32)

    nc = _get_nc()
    in_maps = _prep_inputs(x, side, Wq, bq, Wkv, bkv, Wproj, bproj)
    res = run_bass_kernel_spmd(nc, in_maps, core_ids=list(range(8))).results

    ans = np.empty((B, L, N_OUT), np.float32)
    for i in range(8):
        b, j = i // 4, i % 4
        raw = np.asarray(res[i]["out"])          # [514, 1152] int8
        scales = (raw[2 * CH:].reshape(-1).view(np.float32)[:2 * CH]
                  .reshape(128, 4))              # [partition, chunk]
        vals = raw[:2 * CH].astype(np.float32)   # [512 tokens, 1152]
        for tcn in range(4):
            vals[tcn * 128:(tcn + 1) * 128] *= scales[:, tcn:tcn + 1]
        ans[b, 256 * j:256 * j + 256] = vals[:CH]
        ans[b, 256 * (7 - j):256 * (8 - j)] = vals[CH:]
    # first token: replaced by learned embedding of side[:, 0] (exact, host-side)
    for b in range(B):
        first = side[b, 0].astype(np.float64) @ Wemb.astype(np.float64) + bemb
        ans[b, 0] = (first @ Wproj.astype(np.float64) + bproj).astype(np.float32)
    return ans



# revision 2
# speedup vs baseline: 1.2906x; 1.2906x over previous
"""Trainium2 Bass kernel: AutoregressiveSelfAttention (sparse_attention).

Sharding: 8 cores x 2 pipelined dispatches (one per batch). In each
dispatch all 8 cores work one batch, token-parallel with zigzag causal
load balancing: core j -> query chunks cA = j, cB = 15-j (128 tokens
each). Each core computes the full per-batch KV locally and the output
projection for its 256 query tokens. Host reassembles the 16 disjoint
output slices.

Wire format (dominates wall time through the ~50MB/s axon tunnel): x/side
and ALL weights travel int8 with per-channel f32 scales, dequantized to
bf16 on device (int8+scale beats fp8 e4m3 ~3x in accuracy; fp8 wire fails
the 2e-2 gate); biases f32. The weights blob is uploaded to the device
mesh ONCE per kernel call (jax.device_put) and shared by both per-batch
dispatches; each dispatch additionally carries that batch's xsT shard
(1/8 per core). Both shards are AllGathered on device over the 8-core
group with Shared outputs, so replicated bytes never cross the host link.
The two dispatches overlap: batch 0's D2H result fetch runs concurrently
with batch 1's H2D upload + execute (the tunnel is full-duplex, and
in-flight PJRT dispatches pipeline). Donated output buffers are recycled
device-side between calls (the stock run_bass_kernel_spmd path uploads
fresh host zeros for donation every call - 4.7MB of dead H2D traffic -
and re-jits a fresh closure; both are avoided here with a cached jit and
a persistent spare-buffer pool, seeded once from an on-device zeros jit).

Device layouts (per core):
  scores as sT[kv, q] (kv on partitions) so softmax needs no transpose; the
  denominator is folded into the AV matmul via an augmented V (97th channel);
  exp needs no max-subtraction (scores are O(1)).
  k^T/q^T are head-padded to 32-row strips so score matmuls address them in
  place via tile_position. Compute instructions here may carry only ONE
  semaphore wait, so DMA-loaded tiles get same-engine pre-touches before
  their consumers (with _legalize_waits as the generic backstop).
  Causal masks are generated on device (iota + a per-core q-offset row),
  the q token columns are sliced out of the gathered xs at runtime via
  partition-id, and the output is quantized per token to int8 (PE
  transpose + abs-max), with the f32 scales embedded in the output
  tensor's tail row.
"""

import sys

sys.path.insert(0, "/opt/trn_rl_repo")

import numpy as np
import ml_dtypes

import jax
import jax.numpy as jnp
from jax.sharding import Mesh, PartitionSpec, NamedSharding
from jax.experimental.shard_map import shard_map

# Persistent XLA compilation cache: without this the whole BIR->NEFF
# pipeline reruns per process (~minutes). With it, repeat dispatches
# deserialize the cached executable.
jax.config.update("jax_compilation_cache_dir", "/tmp/jax_cc_cache")
jax.config.update("jax_persistent_cache_min_compile_time_secs", 0)
jax.config.update("jax_persistent_cache_min_entry_size_bytes", 0)

import concourse.bass as bass
import concourse.mybir as mybir
from concourse.tile import TileContext
from concourse.bass2jax import (
    _bass_exec_p,
    install_neuronx_cc_hook,
    partition_id_tensor,
)

BF16 = mybir.dt.bfloat16
F32 = mybir.dt.float32
NP_BF16 = ml_dtypes.bfloat16
I8 = mybir.dt.int8
AF = mybir.ActivationFunctionType

N_HEAD = 12
N_KQ = 192
N_OUT = 1152
HD_K = 16
HD_V = 96
HD_VA = 97             # v head channels + denominator column
N_VA = N_HEAD * HD_VA  # 1164
N_KP = N_HEAD * 32     # 384: head-padded k/q channel count
B, L = 2, 2048
CH = 128               # query chunk per core per zigzag leg

# ---- xblob layout (byte offsets): per-core per-batch activation shard ----
XS_SH_ROWS = N_OUT // 8          # 144 rows of xsT per core (8-way gather)
O_QOFF = XS_SH_ROWS * 2048       # 294912 (int8 xs shard)
O_XSC = O_QOFF + 2 * CH * 4      # 295936
XBLOB_BYTES = O_XSC + N_OUT * 4  # 300544

# ---- wblob layout: per-core weight shard, shared by both dispatches ----
W_ROWS = 944                     # packed q/k/v weight rows (2048 int8 cols)
W_SH_ROWS = W_ROWS // 8          # 118
WPH_SH_ROWS = N_OUT // 8         # 144
O_WPH = W_SH_ROWS * 2048                        # 241664
O_BIAS = O_WPH + WPH_SH_ROWS * N_OUT            # 407552
N_BIAS = N_KP + N_KP + N_VA + N_OUT             # 3084 f32
O_WSC = O_BIAS + N_BIAS * 4                     # 419888
N_WSC = N_KP + N_OUT + N_OUT                    # 2688 f32 w row scales
O_WPSC = O_WSC + N_WSC * 4                      # 430640
WBLOB_BYTES = O_WPSC + N_OUT * 4                # 435248

WQ_ELS = N_KP * N_KP            # 147456 (padded wq is [384, 384])
WK_ELS = N_OUT * N_KP           # 442368
WV_ELS = N_OUT * N_VA           # 1340928

MAGIC = 12582912.0  # 1.5 * 2**23: f32 add/sub rounds to nearest integer
OUT_ROWS = 2 * CH + 1  # 256 token rows int8 + 1 row carrying 256 f32 scales

_RUNNER = None


def _build_graph():
    nc = bass.Bass(num_devices=8)
    xblob = nc.declare_dram_parameter("xblob", [XBLOB_BYTES], mybir.dt.uint8,
                                      isOutput=False)
    wblob = nc.declare_dram_parameter("wblob", [WBLOB_BYTES], mybir.dt.uint8,
                                      isOutput=False)
    out_d = nc.declare_dram_parameter("out", [OUT_ROWS, N_OUT], mybir.dt.int8,
                                      isOutput=True)

    xbap = xblob.ap()
    xs_sh_ap = xbap[0:O_QOFF].bitcast(I8).rearrange("(p n) -> p n",
                                                    p=XS_SH_ROWS)
    qoff_ap = (xbap[O_QOFF:O_XSC].bitcast(F32)
               .rearrange("(o n) -> o n", o=1))
    xsc_ap = (xbap[O_XSC:XBLOB_BYTES].bitcast(F32)
              .rearrange("(e p) -> p e", e=9))

    wbap = wblob.ap()
    w_sh_ap = wbap[0:O_WPH].bitcast(I8).rearrange("(p n) -> p n",
                                                  p=W_SH_ROWS)
    wph_sh_ap = (wbap[O_WPH:O_BIAS].bitcast(I8)
                 .rearrange("(p n) -> p n", p=WPH_SH_ROWS))
    bias_ap = wbap[O_BIAS:O_WSC].bitcast(F32)
    wsc_ap = wbap[O_WSC:O_WPSC].bitcast(F32)
    wpsc_ap = (wbap[O_WPSC:WBLOB_BYTES].bitcast(F32)
               .rearrange("(h p) -> p h", p=96))
    wqsc_ap = wsc_ap[0:N_KP].rearrange("(m p) -> p m", p=128)
    wksc_ap = wsc_ap[N_KP:N_KP + N_OUT].rearrange("(e p) -> p e", p=128)
    wvsc_ap = (wsc_ap[N_KP + N_OUT:N_WSC]
               .rearrange("(e p) -> p e", p=128))
    bq_ap = bias_ap[0:N_KP].rearrange("(m p) -> p m", p=128)
    bk_ap = bias_ap[N_KP:2 * N_KP].rearrange("(m p) -> p m", p=128)
    bv_ap = bias_ap[2 * N_KP:2 * N_KP + N_VA].rearrange("(o n) -> o n", o=1)
    bp_ap = (bias_ap[2 * N_KP + N_VA:N_BIAS]
             .rearrange("(m p) -> p m", p=128))

    # gather staging + outputs (collectives cannot read IO tensors)
    xs_stage = nc.dram_tensor("xs_stage", [XS_SH_ROWS, 2048], I8)
    w_stage = nc.dram_tensor("w_stage", [W_SH_ROWS, 2048], I8)
    wph_stage = nc.dram_tensor("wph_stage", [WPH_SH_ROWS, N_OUT], I8)
    xs_full = nc.dram_tensor("xs_full", [N_OUT, 2048], I8,
                             addr_space="Shared")
    w_full = nc.dram_tensor("w_full", [W_ROWS, 2048], I8, addr_space="Shared")
    wph_full = nc.dram_tensor("wph_full", [N_OUT, N_OUT], I8,
                              addr_space="Shared")

    with TileContext(nc) as tc, tc.tile_pool(name="resident", bufs=1) as pr:
        # ---- resident tiles ----
        kpad = pr.tile([128, 3, L], BF16)        # k^T head-padded (32 rows/head)
        qpad = pr.tile([128, 3, 2 * CH], BF16)
        v_t = pr.tile([128, L // 128, N_VA], BF16)   # augmented v, token-major
        mC_t = pr.tile([128, 8, 2 * CH], BF16)
        mD_t = pr.tile([128, 8, CH], BF16)
        wph_t = pr.tile([96, 12, N_OUT], BF16)
        bp_t = pr.tile([128, 9], F32)
        id_t = pr.tile([128, 128], F32)      # identity for PE transposes
        yts = [pr.tile([HD_V, 2 * CH], BF16, name=f"yt{h}", tag=f"yt{h}")
               for h in range(N_HEAD)]

        with (
            tc.tile_pool(name="loads", bufs=1) as pw,
            tc.tile_pool(name="xsp", bufs=1) as pxs,
            tc.tile_pool(name="xstage", bufs=2) as pst,
            tc.tile_pool(name="scratch", bufs=1) as psc,
            tc.tile_pool(name="ps_small", bufs=2, space="PSUM") as psp,
            tc.tile_pool(name="ps_v", bufs=2, space="PSUM") as psv,
        ):
            # ---- stage shards, all-gather on device ----
            pid = nc.partition_id()
            nc.sync.dma_start(out=xs_stage.ap(), in_=xs_sh_ap)
            nc.sync.dma_start(out=w_stage.ap(), in_=w_sh_ap)
            nc.sync.dma_start(out=wph_stage.ap(), in_=wph_sh_ap)
            nc.gpsimd.collective_compute(
                "AllGather", mybir.AluOpType.bypass,
                replica_groups=[[0, 1, 2, 3, 4, 5, 6, 7]],
                ins=[xs_stage.ap()], outs=[xs_full.ap()],
            )
            nc.gpsimd.collective_compute(
                "AllGather", mybir.AluOpType.bypass,
                replica_groups=[[0, 1, 2, 3, 4, 5, 6, 7]],
                ins=[w_stage.ap()], outs=[w_full.ap()],
            )
            nc.gpsimd.collective_compute(
                "AllGather", mybir.AluOpType.bypass,
                replica_groups=[[0, 1, 2, 3, 4, 5, 6, 7]],
                ins=[wph_stage.ap()], outs=[wph_full.ap()],
            )

            # ---- SBUF loads; xs dequantized per 128-channel slab ----
            xs_t = pxs.tile([128, 9, L], BF16)
            xsc_t = pw.tile([128, 9], F32)
            nc.sync.dma_start(out=xsc_t, in_=xsc_ap)
            scv0 = pxs.tile([1, 16], F32, tag="scv0")
            nc.vector.tensor_copy(scv0[0:1, 0:1], xsc_t[0:1, 0:1])  # pre-touch
            xsf_r = xs_full.ap().rearrange("(e p) n -> p e n", p=128)
            for e in range(9):
                st8 = pst.tile([128, L], I8, tag="st8")
                nc.sync.dma_start(out=st8, in_=xsf_r[:, e, :])
                nc.vector.tensor_scalar(xs_t[:, e, :], st8,
                                        xsc_t[:, e:e + 1], None,
                                        mybir.AluOpType.mult)
            wfl = w_full.ap().flatten()
            wqsc_t = pw.tile([128, 3], F32, tag="wqsc")
            nc.sync.dma_start(out=wqsc_t, in_=wqsc_ap)
            wksc_t = pw.tile([128, 9], F32, tag="wksc")
            nc.sync.dma_start(out=wksc_t, in_=wksc_ap)
            wvsc_t = pw.tile([128, 9], F32, tag="wvsc")
            nc.sync.dma_start(out=wvsc_t, in_=wvsc_ap)
            wq_t = pw.tile([128, 3, N_KP], BF16)
            wq8_t = pw.tile([128, 3, N_KP], I8, tag="wq8")
            nc.sync.dma_start(
                out=wq8_t,
                in_=wfl[0:WQ_ELS].rearrange("(m p n) -> p m n", m=3, p=128))
            for m in range(3):
                nc.vector.tensor_scalar(wq_t[:, m, :], wq8_t[:, m, :],
                                        wqsc_t[:, m:m + 1], None,
                                        mybir.AluOpType.mult)
            wk_t = pw.tile([128, 9, N_KP], BF16)
            wv_t = pw.tile([128, 9, N_VA], BF16)
            for e in range(9):
                st = pst.tile([128, N_KP], I8, tag="wk8")
                nc.sync.dma_start(
                    out=st,
                    in_=wfl[WQ_ELS + e * 128 * N_KP:
                            WQ_ELS + (e + 1) * 128 * N_KP]
                    .rearrange("(p n) -> p n", p=128))
                nc.vector.tensor_scalar(wk_t[:, e, :], st,
                                        wksc_t[:, e:e + 1], None,
                                        mybir.AluOpType.mult)
            for e in range(9):
                st = pst.tile([128, N_VA], I8, tag="wv8")
                nc.sync.dma_start(
                    out=st,
                    in_=wfl[WQ_ELS + WK_ELS + e * 128 * N_VA:
                            WQ_ELS + WK_ELS + (e + 1) * 128 * N_VA]
                    .rearrange("(p n) -> p n", p=128))
                nc.vector.tensor_scalar(wv_t[:, e, :], st,
                                        wvsc_t[:, e:e + 1], None,
                                        mybir.AluOpType.mult)
            wpsc_t = pw.tile([96, 12], F32, tag="wpsc")
            nc.sync.dma_start(out=wpsc_t, in_=wpsc_ap)
            wphf_r = wph_full.ap().rearrange("(h p) n -> p h n", p=96)
            for h in range(N_HEAD):
                st = pst.tile([96, N_OUT], I8, tag="wph8")
                nc.sync.dma_start(out=st, in_=wphf_r[:, h, :])
                nc.vector.tensor_scalar(wph_t[:, h, :], st,
                                        wpsc_t[:, h:h + 1], None,
                                        mybir.AluOpType.mult)
            qb_t = pw.tile([128, 2 * CH], F32)
            nc.sync.dma_start(out=qb_t, in_=qoff_ap.to_broadcast([128, 2 * CH]))
            bq_t = pw.tile([128, 3], F32)
            nc.sync.dma_start(out=bq_t, in_=bq_ap)
            bk_t = pw.tile([128, 3], F32)
            nc.sync.dma_start(out=bk_t, in_=bk_ap)
            bv_t = pw.tile([128, N_VA], F32)
            nc.sync.dma_start(out=bv_t, in_=bv_ap.to_broadcast([128, N_VA]))
            nc.sync.dma_start(out=bp_t, in_=bp_ap)

            # ---- pre-touches: give each engine 1-wait visibility of loads ----
            dps = psp.tile([128, 512], F32, tag="ps")
            for i, t in enumerate(
                [xs_t[0:1, 0, 0:1], wq_t[0:1, 0, 0:1],
                 wk_t[0:1, 0, 0:1], wv_t[0:1, 0, 0:1], wph_t[0:1, 0, 0:1]]
            ):
                nc.tensor.matmul(dps[0:1, i:i + 1], lhsT=t, rhs=t,
                                 start=True, stop=True)
            sc = psc.tile([1, 16], F32)
            nc.scalar.activation(sc[0:1, 0:1], bq_t[0:1, 0:1], AF.Copy)
            nc.scalar.activation(sc[0:1, 1:2], bk_t[0:1, 0:1], AF.Copy)
            nc.scalar.activation(sc[0:1, 2:3], bp_t[0:1, 0:1], AF.Copy)
            scv = psc.tile([1, 16], F32, tag="scv")
            nc.vector.tensor_copy(scv[0:1, 0:1], bv_t[0:1, 0:1])
            nc.vector.tensor_copy(scv[0:1, 1:2], qb_t[0:1, 0:1])
            # ACT warm-up of Exp's implicit const-bias AP
            sce = psc.tile([1, 16], F32, tag="sce")
            nc.scalar.activation(sce[0:1, 0:1], scv[0:1, 0:1], AF.Exp)

            # ---- mask gen: m[p, f] = (qidx[f] - (128*kt + p) > 0) ----
            ci_t = pw.tile([128, 2 * CH], mybir.dt.int32)
            nc.gpsimd.iota(ci_t, pattern=[[1, 2 * CH]], base=0,
                           channel_multiplier=-1)
            cif_t = pw.tile([128, 2 * CH], F32)
            nc.vector.tensor_copy(cif_t, ci_t)
            mb_t = pw.tile([128, 2 * CH], F32)
            nc.vector.tensor_add(mb_t, cif_t, qb_t)
            for kt in range(8):
                nc.vector.tensor_scalar(
                    mC_t[:, kt, :], mb_t, float(128 * kt), None,
                    mybir.AluOpType.is_gt)
            for kt in range(8, 16):
                nc.vector.tensor_scalar(
                    mD_t[:, kt - 8, :], mb_t[:, CH:], float(128 * kt), None,
                    mybir.AluOpType.is_gt)
            nc.vector.tensor_scalar(id_t, cif_t[:, :128], 0.0, None,
                                    mybir.AluOpType.is_equal)

            # ---- q projection: qpad[384, 256]; q-token columns are sliced
            #      out of the gathered xs at runtime via partition-id ----
            offA = nc.s_assert_within(pid * CH, 0, L,
                                      skip_runtime_assert=True)
            offB = nc.s_assert_within((15 - pid) * CH, 0, L,
                                      skip_runtime_assert=True)
            sq_t = pw.tile([128, 3, 2 * CH], BF16)
            for e in range(3):
                nc.vector.tensor_copy(sq_t[:, e, :CH],
                                      xs_t[:, 6 + e, bass.ds(offA, CH)])
                nc.vector.tensor_copy(sq_t[:, e, CH:],
                                      xs_t[:, 6 + e, bass.ds(offB, CH)])
            for m in range(3):
                ps = psp.tile([128, 2 * CH], F32, tag="ps")
                for e in range(3):
                    nc.tensor.matmul(
                        ps, lhsT=wq_t[:, e, m * 128:(m + 1) * 128],
                        rhs=sq_t[:, e, :],
                        start=(e == 0), stop=(e == 2),
                    )
                nc.scalar.activation(qpad[:, m, :], ps, AF.Identity,
                                     bias=bq_t[:, m:m + 1])

            # ---- k projection: kpad[384, 2048], 512-token slabs ----
            for m in range(3):
                for nt in range(L // 512):
                    ps = psp.tile([128, 512], F32, tag="ps")
                    for e in range(9):
                        nc.tensor.matmul(
                            ps,
                            lhsT=wk_t[:, e, m * 128:(m + 1) * 128],
                            rhs=xs_t[:, e, nt * 512:(nt + 1) * 512],
                            start=(e == 0), stop=(e == 8),
                        )
                    nc.scalar.activation(
                        kpad[:, m, nt * 512:(nt + 1) * 512], ps, AF.Identity,
                        bias=bk_t[:, m:m + 1],
                    )

            # ---- v projection: v[2048, 1164] (token-major, augmented) ----
            for c in range(L // 128):
                ps = psv.tile([128, N_VA], F32, tag="vps")
                for e in range(9):
                    for n0, nn in [(0, 512), (512, 512), (1024, N_VA - 1024)]:
                        nc.tensor.matmul(
                            ps[:, n0:n0 + nn],
                            lhsT=xs_t[:, e, c * 128:(c + 1) * 128],
                            rhs=wv_t[:, e, n0:n0 + nn],
                            start=(e == 0), stop=(e == 8),
                        )
                nc.vector.tensor_add(v_t[:, c, :], ps, bv_t)

        # ---- attention ----
        with (
            tc.tile_pool(name="ps_s", bufs=4, space="PSUM") as pss,
            tc.tile_pool(name="ps_y", bufs=3, space="PSUM") as psy,
            tc.tile_pool(name="exps", bufs=20) as pe,
            tc.tile_pool(name="norm", bufs=2) as pn,
            tc.tile_pool(name="rdram", bufs=6, space="DRAM") as pdram,
        ):
            for h in range(N_HEAD):
                t, a = h // 4, 32 * (h % 4)
                ems = []
                for kt in range(8):
                    s_ps = pss.tile([128, 2 * CH], F32, tag="sps")
                    nc.tensor.matmul(
                        s_ps,
                        lhsT=kpad[a:a + HD_K, t, kt * 128:(kt + 1) * 128],
                        rhs=qpad[a:a + HD_K, t, :],
                        start=True, stop=True,
                        tile_position=(a, 0),
                    )
                    e_sb = pe.tile([128, 2 * CH], BF16, tag="esb")
                    nc.scalar.activation(e_sb, s_ps, AF.Exp, scale=0.25)
                    em_sb = pe.tile([128, 2 * CH], BF16, tag="emsb")
                    nc.vector.tensor_mul(em_sb, e_sb, mC_t[:, kt, :])
                    ems.append(em_sb)
                for kt in range(8, 16):
                    s_ps = pss.tile([128, 2 * CH], F32, tag="sps")
                    nc.tensor.matmul(
                        s_ps[:, :CH],
                        lhsT=kpad[a:a + HD_K, t, kt * 128:(kt + 1) * 128],
                        rhs=qpad[a:a + HD_K, t, CH:],
                        start=True, stop=True,
                        tile_position=(a, 0),
                    )
                    e_sb = pe.tile([128, 2 * CH], BF16, tag="esb")
                    nc.scalar.activation(e_sb[:, :CH], s_ps[:, :CH], AF.Exp,
                                         scale=0.25)
                    em_sb = pe.tile([128, 2 * CH], BF16, tag="emsb")
                    nc.vector.tensor_mul(em_sb[:, :CH], e_sb[:, :CH],
                                         mD_t[:, kt - 8, :])
                    ems.append(em_sb)
                y_ps = psy.tile([HD_VA, 2 * CH], F32, tag="yps")
                for kt in range(8):
                    nc.tensor.matmul(
                        y_ps,
                        lhsT=v_t[:, kt, h * HD_VA:(h + 1) * HD_VA],
                        rhs=ems[kt],
                        start=(kt == 0), stop=False,
                    )
                for kt in range(8, 16):
                    nc.tensor.matmul(
                        y_ps[:, CH:],
                        lhsT=v_t[:, kt, h * HD_VA:(h + 1) * HD_VA],
                        rhs=ems[kt][:, :CH],
                        start=False, stop=(kt == 15),
                    )
                # normalize: row 96 of y_ps is the softmax denominator
                # (clamped away from 0 so the dead q=0 column yields 0, not NaN)
                r_sb = pn.tile([128, 2 * CH], F32, tag="rsb")
                rmx = pn.tile([128, 2 * CH], F32, tag="rmx")
                nc.vector.tensor_scalar_max(rmx[96:97, :], y_ps[96:97, :],
                                            1e-30)
                nc.vector.reciprocal(r_sb[96:97, :], rmx[96:97, :])
                rd = pdram.tile([1, 2 * CH], F32, tag="rd")
                nc.sync.dma_start(out=rd, in_=r_sb[96:97, :])
                rb_t = pn.tile([HD_V, 2 * CH], F32, tag="rbt")
                nc.sync.dma_start(
                    out=rb_t, in_=rd[0:1, :].to_broadcast([HD_V, 2 * CH])
                )
                rtc = pn.tile([1, 1], F32, tag="rtc")
                nc.vector.tensor_copy(rtc, rb_t[0:1, 0:1])  # pre-touch
                nc.vector.tensor_mul(yts[h], y_ps[:HD_V, :], rb_t)

        # ---- output projection: outT[1152, 256] = sum_h Wp_h^T @ y_h,
        #      then per-token int8 quantization: transpose, abs-max, scale ----
        with (
            tc.tile_pool(name="ps_o", bufs=2, space="PSUM") as pso,
            tc.tile_pool(name="ps_q", bufs=2, space="PSUM") as psq,
            tc.tile_pool(name="qsb", bufs=3) as pq,
            tc.tile_pool(name="qsc", bufs=1) as pqs,
        ):
            outb = pqs.tile([128, 9, 2 * CH], F32, tag="outb")
            for mo in range(9):
                ps = pso.tile([128, 2 * CH], F32)
                for h in range(N_HEAD):
                    nc.tensor.matmul(
                        ps,
                        lhsT=wph_t[:, h, mo * 128:(mo + 1) * 128],
                        rhs=yts[h],
                        start=(h == 0), stop=(h == N_HEAD - 1),
                    )
                nc.scalar.activation(outb[:, mo, :], ps, AF.Identity,
                                     bias=bp_t[:, mo:mo + 1])
            sc_all = pqs.tile([128, 2], F32)
            rcp = pqs.tile([128, 2], F32, tag="rcp")
            mxs = pqs.tile([128, 2], F32, tag="mxs")
            for tcn in range(2):
                psT = psq.tile([128, N_OUT], F32, tag="psT")
                for mo in range(9):
                    nc.tensor.matmul(
                        psT[:, mo * 128:(mo + 1) * 128],
                        lhsT=outb[:, mo, tcn * 128:(tcn + 1) * 128],
                        rhs=id_t, is_transpose=True,
                        start=True, stop=True,
                    )
                nc.vector.tensor_reduce(
                    mxs[:, tcn:tcn + 1], psT, axis=mybir.AxisListType.X,
                    op=mybir.AluOpType.max, apply_absolute_value=True)
                nc.vector.tensor_scalar_mul(sc_all[:, tcn:tcn + 1],
                                            mxs[:, tcn:tcn + 1], 1.0 / 127.0)
                nc.vector.reciprocal(rcp[:, tcn:tcn + 1],
                                     sc_all[:, tcn:tcn + 1])
                qf = pq.tile([128, N_OUT], F32, tag="qf")
                nc.vector.tensor_scalar(qf, psT, rcp[:, tcn:tcn + 1], MAGIC,
                                        mybir.AluOpType.mult,
                                        mybir.AluOpType.add)
                qg = pq.tile([128, N_OUT], F32, tag="qg")
                nc.vector.tensor_scalar(qg, qf, MAGIC, None,
                                        mybir.AluOpType.subtract)
                qi = pq.tile([128, N_OUT], mybir.dt.int8, tag="qi")
                nc.vector.tensor_copy(qi, qg)
                nc.sync.dma_start(
                    out=out_d.ap()[tcn * 128:(tcn + 1) * 128, :], in_=qi)
            sc_dst = (out_d.ap()[2 * CH:OUT_ROWS, :].flatten()[0:2 * CH * 4]
                      .bitcast(F32).rearrange("(p n) -> p n", p=128))
            nc.sync.dma_start(out=sc_dst, in_=sc_all)
    return nc


def _legalize_waits(nc):
    """This walrus build accepts only ONE sync-wait per regular instruction;
    move overflow waits onto injected same-engine NoOps (like raw-bass
    wait_ge)."""
    keep = ("InstEventSemaphore",)
    cnt = 0
    for bbh in nc.bb_map.values():
        bb = bbh.bb
        new_list = []
        for inst in bb.instructions:
            si = inst.sync_info
            if (si is not None and len(si.on_wait) > 1
                    and type(inst).__name__ not in keep):
                waits = list(si.on_wait)
                for w in waits[:-1]:
                    cnt += 1
                    n = mybir.InstNoOp(name=f"legwait_{cnt}", ins=[], outs=[])
                    n.engine = inst.engine
                    n.sync_info = mybir.SyncInfo(on_wait=[w], on_update=[])
                    try:
                        nc.register_instruction(n)
                    except Exception:
                        pass
                    new_list.append(n)
                inst.sync_info = mybir.SyncInfo(
                    on_wait=[waits[-1]], on_update=list(si.on_update))
            new_list.append(inst)
        bb.instructions = new_list
    return cnt


class _Runner:
    """Cached dispatch path for the 2-call per-batch pipeline.

    Replicates run_bass_via_pjrt's lowering (same _bass_exec_p custom
    call), but keeps the jitted executable, recycles donated output
    buffers device-side instead of uploading fresh host zeros each call,
    uploads the weight blob once per kernel call shared by both
    dispatches, and overlaps batch 0's result fetch with batch 1's
    upload + execute.
    """

    def __init__(self, nc):
        install_neuronx_cc_hook()
        self.nc = nc
        partition_name = (nc.partition_id_tensor.name
                          if nc.partition_id_tensor else None)
        in_names, out_names, out_avals = [], [], []
        for alloc in nc.m.functions[0].allocations:
            if not isinstance(alloc, mybir.MemoryLocationSet):
                continue
            name = alloc.memorylocations[0].name
            if alloc.kind == "ExternalInput":
                if name != partition_name:
                    in_names.append(name)
            elif alloc.kind == "ExternalOutput":
                out_names.append(name)
                out_avals.append(jax.core.ShapedArray(
                    tuple(alloc.tensor_shape), mybir.dt.np(alloc.dtype)))
        n_params, n_outs = len(in_names), len(out_avals)
        assert in_names == ["xblob", "wblob"] and out_names == ["out"]
        in_names_all = in_names + out_names + (
            [partition_name] if partition_name else [])

        def _body(*args):
            operands = list(args)
            if partition_name is not None:
                operands.append(partition_id_tensor())
            return tuple(_bass_exec_p.bind(
                *operands, out_avals=tuple(out_avals),
                in_names=tuple(in_names_all), out_names=tuple(out_names),
                lowering_input_output_aliases=(), sim_require_finite=True,
                sim_require_nnan=True, nc=nc))

        devices = jax.devices()[:8]
        assert len(devices) == 8
        self.mesh = Mesh(np.asarray(devices), ("core",))
        self.sharding = NamedSharding(self.mesh, PartitionSpec("core"))
        donate = tuple(range(n_params, n_params + n_outs))
        self.sharded = jax.jit(
            shard_map(_body, mesh=self.mesh,
                      in_specs=(PartitionSpec("core"),) * (n_params + n_outs),
                      out_specs=(PartitionSpec("core"),) * n_outs,
                      check_rep=False),
            donate_argnums=donate, keep_unused=True)
        self.zmaker = jax.jit(
            lambda: jnp.zeros((8 * OUT_ROWS, N_OUT), jnp.int8),
            out_shardings=self.sharding)
        self.spares = None

    def warm(self):
        """Seed the donated-buffer pool on device (no host traffic)."""
        if self.spares is None:
            s0, s1 = self.zmaker(), self.zmaker()
            jax.block_until_ready((s0, s1))
            self.spares = (s0, s1)

    def __call__(self, xg0, xg1, wg):
        """Full device computation: returns the two batches' raw outputs
        as np.int8 [8*OUT_ROWS, N_OUT] each."""
        self.warm()
        s0, s1 = self.spares
        self.spares = None
        wdev = jax.device_put(wg, self.sharding)
        (o0,) = self.sharded(xg0, wdev, s0)
        (o1,) = self.sharded(xg1, wdev, s1)
        o0.copy_to_host_async()
        o1.copy_to_host_async()
        r0 = np.asarray(o0)
        r1 = np.asarray(o1)
        self.spares = (o0, o1)  # recycle device buffers for next donation
        return r0, r1


def _get_runner():
    global _RUNNER
    if _RUNNER is None:
        nc = _build_graph()
        _legalize_waits(nc)
        # The pjrt lowering re-serializes the (frozen, never-mutated) graph
        # on every trace (~25ms for this BIR); memoize the identical bytes.
        raw = nc.to_json_bytes()
        nc.to_json_bytes = lambda: raw
        _RUNNER = _Runner(nc)
    return _RUNNER


def _head_pad_kq(W, b):
    """[in, 192] -> [in, 384] with head h cols at 128*(h//4)+32*(h%4)."""
    Wp = np.zeros((W.shape[0], N_KP), np.float32)
    bp = np.zeros((N_KP,), np.float32)
    for h in range(N_HEAD):
        c = 128 * (h // 4) + 32 * (h % 4)
        Wp[:, c:c + HD_K] = W[:, h * HD_K:(h + 1) * HD_K]
        bp[c:c + HD_K] = b[h * HD_K:(h + 1) * HD_K]
    return Wp, bp


def _prep_inputs(x, side, Wq, bq, Wkv, bkv, Wproj, bproj):
    """Quantize + pack the wire blobs: returns (xg0, xg1, wg) global
    uint8 arrays (concat of the 8 per-core shards along axis 0)."""
    Wk = Wkv[:, :N_KQ]
    Wv = Wkv[:, N_KQ:]
    bk = bkv[:N_KQ]
    bv = bkv[N_KQ:]
    Wq_p, bq_p = _head_pad_kq(Wq, bq)
    Wk_p, bk_p = _head_pad_kq(Wk, bk)
    # augmented V: per head 96 channels + a zero-weight/one-bias denom channel
    Wv_a = np.zeros((N_OUT, N_VA), np.float32)
    bv_a = np.zeros((N_VA,), np.float32)
    for h in range(N_HEAD):
        Wv_a[:, h * HD_VA:h * HD_VA + HD_V] = Wv[:, h * HD_V:(h + 1) * HD_V]
        bv_a[h * HD_VA:h * HD_VA + HD_V] = bv[h * HD_V:(h + 1) * HD_V]
        bv_a[h * HD_VA + HD_V] = 1.0

    # packed int8 q/k/v weights (per input-channel-row scales), [944, 2048]
    def q8_rows(W):
        sc = np.maximum(np.abs(W).max(axis=1), 1e-30) / 127.0
        q = np.clip(np.round(W / sc[:, None]), -127, 127).astype(np.int8)
        return q, sc.astype(np.float32)

    wq8, wqsc = q8_rows(Wq_p)
    wk8, wksc = q8_rows(Wk_p)
    wv8, wvsc = q8_rows(Wv_a)
    wpack = np.zeros((W_ROWS * 2048,), np.int8)
    wpack[0:WQ_ELS] = wq8.ravel()
    wpack[WQ_ELS:WQ_ELS + WK_ELS] = wk8.ravel()
    wpack[WQ_ELS + WK_ELS:WQ_ELS + WK_ELS + WV_ELS] = wv8.ravel()
    wpack = wpack.reshape(W_ROWS, 2048)
    wscales = np.concatenate([wqsc, wksc, wvsc]).astype(np.float32)

    # Wproj rows per head, int8 per-row [1152, 1152]
    wph_all, wphsc = q8_rows(Wproj.reshape(N_HEAD * HD_V, N_OUT))

    biases = np.concatenate([bq_p, bk_p, bv_a, bproj]).astype(np.float32)

    wg = np.empty((8, WBLOB_BYTES), np.uint8)
    for i in range(8):
        wg[i, 0:O_WPH] = (wpack[W_SH_ROWS * i:W_SH_ROWS * (i + 1), :]
                          .reshape(-1).view(np.uint8))
        wg[i, O_WPH:O_BIAS] = (
            wph_all[WPH_SH_ROWS * i:WPH_SH_ROWS * (i + 1), :]
            .reshape(-1).view(np.uint8))
        wg[i, O_BIAS:O_WSC] = biases.view(np.uint8)
        wg[i, O_WSC:O_WPSC] = wscales.view(np.uint8)
        wg[i, O_WPSC:WBLOB_BYTES] = wphsc.view(np.uint8)

    # per-channel int8 scales for [x|side]^T, shared by all cores of a batch
    xgs = []
    for b in range(B):
        xsT = np.ascontiguousarray(np.concatenate([x[b], side[b]], axis=1).T)
        xsc = np.maximum(np.abs(xsT).max(axis=1), 1e-30) / 127.0
        xsq = np.clip(np.round(xsT / xsc[:, None]), -127, 127).astype(np.int8)
        xscf = xsc.astype(np.float32)
        xg = np.empty((8, XBLOB_BYTES), np.uint8)
        for i in range(8):
            # qidx[f] - f for the mask generator: q token of em column f
            qoff = np.empty((2 * CH,), np.float32)
            qoff[:CH] = CH * i
            qoff[CH:] = CH * (15 - i) - CH
            xg[i, 0:O_QOFF] = (xsq[XS_SH_ROWS * i:XS_SH_ROWS * (i + 1), :]
                               .reshape(-1).view(np.uint8))
            xg[i, O_QOFF:O_XSC] = qoff.view(np.uint8)
            xg[i, O_XSC:XBLOB_BYTES] = xscf.view(np.uint8)
        xgs.append(xg.reshape(-1))
    return xgs[0], xgs[1], wg.reshape(-1)


def _unpack(raw, ans, b):
    """raw: [8*OUT_ROWS, N_OUT] int8 for batch b -> ans[b] float32."""
    for i in range(8):
        core = raw[OUT_ROWS * i:OUT_ROWS * (i + 1)]
        scales = (core[2 * CH:].reshape(-1).view(np.float32)[:2 * CH]
                  .reshape(128, 2))              # [partition, chunk]
        vals = core[:2 * CH].astype(np.float32)  # [256 tokens, 1152]
        for tcn in range(2):
            vals[tcn * 128:(tcn + 1) * 128] *= scales[:, tcn:tcn + 1]
        ans[b, CH * i:CH * (i + 1)] = vals[:CH]
        ans[b, CH * (15 - i):CH * (16 - i)] = vals[CH:]


def kernel(x, side, Wq, bq, Wkv, bkv, Wproj, bproj, Wemb, bemb, **_unused):
    x = np.asarray(x, np.float32)
    side = np.asarray(side, np.float32)
    Wq = np.asarray(Wq, np.float32)
    bq = np.asarray(bq, np.float32)
    Wkv = np.asarray(Wkv, np.float32)
    bkv = np.asarray(bkv, np.float32)
    Wproj = np.asarray(Wproj, np.float32)
    bproj = np.asarray(bproj, np.float32)
    Wemb = np.asarray(Wemb, np.float32)
    bemb = np.asarray(bemb, np.float32)

    runner = _get_runner()
    xg0, xg1, wg = _prep_inputs(x, side, Wq, bq, Wkv, bkv, Wproj, bproj)
    r0, r1 = runner(xg0, xg1, wg)

    ans = np.empty((B, L, N_OUT), np.float32)
    _unpack(r0, ans, 0)
    _unpack(r1, ans, 1)
    # first token: replaced by learned embedding of side[:, 0] (exact, host-side)
    for b in range(B):
        first = side[b, 0].astype(np.float64) @ Wemb.astype(np.float64) + bemb
        ans[b, 0] = (first @ Wproj.astype(np.float64) + bproj).astype(np.float32)
    return ans


# revision 3
# speedup vs baseline: 1.3993x; 1.0843x over previous
"""Trainium2 Bass kernel: AutoregressiveSelfAttention (sparse_attention).

Sharding: 8 cores, token-parallel with zigzag causal load balancing.
  core i -> batch b = i//4, j = i%4, query chunks cA = j, cB = 7-j (256 tokens each).
  Each core computes the full per-batch KV locally, runs attention for its 512
  query tokens, and the output projection for them. Host reassembles the 8
  disjoint output slices.

Wire format (dominates wall time through the ~45MB/s shared-bandwidth axon
tunnel; pipelined multi-dispatch variants measured SLOWER than one dispatch
since up/down mostly share the link): ONE u8 blob input per core + ONE int8
output. x/side and ALL weights (q/k/v and proj) travel int8 with per-channel
f32 scales, dequantized to bf16 on device (int8+scale beats fp8 e4m3 ~3x in
accuracy; fp8 wire fails the 2e-2 gate); biases f32. Wq/Wk travel UNPADDED
(192 cols) and are scattered on device into the head-padded SBUF layout;
biases and all weight scales ride inside the sharded+gathered weight blob
instead of being replicated 8x. The x/side shards (1/4 per core, both
batches) and weight shards (1/8) are AllGathered on device over 8-core
groups with Shared outputs (the 4-group collective's forced-Local path is
~4x slower), so replicated bytes never cross the host link; each core
selects its batch half of xs with one partition-id-offset DMA. Causal masks
are generated on device (iota + a per-core q-offset row), the q token
columns are sliced out of the gathered xs at runtime via partition-id, and
the output is quantized per token to int8 (PE transpose + abs-max), with
the f32 scales embedded in the output tensor's tail rows.

Dispatch path: the stock run_bass_kernel_spmd re-jits a fresh closure every
call and uploads fresh host np.zeros for the donated output buffers (output
bytes counted twice over the wire). The _Runner here replicates its
_bass_exec_p lowering with a cached jit, recycles the previous call's
device-resident output array as the next call's donated buffer (the kernel
writes every output byte the host reads, so zero-fill is unneeded), and
overlaps the D2H result fetch with tail work via copy_to_host_async.

Device layouts (per core):
  scores as sT[kv, q] (kv on partitions) so softmax needs no transpose; the
  denominator is folded into the AV matmul via an augmented V (97th channel);
  exp needs no max-subtraction (scores are O(1)).
  k^T/q^T are head-padded to 32-row strips so score matmuls address them in
  place via tile_position; the pad columns of the scattered weight tiles are
  uninitialized garbage, which only ever flows into kpad/qpad pad rows that
  no score matmul reads. Compute instructions here may carry only ONE
  semaphore wait, so DMA-loaded tiles get same-engine pre-touches before
  their consumers (with _legalize_waits as the generic backstop).
"""

import sys

sys.path.insert(0, "/opt/trn_rl_repo")

import numpy as np
import ml_dtypes

import jax
import jax.numpy as jnp
from jax.sharding import Mesh, PartitionSpec, NamedSharding
from jax.experimental.shard_map import shard_map

# Persistent XLA compilation cache: without it the whole BIR->NEFF pipeline
# reruns per process (~minutes); with it, repeat dispatches deserialize the
# cached executable.
jax.config.update("jax_compilation_cache_dir", "/tmp/jax_cc_cache")
jax.config.update("jax_persistent_cache_min_compile_time_secs", 0)
jax.config.update("jax_persistent_cache_min_entry_size_bytes", 0)

import concourse.bass as bass
import concourse.mybir as mybir
from concourse.tile import TileContext
from concourse.bass2jax import (
    _bass_exec_p,
    install_neuronx_cc_hook,
    partition_id_tensor,
)

BF16 = mybir.dt.bfloat16
F32 = mybir.dt.float32
NP_BF16 = ml_dtypes.bfloat16
I8 = mybir.dt.int8
AF = mybir.ActivationFunctionType

N_HEAD = 12
N_KQ = 192
N_OUT = 1152
HD_K = 16
HD_V = 96
HD_VA = 97             # v head channels + denominator column
N_VA = N_HEAD * HD_VA  # 1164
N_KP = N_HEAD * 32     # 384: head-padded k/q channel count
B, L = 2, 2048
CH = 256

# ---- packed weight image (flat bytes, sharded 1/8 per core, gathered) ----
WQ_ELS = N_KQ * N_KQ * 2        # 73728: unpadded wq is [384, 192]
WK_ELS = N_OUT * N_KQ           # 221184: unpadded wk is [1152, 192]
WV_ELS = N_OUT * N_VA           # 1340928
O_FBIAS = WQ_ELS + WK_ELS + WV_ELS              # 1635840
N_BIAS = N_KP + N_KP + N_VA + N_OUT             # 3084 f32
O_FWSC = O_FBIAS + N_BIAS * 4                   # 1648176
N_WSC = N_KP + N_OUT + N_OUT                    # 2688 f32 w row scales
O_FWPSC = O_FWSC + N_WSC * 4                    # 1658928
W_BYTES_USED = O_FWPSC + N_OUT * 4              # 1663536
W_ROWS = 816                    # 1671168 bytes: W_BYTES_USED padded to 8*2048
W_SH_ROWS = W_ROWS // 8         # 102
WPH_SH_ROWS = N_OUT // 8        # 144

# ---- blob layout (byte offsets) ----
XS_SH_ROWS = N_OUT // 4          # 288 rows of xsT per core (4-way gather)
O_XS = 0
O_W = O_XS + XS_SH_ROWS * 2048                  # 589824 (int8 xs shard)
O_WPH = O_W + W_SH_ROWS * 2048                  # 798720 (int8 w shard)
O_QOFF = O_WPH + WPH_SH_ROWS * N_OUT            # 964608 (int8 wph shard)
O_XSC = O_QOFF + 2 * CH * 4                     # 966656
BLOB_BYTES = O_XSC + N_OUT * 4                  # 971264

MAGIC = 12582912.0  # 1.5 * 2**23: f32 add/sub rounds to nearest integer
OUT_ROWS = 2 * CH + 2  # 512 token rows int8 + 2 rows carrying 512 f32 scales

_RUNNER = None

# padded column offset of head h inside the 384-col head-padded layout
_PC = [128 * (h // 4) + 32 * (h % 4) for h in range(N_HEAD)]


def _build_graph():
    nc = bass.Bass(num_devices=8)
    blob = nc.declare_dram_parameter("blob", [BLOB_BYTES], mybir.dt.uint8,
                                     isOutput=False)
    out_d = nc.declare_dram_parameter("out", [OUT_ROWS, N_OUT], mybir.dt.int8,
                                      isOutput=True)

    bap = blob.ap()
    xs_sh_ap = bap[O_XS:O_W].bitcast(I8).rearrange("(p n) -> p n",
                                                   p=XS_SH_ROWS)
    w_sh_ap = bap[O_W:O_WPH].bitcast(I8).rearrange("(p n) -> p n",
                                                   p=W_SH_ROWS)
    wph_sh_ap = (bap[O_WPH:O_QOFF].bitcast(I8)
                 .rearrange("(p n) -> p n", p=WPH_SH_ROWS))
    qoff_ap = (bap[O_QOFF:O_XSC].bitcast(F32)
               .rearrange("(o n) -> o n", o=1))
    xsc_ap = (bap[O_XSC:BLOB_BYTES].bitcast(F32)
              .rearrange("(e p) -> p e", e=9))

    # gather staging + outputs (collectives cannot read IO tensors)
    xs_stage = nc.dram_tensor("xs_stage", [XS_SH_ROWS, 2048], I8)
    w_stage = nc.dram_tensor("w_stage", [W_SH_ROWS, 2048], I8)
    wph_stage = nc.dram_tensor("wph_stage", [WPH_SH_ROWS, N_OUT], I8)
    # 8-group gather of BOTH batches (Shared output is much faster than the
    # 4-group/Local path); this core's batch half is then selected by one
    # partition-id-offset dram->dram DMA.
    xs_all = nc.dram_tensor("xs_all", [2 * N_OUT, 2048], I8,
                            addr_space="Shared")
    xs_full = nc.dram_tensor("xs_full", [N_OUT, 2048], I8)
    w_full = nc.dram_tensor("w_full", [W_ROWS, 2048], I8, addr_space="Shared")
    wph_full = nc.dram_tensor("wph_full", [N_OUT, N_OUT], I8,
                              addr_space="Shared")

    # biases and weight scales live in the gathered weight image
    wfl = w_full.ap().flatten()
    bias_ap = wfl[O_FBIAS:O_FWSC].bitcast(F32)
    wsc_ap = wfl[O_FWSC:O_FWPSC].bitcast(F32)
    wpsc_ap = (wfl[O_FWPSC:W_BYTES_USED].bitcast(F32)
               .rearrange("(h p) -> p h", p=96))
    wqsc_ap = wsc_ap[0:N_KP].rearrange("(m p) -> p m", p=128)
    wksc_ap = wsc_ap[N_KP:N_KP + N_OUT].rearrange("(e p) -> p e", p=128)
    wvsc_ap = (wsc_ap[N_KP + N_OUT:N_WSC]
               .rearrange("(e p) -> p e", p=128))
    bq_ap = bias_ap[0:N_KP].rearrange("(m p) -> p m", p=128)
    bk_ap = bias_ap[N_KP:2 * N_KP].rearrange("(m p) -> p m", p=128)
    bv_ap = bias_ap[2 * N_KP:2 * N_KP + N_VA].rearrange("(o n) -> o n", o=1)
    bp_ap = (bias_ap[2 * N_KP + N_VA:N_BIAS]
             .rearrange("(m p) -> p m", p=128))

    with TileContext(nc) as tc, tc.tile_pool(name="resident", bufs=1) as pr:
        # ---- resident tiles ----
        kpad = pr.tile([128, 3, L], BF16)        # k^T head-padded (32 rows/head)
        qpad = pr.tile([128, 3, 2 * CH], BF16)
        v_t = pr.tile([128, L // 128, N_VA], BF16)   # augmented v, token-major
        mC_t = pr.tile([128, 8, 2 * CH], BF16)
        mD_t = pr.tile([128, 8, CH], BF16)
        wph_t = pr.tile([96, 12, N_OUT], BF16)
        bp_t = pr.tile([128, 9], F32)
        id_t = pr.tile([128, 128], F32)      # identity for PE transposes
        yts = [pr.tile([HD_V, 2 * CH], BF16, name=f"yt{h}", tag=f"yt{h}")
               for h in range(N_HEAD)]

        with (
            tc.tile_pool(name="loads", bufs=1) as pw,
            tc.tile_pool(name="xsp", bufs=1) as pxs,
            tc.tile_pool(name="xstage", bufs=2) as pst,
            tc.tile_pool(name="scratch", bufs=1) as psc,
            tc.tile_pool(name="ps_small", bufs=2, space="PSUM") as psp,
            tc.tile_pool(name="ps_v", bufs=2, space="PSUM") as psv,
        ):
            # ---- stage shards, all-gather on device ----
            pid = nc.partition_id()
            jv = pid % 4
            nc.sync.dma_start(out=xs_stage.ap(), in_=xs_sh_ap)
            nc.sync.dma_start(out=w_stage.ap(), in_=w_sh_ap)
            nc.sync.dma_start(out=wph_stage.ap(), in_=wph_sh_ap)
            nc.gpsimd.collective_compute(
                "AllGather", mybir.AluOpType.bypass,
                replica_groups=[[0, 1, 2, 3, 4, 5, 6, 7]],
                ins=[xs_stage.ap()], outs=[xs_all.ap()],
            )
            boff = nc.s_assert_within((pid - jv) * (N_OUT // 4), 0, N_OUT,
                                      skip_runtime_assert=True)
            nc.gpsimd.dma_start(
                out=xs_full.ap(),
                in_=xs_all.ap()[bass.ds(boff, N_OUT), :],
            )
            nc.gpsimd.collective_compute(
                "AllGather", mybir.AluOpType.bypass,
                replica_groups=[[0, 1, 2, 3, 4, 5, 6, 7]],
                ins=[w_stage.ap()], outs=[w_full.ap()],
            )
            nc.gpsimd.collective_compute(
                "AllGather", mybir.AluOpType.bypass,
                replica_groups=[[0, 1, 2, 3, 4, 5, 6, 7]],
                ins=[wph_stage.ap()], outs=[wph_full.ap()],
            )

            # ---- SBUF loads; xs dequantized per 128-channel slab ----
            xs_t = pxs.tile([128, 9, L], BF16)
            xsc_t = pw.tile([128, 9], F32)
            nc.sync.dma_start(out=xsc_t, in_=xsc_ap)
            scv0 = pxs.tile([1, 16], F32, tag="scv0")
            nc.vector.tensor_copy(scv0[0:1, 0:1], xsc_t[0:1, 0:1])  # pre-touch
            xsf_r = xs_full.ap().rearrange("(e p) n -> p e n", p=128)
            for e in range(9):
                st8 = pst.tile([128, L], I8, tag="st8")
                nc.sync.dma_start(out=st8, in_=xsf_r[:, e, :])
                nc.vector.tensor_scalar(xs_t[:, e, :], st8,
                                        xsc_t[:, e:e + 1], None,
                                        mybir.AluOpType.mult)
            wqsc_t = pw.tile([128, 3], F32, tag="wqsc")
            nc.sync.dma_start(out=wqsc_t, in_=wqsc_ap)
            wksc_t = pw.tile([128, 9], F32, tag="wksc")
            nc.sync.dma_start(out=wksc_t, in_=wksc_ap)
            wvsc_t = pw.tile([128, 9], F32, tag="wvsc")
            nc.sync.dma_start(out=wvsc_t, in_=wvsc_ap)
            # wq/wk arrive unpadded (192 cols); dequantize then scatter the
            # 16-col head blocks into the head-padded strip layout. The pad
            # columns stay uninitialized — they only feed kpad/qpad pad rows
            # that no score matmul ever reads.
            wq_t = pw.tile([128, 3, N_KP], BF16)
            wq8_t = pw.tile([128, 3, N_KQ], I8, tag="wq8")
            nc.sync.dma_start(
                out=wq8_t,
                in_=wfl[0:WQ_ELS].rearrange("(m p n) -> p m n", m=3, p=128))
            wqf_t = pw.tile([128, 3, N_KQ], BF16, tag="wqf")
            for m in range(3):
                nc.vector.tensor_scalar(wqf_t[:, m, :], wq8_t[:, m, :],
                                        wqsc_t[:, m:m + 1], None,
                                        mybir.AluOpType.mult)
            for h in range(N_HEAD):
                nc.vector.tensor_copy(
                    wq_t[:, :, _PC[h]:_PC[h] + HD_K],
                    wqf_t[:, :, h * HD_K:(h + 1) * HD_K])
            wk_t = pw.tile([128, 9, N_KP], BF16)
            wkf_t = pw.tile([128, 9, N_KQ], BF16, tag="wkf")
            wv_t = pw.tile([128, 9, N_VA], BF16)
            for e in range(9):
                st = pst.tile([128, N_KQ], I8, tag="wk8")
                nc.sync.dma_start(
                    out=st,
                    in_=wfl[WQ_ELS + e * 128 * N_KQ:
                            WQ_ELS + (e + 1) * 128 * N_KQ]
                    .rearrange("(p n) -> p n", p=128))
                nc.vector.tensor_scalar(wkf_t[:, e, :], st,
                                        wksc_t[:, e:e + 1], None,
                                        mybir.AluOpType.mult)
            for h in range(N_HEAD):
                nc.vector.tensor_copy(
                    wk_t[:, :, _PC[h]:_PC[h] + HD_K],
                    wkf_t[:, :, h * HD_K:(h + 1) * HD_K])
            for e in range(9):
                st = pst.tile([128, N_VA], I8, tag="wv8")
                nc.sync.dma_start(
                    out=st,
                    in_=wfl[WQ_ELS + WK_ELS + e * 128 * N_VA:
                            WQ_ELS + WK_ELS + (e + 1) * 128 * N_VA]
                    .rearrange("(p n) -> p n", p=128))
                nc.vector.tensor_scalar(wv_t[:, e, :], st,
                                        wvsc_t[:, e:e + 1], None,
                                        mybir.AluOpType.mult)
            wpsc_t = pw.tile([96, 12], F32, tag="wpsc")
            nc.sync.dma_start(out=wpsc_t, in_=wpsc_ap)
            wphf_r = wph_full.ap().rearrange("(h p) n -> p h n", p=96)
            for h in range(N_HEAD):
                st = pst.tile([96, N_OUT], I8, tag="wph8")
                nc.sync.dma_start(out=st, in_=wphf_r[:, h, :])
                nc.vector.tensor_scalar(wph_t[:, h, :], st,
                                        wpsc_t[:, h:h + 1], None,
                                        mybir.AluOpType.mult)
            qb_t = pw.tile([128, 2 * CH], F32)
            nc.sync.dma_start(out=qb_t, in_=qoff_ap.to_broadcast([128, 2 * CH]))
            bq_t = pw.tile([128, 3], F32)
            nc.sync.dma_start(out=bq_t, in_=bq_ap)
            bk_t = pw.tile([128, 3], F32)
            nc.sync.dma_start(out=bk_t, in_=bk_ap)
            bv_t = pw.tile([128, N_VA], F32)
            nc.sync.dma_start(out=bv_t, in_=bv_ap.to_broadcast([128, N_VA]))
            nc.sync.dma_start(out=bp_t, in_=bp_ap)

            # ---- pre-touches: give each engine 1-wait visibility of loads ----
            dps = psp.tile([128, 512], F32, tag="ps")
            for i, t in enumerate(
                [xs_t[0:1, 0, 0:1], wq_t[0:1, 0, 0:1],
                 wk_t[0:1, 0, 0:1], wv_t[0:1, 0, 0:1], wph_t[0:1, 0, 0:1]]
            ):
                nc.tensor.matmul(dps[0:1, i:i + 1], lhsT=t, rhs=t,
                                 start=True, stop=True)
            sc = psc.tile([1, 16], F32)
            nc.scalar.activation(sc[0:1, 0:1], bq_t[0:1, 0:1], AF.Copy)
            nc.scalar.activation(sc[0:1, 1:2], bk_t[0:1, 0:1], AF.Copy)
            nc.scalar.activation(sc[0:1, 2:3], bp_t[0:1, 0:1], AF.Copy)
            scv = psc.tile([1, 16], F32, tag="scv")
            nc.vector.tensor_copy(scv[0:1, 0:1], bv_t[0:1, 0:1])
            nc.vector.tensor_copy(scv[0:1, 1:2], qb_t[0:1, 0:1])
            # ACT warm-up of Exp's implicit const-bias AP
            sce = psc.tile([1, 16], F32, tag="sce")
            nc.scalar.activation(sce[0:1, 0:1], scv[0:1, 0:1], AF.Exp)

            # ---- mask gen: m[p, f] = (qidx[f] - (128*kt + p) > 0) ----
            ci_t = pw.tile([128, 2 * CH], mybir.dt.int32)
            nc.gpsimd.iota(ci_t, pattern=[[1, 2 * CH]], base=0,
                           channel_multiplier=-1)
            cif_t = pw.tile([128, 2 * CH], F32)
            nc.vector.tensor_copy(cif_t, ci_t)
            mb_t = pw.tile([128, 2 * CH], F32)
            nc.vector.tensor_add(mb_t, cif_t, qb_t)
            for kt in range(8):
                nc.vector.tensor_scalar(
                    mC_t[:, kt, :], mb_t, float(128 * kt), None,
                    mybir.AluOpType.is_gt)
            for kt in range(8, 16):
                nc.vector.tensor_scalar(
                    mD_t[:, kt - 8, :], mb_t[:, CH:], float(128 * kt), None,
                    mybir.AluOpType.is_gt)
            nc.vector.tensor_scalar(id_t, cif_t[:, :128], 0.0, None,
                                    mybir.AluOpType.is_equal)

            # ---- q projection: qpad[384, 512]; q-token columns are sliced
            #      out of the gathered xs at runtime via partition-id ----
            offA = jv * CH
            offB = (7 - jv) * CH
            sq_t = pw.tile([128, 3, 2 * CH], BF16)
            for e in range(3):
                nc.vector.tensor_copy(sq_t[:, e, :CH],
                                      xs_t[:, 6 + e, bass.ds(offA, CH)])
                nc.vector.tensor_copy(sq_t[:, e, CH:],
                                      xs_t[:, 6 + e, bass.ds(offB, CH)])
            for m in range(3):
                ps = psp.tile([128, 2 * CH], F32, tag="ps")
                for e in range(3):
                    nc.tensor.matmul(
                        ps, lhsT=wq_t[:, e, m * 128:(m + 1) * 128],
                        rhs=sq_t[:, e, :],
                        start=(e == 0), stop=(e == 2),
                    )
                nc.scalar.activation(qpad[:, m, :], ps, AF.Identity,
                                     bias=bq_t[:, m:m + 1])

            # ---- k projection: kpad[384, 2048], 512-token slabs ----
            for m in range(3):
                for nt in range(L // 512):
                    ps = psp.tile([128, 512], F32, tag="ps")
                    for e in range(9):
                        nc.tensor.matmul(
                            ps,
                            lhsT=wk_t[:, e, m * 128:(m + 1) * 128],
                            rhs=xs_t[:, e, nt * 512:(nt + 1) * 512],
                            start=(e == 0), stop=(e == 8),
                        )
                    nc.scalar.activation(
                        kpad[:, m, nt * 512:(nt + 1) * 512], ps, AF.Identity,
                        bias=bk_t[:, m:m + 1],
                    )

            # ---- v projection: v[2048, 1164] (token-major, augmented) ----
            for c in range(L // 128):
                ps = psv.tile([128, N_VA], F32, tag="vps")
                for e in range(9):
                    for n0, nn in [(0, 512), (512, 512), (1024, N_VA - 1024)]:
                        nc.tensor.matmul(
                            ps[:, n0:n0 + nn],
                            lhsT=xs_t[:, e, c * 128:(c + 1) * 128],
                            rhs=wv_t[:, e, n0:n0 + nn],
                            start=(e == 0), stop=(e == 8),
                        )
                nc.vector.tensor_add(v_t[:, c, :], ps, bv_t)

        # ---- attention ----
        with (
            tc.tile_pool(name="ps_s", bufs=4, space="PSUM") as pss,
            tc.tile_pool(name="ps_y", bufs=3, space="PSUM") as psy,
            tc.tile_pool(name="exps", bufs=20) as pe,
            tc.tile_pool(name="norm", bufs=2) as pn,
            tc.tile_pool(name="rdram", bufs=6, space="DRAM") as pdram,
        ):
            for h in range(N_HEAD):
                t, a = h // 4, 32 * (h % 4)
                ems = []
                for kt in range(8):
                    s_ps = pss.tile([128, 2 * CH], F32, tag="sps")
                    nc.tensor.matmul(
                        s_ps,
                        lhsT=kpad[a:a + HD_K, t, kt * 128:(kt + 1) * 128],
                        rhs=qpad[a:a + HD_K, t, :],
                        start=True, stop=True,
                        tile_position=(a, 0),
                    )
                    e_sb = pe.tile([128, 2 * CH], BF16, tag="esb")
                    nc.scalar.activation(e_sb, s_ps, AF.Exp, scale=0.25)
                    em_sb = pe.tile([128, 2 * CH], BF16, tag="emsb")
                    nc.vector.tensor_mul(em_sb, e_sb, mC_t[:, kt, :])
                    ems.append(em_sb)
                for kt in range(8, 16):
                    s_ps = pss.tile([128, 2 * CH], F32, tag="sps")
                    nc.tensor.matmul(
                        s_ps[:, :CH],
                        lhsT=kpad[a:a + HD_K, t, kt * 128:(kt + 1) * 128],
                        rhs=qpad[a:a + HD_K, t, CH:],
                        start=True, stop=True,
                        tile_position=(a, 0),
                    )
                    e_sb = pe.tile([128, 2 * CH], BF16, tag="esb")
                    nc.scalar.activation(e_sb[:, :CH], s_ps[:, :CH], AF.Exp,
                                         scale=0.25)
                    em_sb = pe.tile([128, 2 * CH], BF16, tag="emsb")
                    nc.vector.tensor_mul(em_sb[:, :CH], e_sb[:, :CH],
                                         mD_t[:, kt - 8, :])
                    ems.append(em_sb)
                y_ps = psy.tile([HD_VA, 2 * CH], F32, tag="yps")
                for kt in range(8):
                    nc.tensor.matmul(
                        y_ps,
                        lhsT=v_t[:, kt, h * HD_VA:(h + 1) * HD_VA],
                        rhs=ems[kt],
                        start=(kt == 0), stop=False,
                    )
                for kt in range(8, 16):
                    nc.tensor.matmul(
                        y_ps[:, CH:],
                        lhsT=v_t[:, kt, h * HD_VA:(h + 1) * HD_VA],
                        rhs=ems[kt][:, :CH],
                        start=False, stop=(kt == 15),
                    )
                # normalize: row 96 of y_ps is the softmax denominator
                # (clamped away from 0 so the dead q=0 column yields 0, not NaN)
                r_sb = pn.tile([128, 2 * CH], F32, tag="rsb")
                rmx = pn.tile([128, 2 * CH], F32, tag="rmx")
                nc.vector.tensor_scalar_max(rmx[96:97, :], y_ps[96:97, :],
                                            1e-30)
                nc.vector.reciprocal(r_sb[96:97, :], rmx[96:97, :])
                rd = pdram.tile([1, 2 * CH], F32, tag="rd")
                nc.sync.dma_start(out=rd, in_=r_sb[96:97, :])
                rb_t = pn.tile([HD_V, 2 * CH], F32, tag="rbt")
                nc.sync.dma_start(
                    out=rb_t, in_=rd[0:1, :].to_broadcast([HD_V, 2 * CH])
                )
                rtc = pn.tile([1, 1], F32, tag="rtc")
                nc.vector.tensor_copy(rtc, rb_t[0:1, 0:1])  # pre-touch
                nc.vector.tensor_mul(yts[h], y_ps[:HD_V, :], rb_t)

        # ---- output projection: outT[1152, 512] = sum_h Wp_h^T @ y_h,
        #      then per-token int8 quantization: transpose, abs-max, scale ----
        with (
            tc.tile_pool(name="ps_o", bufs=2, space="PSUM") as pso,
            tc.tile_pool(name="ps_q", bufs=2, space="PSUM") as psq,
            tc.tile_pool(name="qsb", bufs=3) as pq,
            tc.tile_pool(name="qsc", bufs=1) as pqs,
        ):
            outb = pqs.tile([128, 9, 2 * CH], F32, tag="outb")
            for mo in range(9):
                ps = pso.tile([128, 2 * CH], F32)
                for h in range(N_HEAD):
                    nc.tensor.matmul(
                        ps,
                        lhsT=wph_t[:, h, mo * 128:(mo + 1) * 128],
                        rhs=yts[h],
                        start=(h == 0), stop=(h == N_HEAD - 1),
                    )
                nc.scalar.activation(outb[:, mo, :], ps, AF.Identity,
                                     bias=bp_t[:, mo:mo + 1])
            sc_all = pqs.tile([128, 4], F32)
            rcp = pqs.tile([128, 4], F32, tag="rcp")
            mxs = pqs.tile([128, 4], F32, tag="mxs")
            for tcn in range(4):
                psT = psq.tile([128, N_OUT], F32, tag="psT")
                for mo in range(9):
                    nc.tensor.matmul(
                        psT[:, mo * 128:(mo + 1) * 128],
                        lhsT=outb[:, mo, tcn * 128:(tcn + 1) * 128],
                        rhs=id_t, is_transpose=True,
                        start=True, stop=True,
                    )
                nc.vector.tensor_reduce(
                    mxs[:, tcn:tcn + 1], psT, axis=mybir.AxisListType.X,
                    op=mybir.AluOpType.max, apply_absolute_value=True)
                nc.vector.tensor_scalar_mul(sc_all[:, tcn:tcn + 1],
                                            mxs[:, tcn:tcn + 1], 1.0 / 127.0)
                nc.vector.reciprocal(rcp[:, tcn:tcn + 1],
                                     sc_all[:, tcn:tcn + 1])
                qf = pq.tile([128, N_OUT], F32, tag="qf")
                nc.vector.tensor_scalar(qf, psT, rcp[:, tcn:tcn + 1], MAGIC,
                                        mybir.AluOpType.mult,
                                        mybir.AluOpType.add)
                qg = pq.tile([128, N_OUT], F32, tag="qg")
                nc.vector.tensor_scalar(qg, qf, MAGIC, None,
                                        mybir.AluOpType.subtract)
                qi = pq.tile([128, N_OUT], mybir.dt.int8, tag="qi")
                nc.vector.tensor_copy(qi, qg)
                nc.sync.dma_start(
                    out=out_d.ap()[tcn * 128:(tcn + 1) * 128, :], in_=qi)
            sc_dst = (out_d.ap()[2 * CH:OUT_ROWS, :].flatten()[0:2 * CH * 4]
                      .bitcast(F32).rearrange("(p n) -> p n", p=128))
            nc.sync.dma_start(out=sc_dst, in_=sc_all)
    return nc


def _legalize_waits(nc):
    """This walrus build accepts only ONE sync-wait per regular instruction;
    move overflow waits onto injected same-engine NoOps (like raw-bass
    wait_ge)."""
    keep = ("InstEventSemaphore",)
    cnt = 0
    for bbh in nc.bb_map.values():
        bb = bbh.bb
        new_list = []
        for inst in bb.instructions:
            si = inst.sync_info
            if (si is not None and len(si.on_wait) > 1
                    and type(inst).__name__ not in keep):
                waits = list(si.on_wait)
                for w in waits[:-1]:
                    cnt += 1
                    n = mybir.InstNoOp(name=f"legwait_{cnt}", ins=[], outs=[])
                    n.engine = inst.engine
                    n.sync_info = mybir.SyncInfo(on_wait=[w], on_update=[])
                    try:
                        nc.register_instruction(n)
                    except Exception:
                        pass
                    new_list.append(n)
                inst.sync_info = mybir.SyncInfo(
                    on_wait=[waits[-1]], on_update=list(si.on_update))
            new_list.append(inst)
        bb.instructions = new_list
    return cnt


class _Runner:
    """Cached single-dispatch path. Replicates run_bass_via_pjrt's
    _bass_exec_p lowering, but keeps the jitted executable across calls,
    recycles the previous call's device-resident output array as the next
    call's donated output buffer (instead of uploading 4.7MB of host
    zeros), and starts the D2H copy asynchronously."""

    def __init__(self, nc):
        install_neuronx_cc_hook()
        self.nc = nc
        partition_name = (nc.partition_id_tensor.name
                          if nc.partition_id_tensor else None)
        in_names, out_names, out_avals = [], [], []
        for alloc in nc.m.functions[0].allocations:
            if not isinstance(alloc, mybir.MemoryLocationSet):
                continue
            name = alloc.memorylocations[0].name
            if alloc.kind == "ExternalInput":
                if name != partition_name:
                    in_names.append(name)
            elif alloc.kind == "ExternalOutput":
                out_names.append(name)
                out_avals.append(jax.core.ShapedArray(
                    tuple(alloc.tensor_shape), mybir.dt.np(alloc.dtype)))
        n_params, n_outs = len(in_names), len(out_avals)
        assert in_names == ["blob"] and out_names == ["out"]
        in_names_all = in_names + out_names + (
            [partition_name] if partition_name else [])

        def _body(*args):
            operands = list(args)
            if partition_name is not None:
                operands.append(partition_id_tensor())
            return tuple(_bass_exec_p.bind(
                *operands, out_avals=tuple(out_avals),
                in_names=tuple(in_names_all), out_names=tuple(out_names),
                lowering_input_output_aliases=(), sim_require_finite=True,
                sim_require_nnan=True, nc=nc))

        devices = jax.devices()[:8]
        assert len(devices) == 8
        self.mesh = Mesh(np.asarray(devices), ("core",))
        self.sharding = NamedSharding(self.mesh, PartitionSpec("core"))
        donate = tuple(range(n_params, n_params + n_outs))
        self.sharded = jax.jit(
            shard_map(_body, mesh=self.mesh,
                      in_specs=(PartitionSpec("core"),) * (n_params + n_outs),
                      out_specs=(PartitionSpec("core"),) * n_outs,
                      check_rep=False),
            donate_argnums=donate, keep_unused=True)
        self.zmaker = jax.jit(
            lambda: jnp.zeros((8 * OUT_ROWS, N_OUT), jnp.int8),
            out_shardings=self.sharding)
        self.spare = None

    def warm(self):
        """Seed the donated-buffer pool on device (no host traffic)."""
        if self.spare is None:
            s = self.zmaker()
            jax.block_until_ready(s)
            self.spare = s

    def __call__(self, blobg):
        """Full device computation: uploads the 8 per-core blobs, runs the
        kernel, returns the raw outputs as np.int8 [8*OUT_ROWS, N_OUT]."""
        self.warm()
        s = self.spare
        self.spare = None
        (o,) = self.sharded(blobg, s)
        o.copy_to_host_async()
        r = np.asarray(o)
        self.spare = o  # recycle device buffer for next donation
        return r


def _get_runner():
    global _RUNNER
    if _RUNNER is None:
        nc = _build_graph()
        _legalize_waits(nc)
        # The pjrt lowering re-serializes the (frozen, never-mutated) graph
        # on every trace (~25ms for this BIR); memoize the identical bytes.
        raw = nc.to_json_bytes()
        nc.to_json_bytes = lambda: raw
        _RUNNER = _Runner(nc)
    return _RUNNER


def _head_pad_bias(b):
    """[192] -> [384] with head h vals at 128*(h//4)+32*(h%4)."""
    bp = np.zeros((N_KP,), np.float32)
    for h in range(N_HEAD):
        bp[_PC[h]:_PC[h] + HD_K] = b[h * HD_K:(h + 1) * HD_K]
    return bp


def _prep_inputs(x, side, Wq, bq, Wkv, bkv, Wproj, bproj):
    """Quantize + pack the wire blobs: returns the global uint8 array
    (concat of the 8 per-core blobs)."""
    Wk = Wkv[:, :N_KQ]
    Wv = Wkv[:, N_KQ:]
    bk = bkv[:N_KQ]
    bv = bkv[N_KQ:]
    bq_p = _head_pad_bias(bq)
    bk_p = _head_pad_bias(bk)
    # augmented V: per head 96 channels + a zero-weight/one-bias denom channel
    Wv_a = np.zeros((N_OUT, N_VA), np.float32)
    bv_a = np.zeros((N_VA,), np.float32)
    for h in range(N_HEAD):
        Wv_a[:, h * HD_VA:h * HD_VA + HD_V] = Wv[:, h * HD_V:(h + 1) * HD_V]
        bv_a[h * HD_VA:h * HD_VA + HD_V] = bv[h * HD_V:(h + 1) * HD_V]
        bv_a[h * HD_VA + HD_V] = 1.0

    # packed int8 q/k/v weights (per input-channel-row scales), unpadded
    def q8_rows(W):
        sc = np.maximum(np.abs(W).max(axis=1), 1e-30) / 127.0
        q = np.clip(np.round(W / sc[:, None]), -127, 127).astype(np.int8)
        return q, sc.astype(np.float32)

    wq8, wqsc = q8_rows(Wq)          # [384, 192]
    wk8, wksc = q8_rows(Wk)          # [1152, 192]
    wv8, wvsc = q8_rows(Wv_a)        # [1152, 1164]
    wscales = np.concatenate([wqsc, wksc, wvsc]).astype(np.float32)
    biases = np.concatenate([bq_p, bk_p, bv_a, bproj]).astype(np.float32)

    # Wproj rows per head, int8 per-row [1152, 1152]
    wph_all, wphsc = q8_rows(Wproj.reshape(N_HEAD * HD_V, N_OUT))

    wbuf = np.zeros((W_ROWS * 2048,), np.uint8)
    wbuf[0:WQ_ELS] = wq8.reshape(-1).view(np.uint8)
    wbuf[WQ_ELS:WQ_ELS + WK_ELS] = wk8.reshape(-1).view(np.uint8)
    wbuf[WQ_ELS + WK_ELS:O_FBIAS] = wv8.reshape(-1).view(np.uint8)
    wbuf[O_FBIAS:O_FWSC] = biases.view(np.uint8)
    wbuf[O_FWSC:O_FWPSC] = wscales.view(np.uint8)
    wbuf[O_FWPSC:W_BYTES_USED] = wphsc.view(np.uint8)

    # per-channel int8 scales for [x|side]^T, shared by the 4 cores of a batch
    xscs, xsqs = [], []
    for b in range(B):
        xsT = np.ascontiguousarray(np.concatenate([x[b], side[b]], axis=1).T)
        xsc = np.maximum(np.abs(xsT).max(axis=1), 1e-30) / 127.0
        xsq = np.clip(np.round(xsT / xsc[:, None]), -127, 127).astype(np.int8)
        xscs.append(xsc.astype(np.float32))
        xsqs.append(xsq)

    blobg = np.empty((8, BLOB_BYTES), np.uint8)
    for i in range(8):
        b, j = i // 4, i % 4
        xs_shard = xsqs[b][XS_SH_ROWS * j:XS_SH_ROWS * (j + 1), :]
        # qidx[f] - f for the mask generator: q token of em column f
        qoff = np.empty((2 * CH,), np.float32)
        qoff[:CH] = 256 * j
        qoff[CH:] = 256 * (7 - j) - CH

        blobg[i, O_XS:O_W] = xs_shard.reshape(-1).view(np.uint8)
        blobg[i, O_W:O_WPH] = wbuf[W_SH_ROWS * 2048 * i:
                                   W_SH_ROWS * 2048 * (i + 1)]
        blobg[i, O_WPH:O_QOFF] = (
            wph_all[WPH_SH_ROWS * i:WPH_SH_ROWS * (i + 1), :]
            .reshape(-1).view(np.uint8))
        blobg[i, O_QOFF:O_XSC] = qoff.view(np.uint8)
        blobg[i, O_XSC:BLOB_BYTES] = xscs[b].view(np.uint8)
    return blobg.reshape(-1)


def kernel(x, side, Wq, bq, Wkv, bkv, Wproj, bproj, Wemb, bemb, **_unused):
    x = np.asarray(x, np.float32)
    side = np.asarray(side, np.float32)
    Wq = np.asarray(Wq, np.float32)
    bq = np.asarray(bq, np.float32)
    Wkv = np.asarray(Wkv, np.float32)
    bkv = np.asarray(bkv, np.float32)
    Wproj = np.asarray(Wproj, np.float32)
    bproj = np.asarray(bproj, np.float32)
    Wemb = np.asarray(Wemb, np.float32)
    bemb = np.asarray(bemb, np.float32)

    runner = _get_runner()
    blobg = _prep_inputs(x, side, Wq, bq, Wkv, bkv, Wproj, bproj)
    res = runner(blobg)

    ans = np.empty((B, L, N_OUT), np.float32)
    for i in range(8):
        b, j = i // 4, i % 4
        raw = res[OUT_ROWS * i:OUT_ROWS * (i + 1)]  # [514, 1152] int8
        scales = (raw[2 * CH:].reshape(-1).view(np.float32)[:2 * CH]
                  .reshape(128, 4))              # [partition, chunk]
        vals = raw[:2 * CH].astype(np.float32)   # [512 tokens, 1152]
        for tcn in range(4):
            vals[tcn * 128:(tcn + 1) * 128] *= scales[:, tcn:tcn + 1]
        ans[b, 256 * j:256 * j + 256] = vals[:CH]
        ans[b, 256 * (7 - j):256 * (8 - j)] = vals[CH:]
    # first token: replaced by learned embedding of side[:, 0] (exact, host-side)
    for b in range(B):
        first = side[b, 0].astype(np.float64) @ Wemb.astype(np.float64) + bemb
        ans[b, 0] = (first @ Wproj.astype(np.float64) + bproj).astype(np.float32)
    return ans


# revision 7
# speedup vs baseline: 1.4507x; 1.0367x over previous
"""Trainium2 Bass kernel: AutoregressiveSelfAttention (sparse_attention).

Sharding: 8 cores, token-parallel with zigzag causal load balancing.
  core i -> batch b = i//4, j = i%4, query chunks cA = j, cB = 7-j (256 tokens each).
  Each core computes the full per-batch KV locally, runs attention for its 512
  query tokens, and the output projection for them. Host reassembles the 8
  disjoint output slices.

Wire format (dominates wall time through the ~45MB/s shared-bandwidth axon
tunnel; pipelined multi-dispatch variants measured SLOWER than one dispatch
since up/down mostly share the link): ONE u8 blob input per core + ONE int8
output. x/side and ALL weights (q/k/v and proj) travel int8 with per-channel
f32 scales, dequantized to bf16 on device (int8+scale beats fp8 e4m3 ~3x in
accuracy; fp8 wire fails the 2e-2 gate); biases f32. Wq/Wk travel UNPADDED
(192 cols) and are scattered on device into the head-padded SBUF layout;
biases and all weight scales ride inside the sharded+gathered weight blob
instead of being replicated 8x. The x/side shards (1/4 per core, both
batches) and weight shards (1/8) are AllGathered on device over 8-core
groups with Shared outputs (the 4-group collective's forced-Local path is
~4x slower), so replicated bytes never cross the host link; each core
selects its batch half of xs with one partition-id-offset DMA. Causal masks
are generated on device (iota + a per-core q-offset row), the q token
columns are sliced out of the gathered xs at runtime via partition-id, and
the output is quantized per token to int8 (PE transpose + abs-max), with
the f32 scales embedded in the output tensor's tail rows.

Dispatch path: the stock run_bass_kernel_spmd re-jits a fresh closure every
call and uploads fresh host np.zeros for the donated output buffers (output
bytes counted twice over the wire). The _Runner here replicates its
_bass_exec_p lowering with a cached jit, recycles the previous call's
device-resident output array as the next call's donated buffer (the kernel
writes every output byte the host reads, so zero-fill is unneeded), and
overlaps the D2H result fetch with tail work via copy_to_host_async.

Device layouts (per core):
  scores as sT[kv, q] (kv on partitions) so softmax needs no transpose; the
  denominator is folded into the AV matmul via an augmented V (97th channel);
  exp needs no max-subtraction (scores are O(1)).
  k^T/q^T are head-padded to 32-row strips so score matmuls address them in
  place via tile_position; the pad columns of the scattered weight tiles are
  uninitialized garbage, which only ever flows into kpad/qpad pad rows that
  no score matmul reads. Compute instructions here may carry only ONE
  semaphore wait, so DMA-loaded tiles get same-engine pre-touches before
  their consumers (with _legalize_waits as the generic backstop).
"""

import sys
from concurrent.futures import ThreadPoolExecutor

sys.path.insert(0, "/opt/trn_rl_repo")

import numpy as np
import ml_dtypes

import jax
import jax.numpy as jnp
from jax.sharding import Mesh, PartitionSpec, NamedSharding
from jax.experimental.shard_map import shard_map

# Persistent XLA compilation cache: without it the whole BIR->NEFF pipeline
# reruns per process (~minutes); with it, repeat dispatches deserialize the
# cached executable.
jax.config.update("jax_compilation_cache_dir", "/tmp/jax_cc_cache")
jax.config.update("jax_persistent_cache_min_compile_time_secs", 0)
jax.config.update("jax_persistent_cache_min_entry_size_bytes", 0)

import concourse.bass as bass
import concourse.mybir as mybir
from concourse.tile import TileContext
from concourse.bass2jax import (
    _bass_exec_p,
    install_neuronx_cc_hook,
    partition_id_tensor,
)

BF16 = mybir.dt.bfloat16
F32 = mybir.dt.float32
NP_BF16 = ml_dtypes.bfloat16
I8 = mybir.dt.int8
AF = mybir.ActivationFunctionType

N_HEAD = 12
N_KQ = 192
N_OUT = 1152
HD_K = 16
HD_V = 96
HD_VA = 97             # v head channels + denominator column
N_VA = N_HEAD * HD_VA  # 1164
N_KP = N_HEAD * 32     # 384: head-padded k/q channel count
B, L = 2, 2048
CH = 256

# ---- packed weight image (flat bytes, sharded 1/8 per core, gathered) ----
WQ_ELS = N_KQ * N_KQ * 2        # 73728: unpadded wq is [384, 192]
WK_ELS = N_OUT * N_KQ           # 221184: unpadded wk is [1152, 192]
WV_ELS = N_OUT * N_VA           # 1340928
O_FBIAS = WQ_ELS + WK_ELS + WV_ELS              # 1635840
N_BIAS = N_KP + N_KP + N_VA + N_OUT             # 3084 f32
O_FWSC = O_FBIAS + N_BIAS * 4                   # 1648176
N_WSC = N_KP + N_OUT + N_OUT                    # 2688 f32 w row scales
O_FWPSC = O_FWSC + N_WSC * 4                    # 1658928
W_BYTES_USED = O_FWPSC + N_OUT * 4              # 1663536
W_ROWS = 816                    # 1671168 bytes: W_BYTES_USED padded to 8*2048
W_SH_ROWS = W_ROWS // 8         # 102
WPH_SH_ROWS = N_OUT // 8        # 144

# ---- blob layout (byte offsets) ----
XS_SH_ROWS = N_OUT // 4          # 288 rows of xsT per core (4-way gather)
O_XS = 0
O_W = O_XS + XS_SH_ROWS * 2048                  # 589824 (int8 xs shard)
O_WPH = O_W + W_SH_ROWS * 2048                  # 798720 (int8 w shard)
O_QOFF = O_WPH + WPH_SH_ROWS * N_OUT            # 964608 (int8 wph shard)
O_XSC = O_QOFF + 2 * CH * 4                     # 966656
BLOB_BYTES = O_XSC + N_OUT * 4                  # 971264

MAGIC = 12582912.0  # 1.5 * 2**23: f32 add/sub rounds to nearest integer
OUT_ROWS = 2 * CH + 2  # 512 token rows int8 + 2 rows carrying 512 f32 scales

_RUNNER = None

# padded column offset of head h inside the 384-col head-padded layout
_PC = [128 * (h // 4) + 32 * (h % 4) for h in range(N_HEAD)]


def _build_graph():
    nc = bass.Bass(num_devices=8)
    blob = nc.declare_dram_parameter("blob", [BLOB_BYTES], mybir.dt.uint8,
                                     isOutput=False)
    out_d = nc.declare_dram_parameter("out", [OUT_ROWS, N_OUT], mybir.dt.int8,
                                      isOutput=True)

    bap = blob.ap()
    xs_sh_ap = bap[O_XS:O_W].bitcast(I8).rearrange("(p n) -> p n",
                                                   p=XS_SH_ROWS)
    w_sh_ap = bap[O_W:O_WPH].bitcast(I8).rearrange("(p n) -> p n",
                                                   p=W_SH_ROWS)
    wph_sh_ap = (bap[O_WPH:O_QOFF].bitcast(I8)
                 .rearrange("(p n) -> p n", p=WPH_SH_ROWS))
    qoff_ap = (bap[O_QOFF:O_XSC].bitcast(F32)
               .rearrange("(o n) -> o n", o=1))
    xsc_ap = (bap[O_XSC:BLOB_BYTES].bitcast(F32)
              .rearrange("(e p) -> p e", e=9))

    # gather staging + outputs (collectives cannot read IO tensors)
    xs_stage = nc.dram_tensor("xs_stage", [XS_SH_ROWS, 2048], I8)
    w_stage = nc.dram_tensor("w_stage", [W_SH_ROWS, 2048], I8)
    wph_stage = nc.dram_tensor("wph_stage", [WPH_SH_ROWS, N_OUT], I8)
    # 8-group gather of BOTH batches (Shared output is much faster than the
    # 4-group/Local path); this core's batch half is then selected by one
    # partition-id-offset dram->dram DMA.
    xs_all = nc.dram_tensor("xs_all", [2 * N_OUT, 2048], I8,
                            addr_space="Shared")
    xs_full = nc.dram_tensor("xs_full", [N_OUT, 2048], I8)
    w_full = nc.dram_tensor("w_full", [W_ROWS, 2048], I8, addr_space="Shared")
    wph_full = nc.dram_tensor("wph_full", [N_OUT, N_OUT], I8,
                              addr_space="Shared")

    # biases and weight scales live in the gathered weight image
    wfl = w_full.ap().flatten()
    bias_ap = wfl[O_FBIAS:O_FWSC].bitcast(F32)
    wsc_ap = wfl[O_FWSC:O_FWPSC].bitcast(F32)
    wpsc_ap = (wfl[O_FWPSC:W_BYTES_USED].bitcast(F32)
               .rearrange("(h p) -> p h", p=96))
    wqsc_ap = wsc_ap[0:N_KP].rearrange("(m p) -> p m", p=128)
    wksc_ap = wsc_ap[N_KP:N_KP + N_OUT].rearrange("(e p) -> p e", p=128)
    wvsc_ap = (wsc_ap[N_KP + N_OUT:N_WSC]
               .rearrange("(e p) -> p e", p=128))
    bq_ap = bias_ap[0:N_KP].rearrange("(m p) -> p m", p=128)
    bk_ap = bias_ap[N_KP:2 * N_KP].rearrange("(m p) -> p m", p=128)
    bv_ap = bias_ap[2 * N_KP:2 * N_KP + N_VA].rearrange("(o n) -> o n", o=1)
    bp_ap = (bias_ap[2 * N_KP + N_VA:N_BIAS]
             .rearrange("(m p) -> p m", p=128))

    with TileContext(nc) as tc, tc.tile_pool(name="resident", bufs=1) as pr:
        # ---- resident tiles ----
        kpad = pr.tile([128, 3, L], BF16)        # k^T head-padded (32 rows/head)
        qpad = pr.tile([128, 3, 2 * CH], BF16)
        v_t = pr.tile([128, L // 128, N_VA], BF16)   # augmented v, token-major
        mC_t = pr.tile([128, 8, 2 * CH], BF16)
        mD_t = pr.tile([128, 8, CH], BF16)
        wph_t = pr.tile([96, 12, N_OUT], BF16)
        bp_t = pr.tile([128, 9], F32)
        id_t = pr.tile([128, 128], F32)      # identity for PE transposes
        yts = [pr.tile([HD_V, 2 * CH], BF16, name=f"yt{h}", tag=f"yt{h}")
               for h in range(N_HEAD)]

        with (
            tc.tile_pool(name="loads", bufs=1) as pw,
            tc.tile_pool(name="xsp", bufs=1) as pxs,
            tc.tile_pool(name="xstage", bufs=2) as pst,
            tc.tile_pool(name="scratch", bufs=1) as psc,
            tc.tile_pool(name="ps_small", bufs=2, space="PSUM") as psp,
            tc.tile_pool(name="ps_v", bufs=2, space="PSUM") as psv,
        ):
            # ---- stage shards, all-gather on device ----
            pid = nc.partition_id()
            jv = pid % 4
            nc.sync.dma_start(out=xs_stage.ap(), in_=xs_sh_ap)
            nc.sync.dma_start(out=w_stage.ap(), in_=w_sh_ap)
            nc.sync.dma_start(out=wph_stage.ap(), in_=wph_sh_ap)
            nc.gpsimd.collective_compute(
                "AllGather", mybir.AluOpType.bypass,
                replica_groups=[[0, 1, 2, 3, 4, 5, 6, 7]],
                ins=[xs_stage.ap()], outs=[xs_all.ap()],
            )
            boff = nc.s_assert_within((pid - jv) * (N_OUT // 4), 0, N_OUT,
                                      skip_runtime_assert=True)
            nc.gpsimd.dma_start(
                out=xs_full.ap(),
                in_=xs_all.ap()[bass.ds(boff, N_OUT), :],
            )
            nc.gpsimd.collective_compute(
                "AllGather", mybir.AluOpType.bypass,
                replica_groups=[[0, 1, 2, 3, 4, 5, 6, 7]],
                ins=[w_stage.ap()], outs=[w_full.ap()],
            )
            nc.gpsimd.collective_compute(
                "AllGather", mybir.AluOpType.bypass,
                replica_groups=[[0, 1, 2, 3, 4, 5, 6, 7]],
                ins=[wph_stage.ap()], outs=[wph_full.ap()],
            )

            # ---- SBUF loads; xs dequantized per 128-channel slab ----
            xs_t = pxs.tile([128, 9, L], BF16)
            xsc_t = pw.tile([128, 9], F32)
            nc.sync.dma_start(out=xsc_t, in_=xsc_ap)
            scv0 = pxs.tile([1, 16], F32, tag="scv0")
            nc.vector.tensor_copy(scv0[0:1, 0:1], xsc_t[0:1, 0:1])  # pre-touch
            xsf_r = xs_full.ap().rearrange("(e p) n -> p e n", p=128)
            for e in range(9):
                st8 = pst.tile([128, L], I8, tag="st8")
                nc.sync.dma_start(out=st8, in_=xsf_r[:, e, :])
                nc.vector.tensor_scalar(xs_t[:, e, :], st8,
                                        xsc_t[:, e:e + 1], None,
                                        mybir.AluOpType.mult)
            wqsc_t = pw.tile([128, 3], F32, tag="wqsc")
            nc.sync.dma_start(out=wqsc_t, in_=wqsc_ap)
            wksc_t = pw.tile([128, 9], F32, tag="wksc")
            nc.sync.dma_start(out=wksc_t, in_=wksc_ap)
            wvsc_t = pw.tile([128, 9], F32, tag="wvsc")
            nc.sync.dma_start(out=wvsc_t, in_=wvsc_ap)
            # wq/wk arrive unpadded (192 cols); dequantize then scatter the
            # 16-col head blocks into the head-padded strip layout. The pad
            # columns stay uninitialized — they only feed kpad/qpad pad rows
            # that no score matmul ever reads.
            wq_t = pw.tile([128, 3, N_KP], BF16)
            wq8_t = pw.tile([128, 3, N_KQ], I8, tag="wq8")
            nc.sync.dma_start(
                out=wq8_t,
                in_=wfl[0:WQ_ELS].rearrange("(m p n) -> p m n", m=3, p=128))
            wqf_t = pw.tile([128, 3, N_KQ], BF16, tag="wqf")
            for m in range(3):
                nc.vector.tensor_scalar(wqf_t[:, m, :], wq8_t[:, m, :],
                                        wqsc_t[:, m:m + 1], None,
                                        mybir.AluOpType.mult)
            for h in range(N_HEAD):
                nc.vector.tensor_copy(
                    wq_t[:, :, _PC[h]:_PC[h] + HD_K],
                    wqf_t[:, :, h * HD_K:(h + 1) * HD_K])
            wk_t = pw.tile([128, 9, N_KP], BF16)
            wkf_t = pw.tile([128, 9, N_KQ], BF16, tag="wkf")
            wv_t = pw.tile([128, 9, N_VA], BF16)
            for e in range(9):
                st = pst.tile([128, N_KQ], I8, tag="wk8")
                nc.sync.dma_start(
                    out=st,
                    in_=wfl[WQ_ELS + e * 128 * N_KQ:
                            WQ_ELS + (e + 1) * 128 * N_KQ]
                    .rearrange("(p n) -> p n", p=128))
                nc.vector.tensor_scalar(wkf_t[:, e, :], st,
                                        wksc_t[:, e:e + 1], None,
                                        mybir.AluOpType.mult)
            for h in range(N_HEAD):
                nc.vector.tensor_copy(
                    wk_t[:, :, _PC[h]:_PC[h] + HD_K],
                    wkf_t[:, :, h * HD_K:(h + 1) * HD_K])
            for e in range(9):
                st = pst.tile([128, N_VA], I8, tag="wv8")
                nc.sync.dma_start(
                    out=st,
                    in_=wfl[WQ_ELS + WK_ELS + e * 128 * N_VA:
                            WQ_ELS + WK_ELS + (e + 1) * 128 * N_VA]
                    .rearrange("(p n) -> p n", p=128))
                nc.vector.tensor_scalar(wv_t[:, e, :], st,
                                        wvsc_t[:, e:e + 1], None,
                                        mybir.AluOpType.mult)
            wpsc_t = pw.tile([96, 12], F32, tag="wpsc")
            nc.sync.dma_start(out=wpsc_t, in_=wpsc_ap)
            wphf_r = wph_full.ap().rearrange("(h p) n -> p h n", p=96)
            for h in range(N_HEAD):
                st = pst.tile([96, N_OUT], I8, tag="wph8")
                nc.sync.dma_start(out=st, in_=wphf_r[:, h, :])
                nc.vector.tensor_scalar(wph_t[:, h, :], st,
                                        wpsc_t[:, h:h + 1], None,
                                        mybir.AluOpType.mult)
            qb_t = pw.tile([128, 2 * CH], F32)
            nc.sync.dma_start(out=qb_t, in_=qoff_ap.to_broadcast([128, 2 * CH]))
            bq_t = pw.tile([128, 3], F32)
            nc.sync.dma_start(out=bq_t, in_=bq_ap)
            bk_t = pw.tile([128, 3], F32)
            nc.sync.dma_start(out=bk_t, in_=bk_ap)
            bv_t = pw.tile([128, N_VA], F32)
            nc.sync.dma_start(out=bv_t, in_=bv_ap.to_broadcast([128, N_VA]))
            nc.sync.dma_start(out=bp_t, in_=bp_ap)

            # ---- pre-touches: give each engine 1-wait visibility of loads ----
            dps = psp.tile([128, 512], F32, tag="ps")
            for i, t in enumerate(
                [xs_t[0:1, 0, 0:1], wq_t[0:1, 0, 0:1],
                 wk_t[0:1, 0, 0:1], wv_t[0:1, 0, 0:1], wph_t[0:1, 0, 0:1]]
            ):
                nc.tensor.matmul(dps[0:1, i:i + 1], lhsT=t, rhs=t,
                                 start=True, stop=True)
            sc = psc.tile([1, 16], F32)
            nc.scalar.activation(sc[0:1, 0:1], bq_t[0:1, 0:1], AF.Copy)
            nc.scalar.activation(sc[0:1, 1:2], bk_t[0:1, 0:1], AF.Copy)
            nc.scalar.activation(sc[0:1, 2:3], bp_t[0:1, 0:1], AF.Copy)
            scv = psc.tile([1, 16], F32, tag="scv")
            nc.vector.tensor_copy(scv[0:1, 0:1], bv_t[0:1, 0:1])
            nc.vector.tensor_copy(scv[0:1, 1:2], qb_t[0:1, 0:1])
            # ACT warm-up of Exp's implicit const-bias AP
            sce = psc.tile([1, 16], F32, tag="sce")
            nc.scalar.activation(sce[0:1, 0:1], scv[0:1, 0:1], AF.Exp)

            # ---- mask gen: m[p, f] = (qidx[f] - (128*kt + p) > 0) ----
            ci_t = pw.tile([128, 2 * CH], mybir.dt.int32)
            nc.gpsimd.iota(ci_t, pattern=[[1, 2 * CH]], base=0,
                           channel_multiplier=-1)
            cif_t = pw.tile([128, 2 * CH], F32)
            nc.vector.tensor_copy(cif_t, ci_t)
            mb_t = pw.tile([128, 2 * CH], F32)
            nc.vector.tensor_add(mb_t, cif_t, qb_t)
            for kt in range(8):
                nc.vector.tensor_scalar(
                    mC_t[:, kt, :], mb_t, float(128 * kt), None,
                    mybir.AluOpType.is_gt)
            for kt in range(8, 16):
                nc.vector.tensor_scalar(
                    mD_t[:, kt - 8, :], mb_t[:, CH:], float(128 * kt), None,
                    mybir.AluOpType.is_gt)
            nc.vector.tensor_scalar(id_t, cif_t[:, :128], 0.0, None,
                                    mybir.AluOpType.is_equal)

            # ---- q projection: qpad[384, 512]; q-token columns are sliced
            #      out of the gathered xs at runtime via partition-id ----
            offA = jv * CH
            offB = (7 - jv) * CH
            sq_t = pw.tile([128, 3, 2 * CH], BF16)
            for e in range(3):
                nc.vector.tensor_copy(sq_t[:, e, :CH],
                                      xs_t[:, 6 + e, bass.ds(offA, CH)])
                nc.vector.tensor_copy(sq_t[:, e, CH:],
                                      xs_t[:, 6 + e, bass.ds(offB, CH)])
            for m in range(3):
                ps = psp.tile([128, 2 * CH], F32, tag="ps")
                for e in range(3):
                    nc.tensor.matmul(
                        ps, lhsT=wq_t[:, e, m * 128:(m + 1) * 128],
                        rhs=sq_t[:, e, :],
                        start=(e == 0), stop=(e == 2),
                    )
                nc.scalar.activation(qpad[:, m, :], ps, AF.Identity,
                                     bias=bq_t[:, m:m + 1])

            # ---- k projection: kpad[384, 2048], 512-token slabs ----
            for m in range(3):
                for nt in range(L // 512):
                    ps = psp.tile([128, 512], F32, tag="ps")
                    for e in range(9):
                        nc.tensor.matmul(
                            ps,
                            lhsT=wk_t[:, e, m * 128:(m + 1) * 128],
                            rhs=xs_t[:, e, nt * 512:(nt + 1) * 512],
                            start=(e == 0), stop=(e == 8),
                        )
                    nc.scalar.activation(
                        kpad[:, m, nt * 512:(nt + 1) * 512], ps, AF.Identity,
                        bias=bk_t[:, m:m + 1],
                    )

            # ---- v projection: v[2048, 1164] (token-major, augmented) ----
            for c in range(L // 128):
                ps = psv.tile([128, N_VA], F32, tag="vps")
                for e in range(9):
                    for n0, nn in [(0, 512), (512, 512), (1024, N_VA - 1024)]:
                        nc.tensor.matmul(
                            ps[:, n0:n0 + nn],
                            lhsT=xs_t[:, e, c * 128:(c + 1) * 128],
                            rhs=wv_t[:, e, n0:n0 + nn],
                            start=(e == 0), stop=(e == 8),
                        )
                nc.vector.tensor_add(v_t[:, c, :], ps, bv_t)

        # ---- attention ----
        with (
            tc.tile_pool(name="ps_s", bufs=4, space="PSUM") as pss,
            tc.tile_pool(name="ps_y", bufs=3, space="PSUM") as psy,
            tc.tile_pool(name="exps", bufs=20) as pe,
            tc.tile_pool(name="norm", bufs=2) as pn,
            tc.tile_pool(name="rdram", bufs=6, space="DRAM") as pdram,
        ):
            for h in range(N_HEAD):
                t, a = h // 4, 32 * (h % 4)
                ems = []
                for kt in range(8):
                    s_ps = pss.tile([128, 2 * CH], F32, tag="sps")
                    nc.tensor.matmul(
                        s_ps,
                        lhsT=kpad[a:a + HD_K, t, kt * 128:(kt + 1) * 128],
                        rhs=qpad[a:a + HD_K, t, :],
                        start=True, stop=True,
                        tile_position=(a, 0),
                    )
                    e_sb = pe.tile([128, 2 * CH], BF16, tag="esb")
                    nc.scalar.activation(e_sb, s_ps, AF.Exp, scale=0.25)
                    em_sb = pe.tile([128, 2 * CH], BF16, tag="emsb")
                    nc.vector.tensor_mul(em_sb, e_sb, mC_t[:, kt, :])
                    ems.append(em_sb)
                for kt in range(8, 16):
                    s_ps = pss.tile([128, 2 * CH], F32, tag="sps")
                    nc.tensor.matmul(
                        s_ps[:, :CH],
                        lhsT=kpad[a:a + HD_K, t, kt * 128:(kt + 1) * 128],
                        rhs=qpad[a:a + HD_K, t, CH:],
                        start=True, stop=True,
                        tile_position=(a, 0),
                    )
                    e_sb = pe.tile([128, 2 * CH], BF16, tag="esb")
                    nc.scalar.activation(e_sb[:, :CH], s_ps[:, :CH], AF.Exp,
                                         scale=0.25)
                    em_sb = pe.tile([128, 2 * CH], BF16, tag="emsb")
                    nc.vector.tensor_mul(em_sb[:, :CH], e_sb[:, :CH],
                                         mD_t[:, kt - 8, :])
                    ems.append(em_sb)
                y_ps = psy.tile([HD_VA, 2 * CH], F32, tag="yps")
                for kt in range(8):
                    nc.tensor.matmul(
                        y_ps,
                        lhsT=v_t[:, kt, h * HD_VA:(h + 1) * HD_VA],
                        rhs=ems[kt],
                        start=(kt == 0), stop=False,
                    )
                for kt in range(8, 16):
                    nc.tensor.matmul(
                        y_ps[:, CH:],
                        lhsT=v_t[:, kt, h * HD_VA:(h + 1) * HD_VA],
                        rhs=ems[kt][:, :CH],
                        start=False, stop=(kt == 15),
                    )
                # normalize: row 96 of y_ps is the softmax denominator
                # (clamped away from 0 so the dead q=0 column yields 0, not NaN)
                r_sb = pn.tile([128, 2 * CH], F32, tag="rsb")
                rmx = pn.tile([128, 2 * CH], F32, tag="rmx")
                nc.vector.tensor_scalar_max(rmx[96:97, :], y_ps[96:97, :],
                                            1e-30)
                nc.vector.reciprocal(r_sb[96:97, :], rmx[96:97, :])
                rd = pdram.tile([1, 2 * CH], F32, tag="rd")
                nc.sync.dma_start(out=rd, in_=r_sb[96:97, :])
                rb_t = pn.tile([HD_V, 2 * CH], F32, tag="rbt")
                nc.sync.dma_start(
                    out=rb_t, in_=rd[0:1, :].to_broadcast([HD_V, 2 * CH])
                )
                rtc = pn.tile([1, 1], F32, tag="rtc")
                nc.vector.tensor_copy(rtc, rb_t[0:1, 0:1])  # pre-touch
                nc.vector.tensor_mul(yts[h], y_ps[:HD_V, :], rb_t)

        # ---- output projection: outT[1152, 512] = sum_h Wp_h^T @ y_h,
        #      then per-token int8 quantization: transpose, abs-max, scale ----
        with (
            tc.tile_pool(name="ps_o", bufs=2, space="PSUM") as pso,
            tc.tile_pool(name="ps_q", bufs=2, space="PSUM") as psq,
            tc.tile_pool(name="qsb", bufs=3) as pq,
            tc.tile_pool(name="qsc", bufs=1) as pqs,
        ):
            outb = pqs.tile([128, 9, 2 * CH], F32, tag="outb")
            for mo in range(9):
                ps = pso.tile([128, 2 * CH], F32)
                for h in range(N_HEAD):
                    nc.tensor.matmul(
                        ps,
                        lhsT=wph_t[:, h, mo * 128:(mo + 1) * 128],
                        rhs=yts[h],
                        start=(h == 0), stop=(h == N_HEAD - 1),
                    )
                nc.scalar.activation(outb[:, mo, :], ps, AF.Identity,
                                     bias=bp_t[:, mo:mo + 1])
            sc_all = pqs.tile([128, 4], F32)
            rcp = pqs.tile([128, 4], F32, tag="rcp")
            mxs = pqs.tile([128, 4], F32, tag="mxs")
            for tcn in range(4):
                psT = psq.tile([128, N_OUT], F32, tag="psT")
                for mo in range(9):
                    nc.tensor.matmul(
                        psT[:, mo * 128:(mo + 1) * 128],
                        lhsT=outb[:, mo, tcn * 128:(tcn + 1) * 128],
                        rhs=id_t, is_transpose=True,
                        start=True, stop=True,
                    )
                nc.vector.tensor_reduce(
                    mxs[:, tcn:tcn + 1], psT, axis=mybir.AxisListType.X,
                    op=mybir.AluOpType.max, apply_absolute_value=True)
                nc.vector.tensor_scalar_mul(sc_all[:, tcn:tcn + 1],
                                            mxs[:, tcn:tcn + 1], 1.0 / 127.0)
                nc.vector.reciprocal(rcp[:, tcn:tcn + 1],
                                     sc_all[:, tcn:tcn + 1])
                qf = pq.tile([128, N_OUT], F32, tag="qf")
                nc.vector.tensor_scalar(qf, psT, rcp[:, tcn:tcn + 1], MAGIC,
                                        mybir.AluOpType.mult,
                                        mybir.AluOpType.add)
                qg = pq.tile([128, N_OUT], F32, tag="qg")
                nc.vector.tensor_scalar(qg, qf, MAGIC, None,
                                        mybir.AluOpType.subtract)
                qi = pq.tile([128, N_OUT], mybir.dt.int8, tag="qi")
                nc.vector.tensor_copy(qi, qg)
                nc.sync.dma_start(
                    out=out_d.ap()[tcn * 128:(tcn + 1) * 128, :], in_=qi)
            sc_dst = (out_d.ap()[2 * CH:OUT_ROWS, :].flatten()[0:2 * CH * 4]
                      .bitcast(F32).rearrange("(p n) -> p n", p=128))
            nc.sync.dma_start(out=sc_dst, in_=sc_all)
    return nc


def _legalize_waits(nc):
    """This walrus build accepts only ONE sync-wait per regular instruction;
    move overflow waits onto injected same-engine NoOps (like raw-bass
    wait_ge)."""
    keep = ("InstEventSemaphore",)
    cnt = 0
    for bbh in nc.bb_map.values():
        bb = bbh.bb
        new_list = []
        for inst in bb.instructions:
            si = inst.sync_info
            if (si is not None and len(si.on_wait) > 1
                    and type(inst).__name__ not in keep):
                waits = list(si.on_wait)
                for w in waits[:-1]:
                    cnt += 1
                    n = mybir.InstNoOp(name=f"legwait_{cnt}", ins=[], outs=[])
                    n.engine = inst.engine
                    n.sync_info = mybir.SyncInfo(on_wait=[w], on_update=[])
                    try:
                        nc.register_instruction(n)
                    except Exception:
                        pass
                    new_list.append(n)
                inst.sync_info = mybir.SyncInfo(
                    on_wait=[waits[-1]], on_update=list(si.on_update))
            new_list.append(inst)
        bb.instructions = new_list
    return cnt


class _Runner:
    """Cached single-dispatch path. Replicates run_bass_via_pjrt's
    _bass_exec_p lowering, but keeps the jitted executable across calls,
    recycles the previous call's device-resident output array as the next
    call's donated output buffer (instead of uploading 4.7MB of host
    zeros), and starts the D2H copy asynchronously."""

    def __init__(self, nc):
        install_neuronx_cc_hook()
        self.nc = nc
        partition_name = (nc.partition_id_tensor.name
                          if nc.partition_id_tensor else None)
        in_names, out_names, out_avals = [], [], []
        for alloc in nc.m.functions[0].allocations:
            if not isinstance(alloc, mybir.MemoryLocationSet):
                continue
            name = alloc.memorylocations[0].name
            if alloc.kind == "ExternalInput":
                if name != partition_name:
                    in_names.append(name)
            elif alloc.kind == "ExternalOutput":
                out_names.append(name)
                out_avals.append(jax.core.ShapedArray(
                    tuple(alloc.tensor_shape), mybir.dt.np(alloc.dtype)))
        n_params, n_outs = len(in_names), len(out_avals)
        assert in_names == ["blob"] and out_names == ["out"]
        in_names_all = in_names + out_names + (
            [partition_name] if partition_name else [])

        def _body(*args):
            operands = list(args)
            if partition_name is not None:
                operands.append(partition_id_tensor())
            return tuple(_bass_exec_p.bind(
                *operands, out_avals=tuple(out_avals),
                in_names=tuple(in_names_all), out_names=tuple(out_names),
                lowering_input_output_aliases=(), sim_require_finite=True,
                sim_require_nnan=True, nc=nc))

        devices = jax.devices()[:8]
        assert len(devices) == 8
        self.mesh = Mesh(np.asarray(devices), ("core",))
        self.sharding = NamedSharding(self.mesh, PartitionSpec("core"))
        donate = tuple(range(n_params, n_params + n_outs))
        self.sharded = jax.jit(
            shard_map(_body, mesh=self.mesh,
                      in_specs=(PartitionSpec("core"),) * (n_params + n_outs),
                      out_specs=(PartitionSpec("core"),) * n_outs,
                      check_rep=False),
            donate_argnums=donate, keep_unused=True)
        self.zmaker = jax.jit(
            lambda: jnp.zeros((8 * OUT_ROWS, N_OUT), jnp.int8),
            out_shardings=self.sharding)
        self.spare = None
        self.pool = ThreadPoolExecutor(8)

    def warm(self):
        """Seed the donated-buffer pool on device (no host traffic)."""
        if self.spare is None:
            s = self.zmaker()
            jax.block_until_ready(s)
            self.spare = s

    def __call__(self, blobg):
        """Full device computation: uploads the 8 per-core blobs, runs the
        kernel, returns the 8 per-core raw outputs, each np.int8
        [OUT_ROWS, N_OUT]. Shards are fetched concurrently (the tunnel's
        per-fetch fixed cost overlaps across shards)."""
        self.warm()
        s = self.spare
        self.spare = None
        (o,) = self.sharded(blobg, s)
        o.copy_to_host_async()
        shards = sorted(o.addressable_shards,
                        key=lambda sh: sh.index[0].start
                        if sh.index[0].start else 0)
        assert len(shards) == 8
        parts = list(self.pool.map(lambda sh: np.asarray(sh.data), shards))
        self.spare = o  # recycle device buffer for next donation
        return parts


def _get_runner():
    global _RUNNER
    if _RUNNER is None:
        nc = _build_graph()
        _legalize_waits(nc)
        # The pjrt lowering re-serializes the (frozen, never-mutated) graph
        # on every trace (~25ms for this BIR); memoize the identical bytes.
        raw = nc.to_json_bytes()
        nc.to_json_bytes = lambda: raw
        _RUNNER = _Runner(nc)
    return _RUNNER


def _head_pad_bias(b):
    """[192] -> [384] with head h vals at 128*(h//4)+32*(h%4)."""
    bp = np.zeros((N_KP,), np.float32)
    for h in range(N_HEAD):
        bp[_PC[h]:_PC[h] + HD_K] = b[h * HD_K:(h + 1) * HD_K]
    return bp


def _prep_inputs(x, side, Wq, bq, Wkv, bkv, Wproj, bproj):
    """Quantize + pack the wire blobs: returns the global uint8 array
    (concat of the 8 per-core blobs)."""
    Wk = Wkv[:, :N_KQ]
    Wv = Wkv[:, N_KQ:]
    bk = bkv[:N_KQ]
    bv = bkv[N_KQ:]
    bq_p = _head_pad_bias(bq)
    bk_p = _head_pad_bias(bk)
    # augmented V: per head 96 channels + a zero-weight/one-bias denom channel
    Wv_a = np.zeros((N_OUT, N_VA), np.float32)
    bv_a = np.zeros((N_VA,), np.float32)
    for h in range(N_HEAD):
        Wv_a[:, h * HD_VA:h * HD_VA + HD_V] = Wv[:, h * HD_V:(h + 1) * HD_V]
        bv_a[h * HD_VA:h * HD_VA + HD_V] = bv[h * HD_V:(h + 1) * HD_V]
        bv_a[h * HD_VA + HD_V] = 1.0

    # packed int8 q/k/v weights (per input-channel-row scales), unpadded
    def q8_rows(W):
        sc = np.maximum(np.abs(W).max(axis=1), 1e-30) / 127.0
        q = np.clip(np.round(W / sc[:, None]), -127, 127).astype(np.int8)
        return q, sc.astype(np.float32)

    wq8, wqsc = q8_rows(Wq)          # [384, 192]
    wk8, wksc = q8_rows(Wk)          # [1152, 192]
    wv8, wvsc = q8_rows(Wv_a)        # [1152, 1164]
    wscales = np.concatenate([wqsc, wksc, wvsc]).astype(np.float32)
    biases = np.concatenate([bq_p, bk_p, bv_a, bproj]).astype(np.float32)

    # Wproj rows per head, int8 per-row [1152, 1152]
    wph_all, wphsc = q8_rows(Wproj.reshape(N_HEAD * HD_V, N_OUT))

    wbuf = np.zeros((W_ROWS * 2048,), np.uint8)
    wbuf[0:WQ_ELS] = wq8.reshape(-1).view(np.uint8)
    wbuf[WQ_ELS:WQ_ELS + WK_ELS] = wk8.reshape(-1).view(np.uint8)
    wbuf[WQ_ELS + WK_ELS:O_FBIAS] = wv8.reshape(-1).view(np.uint8)
    wbuf[O_FBIAS:O_FWSC] = biases.view(np.uint8)
    wbuf[O_FWSC:O_FWPSC] = wscales.view(np.uint8)
    wbuf[O_FWPSC:W_BYTES_USED] = wphsc.view(np.uint8)

    # per-channel int8 scales for [x|side]^T, shared by the 4 cores of a batch
    xscs, xsqs = [], []
    for b in range(B):
        xsT = np.ascontiguousarray(np.concatenate([x[b], side[b]], axis=1).T)
        xsc = np.maximum(np.abs(xsT).max(axis=1), 1e-30) / 127.0
        xsq = np.clip(np.round(xsT / xsc[:, None]), -127, 127).astype(np.int8)
        xscs.append(xsc.astype(np.float32))
        xsqs.append(xsq)

    blobg = np.empty((8, BLOB_BYTES), np.uint8)
    for i in range(8):
        b, j = i // 4, i % 4
        xs_shard = xsqs[b][XS_SH_ROWS * j:XS_SH_ROWS * (j + 1), :]
        # qidx[f] - f for the mask generator: q token of em column f
        qoff = np.empty((2 * CH,), np.float32)
        qoff[:CH] = 256 * j
        qoff[CH:] = 256 * (7 - j) - CH

        blobg[i, O_XS:O_W] = xs_shard.reshape(-1).view(np.uint8)
        blobg[i, O_W:O_WPH] = wbuf[W_SH_ROWS * 2048 * i:
                                   W_SH_ROWS * 2048 * (i + 1)]
        blobg[i, O_WPH:O_QOFF] = (
            wph_all[WPH_SH_ROWS * i:WPH_SH_ROWS * (i + 1), :]
            .reshape(-1).view(np.uint8))
        blobg[i, O_QOFF:O_XSC] = qoff.view(np.uint8)
        blobg[i, O_XSC:BLOB_BYTES] = xscs[b].view(np.uint8)
    return blobg.reshape(-1)


def kernel(x, side, Wq, bq, Wkv, bkv, Wproj, bproj, Wemb, bemb, **_unused):
    x = np.asarray(x, np.float32)
    side = np.asarray(side, np.float32)
    Wq = np.asarray(Wq, np.float32)
    bq = np.asarray(bq, np.float32)
    Wkv = np.asarray(Wkv, np.float32)
    bkv = np.asarray(bkv, np.float32)
    Wproj = np.asarray(Wproj, np.float32)
    bproj = np.asarray(bproj, np.float32)
    Wemb = np.asarray(Wemb, np.float32)
    bemb = np.asarray(bemb, np.float32)

    runner = _get_runner()
    blobg = _prep_inputs(x, side, Wq, bq, Wkv, bkv, Wproj, bproj)
    res = runner(blobg)

    ans = np.empty((B, L, N_OUT), np.float32)
    for i in range(8):
        b, j = i // 4, i % 4
        raw = res[i]                                 # [514, 1152] int8
        scales = (raw[2 * CH:].reshape(-1).view(np.float32)[:2 * CH]
                  .reshape(128, 4))              # [partition, chunk]
        vals = raw[:2 * CH].astype(np.float32)   # [512 tokens, 1152]
        for tcn in range(4):
            vals[tcn * 128:(tcn + 1) * 128] *= scales[:, tcn:tcn + 1]
        ans[b, 256 * j:256 * j + 256] = vals[:CH]
        ans[b, 256 * (7 - j):256 * (8 - j)] = vals[CH:]
    # first token: replaced by learned embedding of side[:, 0] (exact, host-side)
    for b in range(B):
        first = side[b, 0].astype(np.float64) @ Wemb.astype(np.float64) + bemb
        ans[b, 0] = (first @ Wproj.astype(np.float64) + bproj).astype(np.float32)
    return ans


# revision 8
# speedup vs baseline: 1.4740x; 1.0160x over previous
"""Trainium2 Bass kernel: AutoregressiveSelfAttention (sparse_attention).

Sharding: 8 cores x 2 pipelined dispatches (one per batch). In each
dispatch all 8 cores work one batch, token-parallel with zigzag causal
load balancing: core j -> query chunks cA = j, cB = 15-j (128 tokens
each). Each core computes the full per-batch KV locally and the output
projection for its 256 query tokens. Host reassembles the 16 disjoint
output slices.

Wire format (dominates wall time through the ~50MB/s axon tunnel): x/side
and ALL weights travel int8 with per-channel f32 scales, dequantized to
bf16 on device; biases f32. Wq/Wk travel UNPADDED (192 cols) and are
scattered on device into the head-padded SBUF layout; biases and all
weight scales ride inside the sharded+gathered weight image instead of
being replicated 8x. The weight image is uploaded to the device mesh ONCE
per kernel call (jax.device_put) and shared by both per-batch dispatches;
each dispatch additionally carries that batch's xsT shard (1/8 per core).
Shards are AllGathered on device over the 8-core group with Shared
outputs, so replicated bytes never cross the host link.

Dispatch path: the relay serves requests FIFO, so batch 0's result-copy
request is issued BETWEEN the two executes — its D2H download then
streams while batch 1's H2D upload/execute proceeds (issuing both copies
after both executes measurably starves the first download until all
uploads drain). Donated output buffers are recycled device-side between
calls via a persistent spare pool (the stock run_bass_kernel_spmd path
re-jits per call and uploads fresh host zeros for donation — both
avoided with a cached jit; the kernel writes every output byte the host
reads, so zero content is unneeded). Output shards are fetched with a
thread pool so per-fetch fixed costs overlap.

Device layouts (per core):
  scores as sT[kv, q] (kv on partitions) so softmax needs no transpose; the
  denominator is folded into the AV matmul via an augmented V (97th channel);
  exp needs no max-subtraction (scores are O(1)).
  k^T/q^T are head-padded to 32-row strips so score matmuls address them in
  place via tile_position; the pad columns of the scattered weight tiles are
  uninitialized garbage, which only ever flows into kpad/qpad pad rows that
  no score matmul reads. Compute instructions here may carry only ONE
  semaphore wait, so DMA-loaded tiles get same-engine pre-touches before
  their consumers (with _legalize_waits as the generic backstop).
  Causal masks are generated on device (iota + a per-core q-offset row),
  the q token columns are sliced out of the gathered xs at runtime via
  partition-id, and the output is quantized per token to int8 (PE
  transpose + abs-max), with the f32 scales embedded in the output
  tensor's tail row.
"""

import sys
from concurrent.futures import ThreadPoolExecutor

sys.path.insert(0, "/opt/trn_rl_repo")

import numpy as np
import ml_dtypes

import jax
import jax.numpy as jnp
from jax.sharding import Mesh, PartitionSpec, NamedSharding
from jax.experimental.shard_map import shard_map

# Persistent XLA compilation cache: without it the whole BIR->NEFF pipeline
# reruns per process (~minutes); with it, repeat dispatches deserialize the
# cached executable.
jax.config.update("jax_compilation_cache_dir", "/tmp/jax_cc_cache")
jax.config.update("jax_persistent_cache_min_compile_time_secs", 0)
jax.config.update("jax_persistent_cache_min_entry_size_bytes", 0)

import concourse.bass as bass
import concourse.mybir as mybir
from concourse.tile import TileContext
from concourse.bass2jax import (
    _bass_exec_p,
    install_neuronx_cc_hook,
    partition_id_tensor,
)

BF16 = mybir.dt.bfloat16
F32 = mybir.dt.float32
NP_BF16 = ml_dtypes.bfloat16
I8 = mybir.dt.int8
AF = mybir.ActivationFunctionType

N_HEAD = 12
N_KQ = 192
N_OUT = 1152
HD_K = 16
HD_V = 96
HD_VA = 97             # v head channels + denominator column
N_VA = N_HEAD * HD_VA  # 1164
N_KP = N_HEAD * 32     # 384: head-padded k/q channel count
B, L = 2, 2048
CH = 128               # query chunk per core per zigzag leg

# ---- packed weight image (flat bytes, sharded 1/8 per core, gathered) ----
WQ_ELS = N_KQ * N_KQ * 2        # 73728: unpadded wq is [384, 192]
WK_ELS = N_OUT * N_KQ           # 221184: unpadded wk is [1152, 192]
WV_ELS = N_OUT * N_VA           # 1340928
O_FBIAS = WQ_ELS + WK_ELS + WV_ELS              # 1635840
N_BIAS = N_KP + N_KP + N_VA + N_OUT             # 3084 f32
O_FWSC = O_FBIAS + N_BIAS * 4                   # 1648176
N_WSC = N_KP + N_OUT + N_OUT                    # 2688 f32 w row scales
O_FWPSC = O_FWSC + N_WSC * 4                    # 1658928
W_BYTES_USED = O_FWPSC + N_OUT * 4              # 1663536
W_ROWS = 816                    # 1671168 bytes: W_BYTES_USED padded to 8*2048
W_SH_ROWS = W_ROWS // 8         # 102
WPH_SH_ROWS = N_OUT // 8        # 144

# ---- wblob layout: per-core weight shard, shared by both dispatches ----
O_WPH_B = W_SH_ROWS * 2048                      # 208896
WBLOB_BYTES = O_WPH_B + WPH_SH_ROWS * N_OUT     # 374784

# ---- xblob layout: per-core per-batch activation shard ----
XS_SH_ROWS = N_OUT // 8          # 144 rows of xsT per core (8-way gather)
O_QOFF = XS_SH_ROWS * 2048       # 294912 (int8 xs shard)
O_XSC = O_QOFF + 2 * CH * 4      # 295936
XBLOB_BYTES = O_XSC + N_OUT * 4  # 300544

MAGIC = 12582912.0  # 1.5 * 2**23: f32 add/sub rounds to nearest integer
OUT_ROWS = 2 * CH + 1  # 256 token rows int8 + 1 row carrying 256 f32 scales

_RUNNER = None

# padded column offset of head h inside the 384-col head-padded layout
_PC = [128 * (h // 4) + 32 * (h % 4) for h in range(N_HEAD)]


def _build_graph():
    nc = bass.Bass(num_devices=8)
    xblob = nc.declare_dram_parameter("xblob", [XBLOB_BYTES], mybir.dt.uint8,
                                      isOutput=False)
    wblob = nc.declare_dram_parameter("wblob", [WBLOB_BYTES], mybir.dt.uint8,
                                      isOutput=False)
    out_d = nc.declare_dram_parameter("out", [OUT_ROWS, N_OUT], mybir.dt.int8,
                                      isOutput=True)

    xbap = xblob.ap()
    xs_sh_ap = xbap[0:O_QOFF].bitcast(I8).rearrange("(p n) -> p n",
                                                    p=XS_SH_ROWS)
    qoff_ap = (xbap[O_QOFF:O_XSC].bitcast(F32)
               .rearrange("(o n) -> o n", o=1))
    xsc_ap = (xbap[O_XSC:XBLOB_BYTES].bitcast(F32)
              .rearrange("(e p) -> p e", e=9))

    wbap = wblob.ap()
    w_sh_ap = wbap[0:O_WPH_B].bitcast(I8).rearrange("(p n) -> p n",
                                                    p=W_SH_ROWS)
    wph_sh_ap = (wbap[O_WPH_B:WBLOB_BYTES].bitcast(I8)
                 .rearrange("(p n) -> p n", p=WPH_SH_ROWS))

    # gather staging + outputs (collectives cannot read IO tensors)
    xs_stage = nc.dram_tensor("xs_stage", [XS_SH_ROWS, 2048], I8)
    w_stage = nc.dram_tensor("w_stage", [W_SH_ROWS, 2048], I8)
    wph_stage = nc.dram_tensor("wph_stage", [WPH_SH_ROWS, N_OUT], I8)
    xs_full = nc.dram_tensor("xs_full", [N_OUT, 2048], I8,
                             addr_space="Shared")
    w_full = nc.dram_tensor("w_full", [W_ROWS, 2048], I8, addr_space="Shared")
    wph_full = nc.dram_tensor("wph_full", [N_OUT, N_OUT], I8,
                              addr_space="Shared")

    # biases and weight scales live in the gathered weight image
    wfl = w_full.ap().flatten()
    bias_ap = wfl[O_FBIAS:O_FWSC].bitcast(F32)
    wsc_ap = wfl[O_FWSC:O_FWPSC].bitcast(F32)
    wpsc_ap = (wfl[O_FWPSC:W_BYTES_USED].bitcast(F32)
               .rearrange("(h p) -> p h", p=96))
    wqsc_ap = wsc_ap[0:N_KP].rearrange("(m p) -> p m", p=128)
    wksc_ap = wsc_ap[N_KP:N_KP + N_OUT].rearrange("(e p) -> p e", p=128)
    wvsc_ap = (wsc_ap[N_KP + N_OUT:N_WSC]
               .rearrange("(e p) -> p e", p=128))
    bq_ap = bias_ap[0:N_KP].rearrange("(m p) -> p m", p=128)
    bk_ap = bias_ap[N_KP:2 * N_KP].rearrange("(m p) -> p m", p=128)
    bv_ap = bias_ap[2 * N_KP:2 * N_KP + N_VA].rearrange("(o n) -> o n", o=1)
    bp_ap = (bias_ap[2 * N_KP + N_VA:N_BIAS]
             .rearrange("(m p) -> p m", p=128))

    with TileContext(nc) as tc, tc.tile_pool(name="resident", bufs=1) as pr:
        # ---- resident tiles ----
        kpad = pr.tile([128, 3, L], BF16)        # k^T head-padded (32 rows/head)
        qpad = pr.tile([128, 3, 2 * CH], BF16)
        v_t = pr.tile([128, L // 128, N_VA], BF16)   # augmented v, token-major
        mC_t = pr.tile([128, 8, 2 * CH], BF16)
        mD_t = pr.tile([128, 8, CH], BF16)
        wph_t = pr.tile([96, 12, N_OUT], BF16)
        bp_t = pr.tile([128, 9], F32)
        id_t = pr.tile([128, 128], F32)      # identity for PE transposes
        yts = [pr.tile([HD_V, 2 * CH], BF16, name=f"yt{h}", tag=f"yt{h}")
               for h in range(N_HEAD)]

        with (
            tc.tile_pool(name="loads", bufs=1) as pw,
            tc.tile_pool(name="xsp", bufs=1) as pxs,
            tc.tile_pool(name="xstage", bufs=2) as pst,
            tc.tile_pool(name="scratch", bufs=1) as psc,
            tc.tile_pool(name="ps_small", bufs=2, space="PSUM") as psp,
            tc.tile_pool(name="ps_v", bufs=2, space="PSUM") as psv,
        ):
            # ---- stage shards, all-gather on device ----
            pid = nc.partition_id()
            nc.sync.dma_start(out=xs_stage.ap(), in_=xs_sh_ap)
            nc.sync.dma_start(out=w_stage.ap(), in_=w_sh_ap)
            nc.sync.dma_start(out=wph_stage.ap(), in_=wph_sh_ap)
            nc.gpsimd.collective_compute(
                "AllGather", mybir.AluOpType.bypass,
                replica_groups=[[0, 1, 2, 3, 4, 5, 6, 7]],
                ins=[xs_stage.ap()], outs=[xs_full.ap()],
            )
            nc.gpsimd.collective_compute(
                "AllGather", mybir.AluOpType.bypass,
                replica_groups=[[0, 1, 2, 3, 4, 5, 6, 7]],
                ins=[w_stage.ap()], outs=[w_full.ap()],
            )
            nc.gpsimd.collective_compute(
                "AllGather", mybir.AluOpType.bypass,
                replica_groups=[[0, 1, 2, 3, 4, 5, 6, 7]],
                ins=[wph_stage.ap()], outs=[wph_full.ap()],
            )

            # ---- SBUF loads; xs dequantized per 128-channel slab ----
            xs_t = pxs.tile([128, 9, L], BF16)
            xsc_t = pw.tile([128, 9], F32)
            nc.sync.dma_start(out=xsc_t, in_=xsc_ap)
            scv0 = pxs.tile([1, 16], F32, tag="scv0")
            nc.vector.tensor_copy(scv0[0:1, 0:1], xsc_t[0:1, 0:1])  # pre-touch
            xsf_r = xs_full.ap().rearrange("(e p) n -> p e n", p=128)
            for e in range(9):
                st8 = pst.tile([128, L], I8, tag="st8")
                nc.sync.dma_start(out=st8, in_=xsf_r[:, e, :])
                nc.vector.tensor_scalar(xs_t[:, e, :], st8,
                                        xsc_t[:, e:e + 1], None,
                                        mybir.AluOpType.mult)
            wqsc_t = pw.tile([128, 3], F32, tag="wqsc")
            nc.sync.dma_start(out=wqsc_t, in_=wqsc_ap)
            wksc_t = pw.tile([128, 9], F32, tag="wksc")
            nc.sync.dma_start(out=wksc_t, in_=wksc_ap)
            wvsc_t = pw.tile([128, 9], F32, tag="wvsc")
            nc.sync.dma_start(out=wvsc_t, in_=wvsc_ap)
            # wq/wk arrive unpadded (192 cols); dequantize then scatter the
            # 16-col head blocks into the head-padded strip layout. The pad
            # columns stay uninitialized — they only feed kpad/qpad pad rows
            # that no score matmul ever reads.
            wq_t = pw.tile([128, 3, N_KP], BF16)
            wq8_t = pw.tile([128, 3, N_KQ], I8, tag="wq8")
            nc.sync.dma_start(
                out=wq8_t,
                in_=wfl[0:WQ_ELS].rearrange("(m p n) -> p m n", m=3, p=128))
            wqf_t = pw.tile([128, 3, N_KQ], BF16, tag="wqf")
            for m in range(3):
                nc.vector.tensor_scalar(wqf_t[:, m, :], wq8_t[:, m, :],
                                        wqsc_t[:, m:m + 1], None,
                                        mybir.AluOpType.mult)
            for h in range(N_HEAD):
                nc.vector.tensor_copy(
                    wq_t[:, :, _PC[h]:_PC[h] + HD_K],
                    wqf_t[:, :, h * HD_K:(h + 1) * HD_K])
            wk_t = pw.tile([128, 9, N_KP], BF16)
            wkf_t = pw.tile([128, 9, N_KQ], BF16, tag="wkf")
            wv_t = pw.tile([128, 9, N_VA], BF16)
            for e in range(9):
                st = pst.tile([128, N_KQ], I8, tag="wk8")
                nc.sync.dma_start(
                    out=st,
                    in_=wfl[WQ_ELS + e * 128 * N_KQ:
                            WQ_ELS + (e + 1) * 128 * N_KQ]
                    .rearrange("(p n) -> p n", p=128))
                nc.vector.tensor_scalar(wkf_t[:, e, :], st,
                                        wksc_t[:, e:e + 1], None,
                                        mybir.AluOpType.mult)
            for h in range(N_HEAD):
                nc.vector.tensor_copy(
                    wk_t[:, :, _PC[h]:_PC[h] + HD_K],
                    wkf_t[:, :, h * HD_K:(h + 1) * HD_K])
            for e in range(9):
                st = pst.tile([128, N_VA], I8, tag="wv8")
                nc.sync.dma_start(
                    out=st,
                    in_=wfl[WQ_ELS + WK_ELS + e * 128 * N_VA:
                            WQ_ELS + WK_ELS + (e + 1) * 128 * N_VA]
                    .rearrange("(p n) -> p n", p=128))
                nc.vector.tensor_scalar(wv_t[:, e, :], st,
                                        wvsc_t[:, e:e + 1], None,
                                        mybir.AluOpType.mult)
            wpsc_t = pw.tile([96, 12], F32, tag="wpsc")
            nc.sync.dma_start(out=wpsc_t, in_=wpsc_ap)
            wphf_r = wph_full.ap().rearrange("(h p) n -> p h n", p=96)
            for h in range(N_HEAD):
                st = pst.tile([96, N_OUT], I8, tag="wph8")
                nc.sync.dma_start(out=st, in_=wphf_r[:, h, :])
                nc.vector.tensor_scalar(wph_t[:, h, :], st,
                                        wpsc_t[:, h:h + 1], None,
                                        mybir.AluOpType.mult)
            qb_t = pw.tile([128, 2 * CH], F32)
            nc.sync.dma_start(out=qb_t, in_=qoff_ap.to_broadcast([128, 2 * CH]))
            bq_t = pw.tile([128, 3], F32)
            nc.sync.dma_start(out=bq_t, in_=bq_ap)
            bk_t = pw.tile([128, 3], F32)
            nc.sync.dma_start(out=bk_t, in_=bk_ap)
            bv_t = pw.tile([128, N_VA], F32)
            nc.sync.dma_start(out=bv_t, in_=bv_ap.to_broadcast([128, N_VA]))
            nc.sync.dma_start(out=bp_t, in_=bp_ap)

            # ---- pre-touches: give each engine 1-wait visibility of loads ----
            dps = psp.tile([128, 512], F32, tag="ps")
            for i, t in enumerate(
                [xs_t[0:1, 0, 0:1], wq_t[0:1, 0, 0:1],
                 wk_t[0:1, 0, 0:1], wv_t[0:1, 0, 0:1], wph_t[0:1, 0, 0:1]]
            ):
                nc.tensor.matmul(dps[0:1, i:i + 1], lhsT=t, rhs=t,
                                 start=True, stop=True)
            sc = psc.tile([1, 16], F32)
            nc.scalar.activation(sc[0:1, 0:1], bq_t[0:1, 0:1], AF.Copy)
            nc.scalar.activation(sc[0:1, 1:2], bk_t[0:1, 0:1], AF.Copy)
            nc.scalar.activation(sc[0:1, 2:3], bp_t[0:1, 0:1], AF.Copy)
            scv = psc.tile([1, 16], F32, tag="scv")
            nc.vector.tensor_copy(scv[0:1, 0:1], bv_t[0:1, 0:1])
            nc.vector.tensor_copy(scv[0:1, 1:2], qb_t[0:1, 0:1])
            # ACT warm-up of Exp's implicit const-bias AP
            sce = psc.tile([1, 16], F32, tag="sce")
            nc.scalar.activation(sce[0:1, 0:1], scv[0:1, 0:1], AF.Exp)

            # ---- mask gen: m[p, f] = (qidx[f] - (128*kt + p) > 0) ----
            ci_t = pw.tile([128, 2 * CH], mybir.dt.int32)
            nc.gpsimd.iota(ci_t, pattern=[[1, 2 * CH]], base=0,
                           channel_multiplier=-1)
            cif_t = pw.tile([128, 2 * CH], F32)
            nc.vector.tensor_copy(cif_t, ci_t)
            mb_t = pw.tile([128, 2 * CH], F32)
            nc.vector.tensor_add(mb_t, cif_t, qb_t)
            for kt in range(8):
                nc.vector.tensor_scalar(
                    mC_t[:, kt, :], mb_t, float(128 * kt), None,
                    mybir.AluOpType.is_gt)
            for kt in range(8, 16):
                nc.vector.tensor_scalar(
                    mD_t[:, kt - 8, :], mb_t[:, CH:], float(128 * kt), None,
                    mybir.AluOpType.is_gt)
            nc.vector.tensor_scalar(id_t, cif_t[:, :128], 0.0, None,
                                    mybir.AluOpType.is_equal)

            # ---- q projection: qpad[384, 256]; q-token columns are sliced
            #      out of the gathered xs at runtime via partition-id ----
            offA = nc.s_assert_within(pid * CH, 0, L,
                                      skip_runtime_assert=True)
            offB = nc.s_assert_within((15 - pid) * CH, 0, L,
                                      skip_runtime_assert=True)
            sq_t = pw.tile([128, 3, 2 * CH], BF16)
            for e in range(3):
                nc.vector.tensor_copy(sq_t[:, e, :CH],
                                      xs_t[:, 6 + e, bass.ds(offA, CH)])
                nc.vector.tensor_copy(sq_t[:, e, CH:],
                                      xs_t[:, 6 + e, bass.ds(offB, CH)])
            for m in range(3):
                ps = psp.tile([128, 2 * CH], F32, tag="ps")
                for e in range(3):
                    nc.tensor.matmul(
                        ps, lhsT=wq_t[:, e, m * 128:(m + 1) * 128],
                        rhs=sq_t[:, e, :],
                        start=(e == 0), stop=(e == 2),
                    )
                nc.scalar.activation(qpad[:, m, :], ps, AF.Identity,
                                     bias=bq_t[:, m:m + 1])

            # ---- k projection: kpad[384, 2048], 512-token slabs ----
            for m in range(3):
                for nt in range(L // 512):
                    ps = psp.tile([128, 512], F32, tag="ps")
                    for e in range(9):
                        nc.tensor.matmul(
                            ps,
                            lhsT=wk_t[:, e, m * 128:(m + 1) * 128],
                            rhs=xs_t[:, e, nt * 512:(nt + 1) * 512],
                            start=(e == 0), stop=(e == 8),
                        )
                    nc.scalar.activation(
                        kpad[:, m, nt * 512:(nt + 1) * 512], ps, AF.Identity,
                        bias=bk_t[:, m:m + 1],
                    )

            # ---- v projection: v[2048, 1164] (token-major, augmented) ----
            for c in range(L // 128):
                ps = psv.tile([128, N_VA], F32, tag="vps")
                for e in range(9):
                    for n0, nn in [(0, 512), (512, 512), (1024, N_VA - 1024)]:
                        nc.tensor.matmul(
                            ps[:, n0:n0 + nn],
                            lhsT=xs_t[:, e, c * 128:(c + 1) * 128],
                            rhs=wv_t[:, e, n0:n0 + nn],
                            start=(e == 0), stop=(e == 8),
                        )
                nc.vector.tensor_add(v_t[:, c, :], ps, bv_t)

        # ---- attention ----
        with (
            tc.tile_pool(name="ps_s", bufs=4, space="PSUM") as pss,
            tc.tile_pool(name="ps_y", bufs=3, space="PSUM") as psy,
            tc.tile_pool(name="exps", bufs=20) as pe,
            tc.tile_pool(name="norm", bufs=2) as pn,
            tc.tile_pool(name="rdram", bufs=6, space="DRAM") as pdram,
        ):
            for h in range(N_HEAD):
                t, a = h // 4, 32 * (h % 4)
                ems = []
                for kt in range(8):
                    s_ps = pss.tile([128, 2 * CH], F32, tag="sps")
                    nc.tensor.matmul(
                        s_ps,
                        lhsT=kpad[a:a + HD_K, t, kt * 128:(kt + 1) * 128],
                        rhs=qpad[a:a + HD_K, t, :],
                        start=True, stop=True,
                        tile_position=(a, 0),
                    )
                    e_sb = pe.tile([128, 2 * CH], BF16, tag="esb")
                    nc.scalar.activation(e_sb, s_ps, AF.Exp, scale=0.25)
                    em_sb = pe.tile([128, 2 * CH], BF16, tag="emsb")
                    nc.vector.tensor_mul(em_sb, e_sb, mC_t[:, kt, :])
                    ems.append(em_sb)
                for kt in range(8, 16):
                    s_ps = pss.tile([128, 2 * CH], F32, tag="sps")
                    nc.tensor.matmul(
                        s_ps[:, :CH],
                        lhsT=kpad[a:a + HD_K, t, kt * 128:(kt + 1) * 128],
                        rhs=qpad[a:a + HD_K, t, CH:],
                        start=True, stop=True,
                        tile_position=(a, 0),
                    )
                    e_sb = pe.tile([128, 2 * CH], BF16, tag="esb")
                    nc.scalar.activation(e_sb[:, :CH], s_ps[:, :CH], AF.Exp,
                                         scale=0.25)
                    em_sb = pe.tile([128, 2 * CH], BF16, tag="emsb")
                    nc.vector.tensor_mul(em_sb[:, :CH], e_sb[:, :CH],
                                         mD_t[:, kt - 8, :])
                    ems.append(em_sb)
                y_ps = psy.tile([HD_VA, 2 * CH], F32, tag="yps")
                for kt in range(8):
                    nc.tensor.matmul(
                        y_ps,
                        lhsT=v_t[:, kt, h * HD_VA:(h + 1) * HD_VA],
                        rhs=ems[kt],
                        start=(kt == 0), stop=False,
                    )
                for kt in range(8, 16):
                    nc.tensor.matmul(
                        y_ps[:, CH:],
                        lhsT=v_t[:, kt, h * HD_VA:(h + 1) * HD_VA],
                        rhs=ems[kt][:, :CH],
                        start=False, stop=(kt == 15),
                    )
                # normalize: row 96 of y_ps is the softmax denominator
                # (clamped away from 0 so the dead q=0 column yields 0, not NaN)
                r_sb = pn.tile([128, 2 * CH], F32, tag="rsb")
                rmx = pn.tile([128, 2 * CH], F32, tag="rmx")
                nc.vector.tensor_scalar_max(rmx[96:97, :], y_ps[96:97, :],
                                            1e-30)
                nc.vector.reciprocal(r_sb[96:97, :], rmx[96:97, :])
                rd = pdram.tile([1, 2 * CH], F32, tag="rd")
                nc.sync.dma_start(out=rd, in_=r_sb[96:97, :])
                rb_t = pn.tile([HD_V, 2 * CH], F32, tag="rbt")
                nc.sync.dma_start(
                    out=rb_t, in_=rd[0:1, :].to_broadcast([HD_V, 2 * CH])
                )
                rtc = pn.tile([1, 1], F32, tag="rtc")
                nc.vector.tensor_copy(rtc, rb_t[0:1, 0:1])  # pre-touch
                nc.vector.tensor_mul(yts[h], y_ps[:HD_V, :], rb_t)

        # ---- output projection: outT[1152, 256] = sum_h Wp_h^T @ y_h,
        #      then per-token int8 quantization: transpose, abs-max, scale ----
        with (
            tc.tile_pool(name="ps_o", bufs=2, space="PSUM") as pso,
            tc.tile_pool(name="ps_q", bufs=2, space="PSUM") as psq,
            tc.tile_pool(name="qsb", bufs=3) as pq,
            tc.tile_pool(name="qsc", bufs=1) as pqs,
        ):
            outb = pqs.tile([128, 9, 2 * CH], F32, tag="outb")
            for mo in range(9):
                ps = pso.tile([128, 2 * CH], F32)
                for h in range(N_HEAD):
                    nc.tensor.matmul(
                        ps,
                        lhsT=wph_t[:, h, mo * 128:(mo + 1) * 128],
                        rhs=yts[h],
                        start=(h == 0), stop=(h == N_HEAD - 1),
                    )
                nc.scalar.activation(outb[:, mo, :], ps, AF.Identity,
                                     bias=bp_t[:, mo:mo + 1])
            sc_all = pqs.tile([128, 2], F32)
            rcp = pqs.tile([128, 2], F32, tag="rcp")
            mxs = pqs.tile([128, 2], F32, tag="mxs")
            for tcn in range(2):
                psT = psq.tile([128, N_OUT], F32, tag="psT")
                for mo in range(9):
                    nc.tensor.matmul(
                        psT[:, mo * 128:(mo + 1) * 128],
                        lhsT=outb[:, mo, tcn * 128:(tcn + 1) * 128],
                        rhs=id_t, is_transpose=True,
                        start=True, stop=True,
                    )
                nc.vector.tensor_reduce(
                    mxs[:, tcn:tcn + 1], psT, axis=mybir.AxisListType.X,
                    op=mybir.AluOpType.max, apply_absolute_value=True)
                nc.vector.tensor_scalar_mul(sc_all[:, tcn:tcn + 1],
                                            mxs[:, tcn:tcn + 1], 1.0 / 127.0)
                nc.vector.reciprocal(rcp[:, tcn:tcn + 1],
                                     sc_all[:, tcn:tcn + 1])
                qf = pq.tile([128, N_OUT], F32, tag="qf")
                nc.vector.tensor_scalar(qf, psT, rcp[:, tcn:tcn + 1], MAGIC,
                                        mybir.AluOpType.mult,
                                        mybir.AluOpType.add)
                qg = pq.tile([128, N_OUT], F32, tag="qg")
                nc.vector.tensor_scalar(qg, qf, MAGIC, None,
                                        mybir.AluOpType.subtract)
                qi = pq.tile([128, N_OUT], mybir.dt.int8, tag="qi")
                nc.vector.tensor_copy(qi, qg)
                nc.sync.dma_start(
                    out=out_d.ap()[tcn * 128:(tcn + 1) * 128, :], in_=qi)
            sc_dst = (out_d.ap()[2 * CH:OUT_ROWS, :].flatten()[0:2 * CH * 4]
                      .bitcast(F32).rearrange("(p n) -> p n", p=128))
            nc.sync.dma_start(out=sc_dst, in_=sc_all)
    return nc


def _legalize_waits(nc):
    """This walrus build accepts only ONE sync-wait per regular instruction;
    move overflow waits onto injected same-engine NoOps (like raw-bass
    wait_ge)."""
    keep = ("InstEventSemaphore",)
    cnt = 0
    for bbh in nc.bb_map.values():
        bb = bbh.bb
        new_list = []
        for inst in bb.instructions:
            si = inst.sync_info
            if (si is not None and len(si.on_wait) > 1
                    and type(inst).__name__ not in keep):
                waits = list(si.on_wait)
                for w in waits[:-1]:
                    cnt += 1
                    n = mybir.InstNoOp(name=f"legwait_{cnt}", ins=[], outs=[])
                    n.engine = inst.engine
                    n.sync_info = mybir.SyncInfo(on_wait=[w], on_update=[])
                    try:
                        nc.register_instruction(n)
                    except Exception:
                        pass
                    new_list.append(n)
                inst.sync_info = mybir.SyncInfo(
                    on_wait=[waits[-1]], on_update=list(si.on_update))
            new_list.append(inst)
        bb.instructions = new_list
    return cnt


class _Runner:
    """Cached 2-dispatch pipeline. Replicates run_bass_via_pjrt's
    _bass_exec_p lowering with a cached jit; uploads the weight image once
    per call (shared device array); issues batch 0's result-copy request
    BETWEEN the two executes so its download streams during batch 1's
    upload/execute (the relay serves requests FIFO); recycles previous
    outputs as donated buffers; fetches output shards with a thread pool."""

    def __init__(self, nc):
        install_neuronx_cc_hook()
        self.nc = nc
        partition_name = (nc.partition_id_tensor.name
                          if nc.partition_id_tensor else None)
        in_names, out_names, out_avals = [], [], []
        for alloc in nc.m.functions[0].allocations:
            if not isinstance(alloc, mybir.MemoryLocationSet):
                continue
            name = alloc.memorylocations[0].name
            if alloc.kind == "ExternalInput":
                if name != partition_name:
                    in_names.append(name)
            elif alloc.kind == "ExternalOutput":
                out_names.append(name)
                out_avals.append(jax.core.ShapedArray(
                    tuple(alloc.tensor_shape), mybir.dt.np(alloc.dtype)))
        n_params, n_outs = len(in_names), len(out_avals)
        assert in_names == ["xblob", "wblob"] and out_names == ["out"]
        in_names_all = in_names + out_names + (
            [partition_name] if partition_name else [])

        def _body(*args):
            operands = list(args)
            if partition_name is not None:
                operands.append(partition_id_tensor())
            return tuple(_bass_exec_p.bind(
                *operands, out_avals=tuple(out_avals),
                in_names=tuple(in_names_all), out_names=tuple(out_names),
                lowering_input_output_aliases=(), sim_require_finite=True,
                sim_require_nnan=True, nc=nc))

        devices = jax.devices()[:8]
        assert len(devices) == 8
        self.mesh = Mesh(np.asarray(devices), ("core",))
        self.sharding = NamedSharding(self.mesh, PartitionSpec("core"))
        donate = tuple(range(n_params, n_params + n_outs))
        self.sharded = jax.jit(
            shard_map(_body, mesh=self.mesh,
                      in_specs=(PartitionSpec("core"),) * (n_params + n_outs),
                      out_specs=(PartitionSpec("core"),) * n_outs,
                      check_rep=False),
            donate_argnums=donate, keep_unused=True)
        self.zmaker = jax.jit(
            lambda: jnp.zeros((8 * OUT_ROWS, N_OUT), jnp.int8),
            out_shardings=self.sharding)
        self.spares = None
        self.pool = ThreadPoolExecutor(8)

    def warm(self):
        """Seed the donated-buffer pool on device (no host traffic)."""
        if self.spares is None:
            s0, s1 = self.zmaker(), self.zmaker()
            jax.block_until_ready((s0, s1))
            self.spares = (s0, s1)

    def _fetch(self, o):
        shards = sorted(o.addressable_shards,
                        key=lambda sh: sh.index[0].start
                        if sh.index[0].start else 0)
        assert len(shards) == 8
        return list(self.pool.map(lambda sh: np.asarray(sh.data), shards))

    def __call__(self, xg0, xg1, wg):
        """Full device computation: returns per-core raw outputs for each
        batch, 8 x np.int8 [OUT_ROWS, N_OUT] per batch."""
        self.warm()
        s0, s1 = self.spares
        self.spares = None
        wdev = jax.device_put(wg, self.sharding)
        (o0,) = self.sharded(xg0, wdev, s0)
        o0.copy_to_host_async()  # MUST precede call 2: relay serves FIFO
        (o1,) = self.sharded(xg1, wdev, s1)
        o1.copy_to_host_async()
        parts0 = self._fetch(o0)
        parts1 = self._fetch(o1)
        self.spares = (o0, o1)  # recycle device buffers for next donation
        return parts0, parts1


def _get_runner():
    global _RUNNER
    if _RUNNER is None:
        nc = _build_graph()
        _legalize_waits(nc)
        # The pjrt lowering re-serializes the (frozen, never-mutated) graph
        # on every trace (~25ms for this BIR); memoize the identical bytes.
        raw = nc.to_json_bytes()
        nc.to_json_bytes = lambda: raw
        _RUNNER = _Runner(nc)
    return _RUNNER


def _head_pad_bias(b):
    """[192] -> [384] with head h vals at 128*(h//4)+32*(h%4)."""
    bp = np.zeros((N_KP,), np.float32)
    for h in range(N_HEAD):
        bp[_PC[h]:_PC[h] + HD_K] = b[h * HD_K:(h + 1) * HD_K]
    return bp


def _prep_inputs(x, side, Wq, bq, Wkv, bkv, Wproj, bproj):
    """Quantize + pack the wire blobs: returns (xg0, xg1, wg) global uint8
    arrays (concat of the 8 per-core shards along axis 0)."""
    Wk = Wkv[:, :N_KQ]
    Wv = Wkv[:, N_KQ:]
    bk = bkv[:N_KQ]
    bv = bkv[N_KQ:]
    bq_p = _head_pad_bias(bq)
    bk_p = _head_pad_bias(bk)
    # augmented V: per head 96 channels + a zero-weight/one-bias denom channel
    Wv_a = np.zeros((N_OUT, N_VA), np.float32)
    bv_a = np.zeros((N_VA,), np.float32)
    for h in range(N_HEAD):
        Wv_a[:, h * HD_VA:h * HD_VA + HD_V] = Wv[:, h * HD_V:(h + 1) * HD_V]
        bv_a[h * HD_VA:h * HD_VA + HD_V] = bv[h * HD_V:(h + 1) * HD_V]
        bv_a[h * HD_VA + HD_V] = 1.0

    # packed int8 q/k/v weights (per input-channel-row scales), unpadded
    def q8_rows(W):
        sc = np.maximum(np.abs(W).max(axis=1), 1e-30) / 127.0
        q = np.clip(np.round(W / sc[:, None]), -127, 127).astype(np.int8)
        return q, sc.astype(np.float32)

    wq8, wqsc = q8_rows(Wq)          # [384, 192]
    wk8, wksc = q8_rows(Wk)          # [1152, 192]
    wv8, wvsc = q8_rows(Wv_a)        # [1152, 1164]
    wscales = np.concatenate([wqsc, wksc, wvsc]).astype(np.float32)
    biases = np.concatenate([bq_p, bk_p, bv_a, bproj]).astype(np.float32)

    # Wproj rows per head, int8 per-row [1152, 1152]
    wph_all, wphsc = q8_rows(Wproj.reshape(N_HEAD * HD_V, N_OUT))

    wbuf = np.zeros((W_ROWS * 2048,), np.uint8)
    wbuf[0:WQ_ELS] = wq8.reshape(-1).view(np.uint8)
    wbuf[WQ_ELS:WQ_ELS + WK_ELS] = wk8.reshape(-1).view(np.uint8)
    wbuf[WQ_ELS + WK_ELS:O_FBIAS] = wv8.reshape(-1).view(np.uint8)
    wbuf[O_FBIAS:O_FWSC] = biases.view(np.uint8)
    wbuf[O_FWSC:O_FWPSC] = wscales.view(np.uint8)
    wbuf[O_FWPSC:W_BYTES_USED] = wphsc.view(np.uint8)

    wg = np.empty((8, WBLOB_BYTES), np.uint8)
    for i in range(8):
        wg[i, 0:O_WPH_B] = wbuf[W_SH_ROWS * 2048 * i:
                                W_SH_ROWS * 2048 * (i + 1)]
        wg[i, O_WPH_B:WBLOB_BYTES] = (
            wph_all[WPH_SH_ROWS * i:WPH_SH_ROWS * (i + 1), :]
            .reshape(-1).view(np.uint8))

    # per-channel int8 scales for [x|side]^T, shared by all cores of a batch
    xgs = []
    for b in range(B):
        xsT = np.ascontiguousarray(np.concatenate([x[b], side[b]], axis=1).T)
        xsc = np.maximum(np.abs(xsT).max(axis=1), 1e-30) / 127.0
        xsq = np.clip(np.round(xsT / xsc[:, None]), -127, 127).astype(np.int8)
        xscf = xsc.astype(np.float32)
        xg = np.empty((8, XBLOB_BYTES), np.uint8)
        for i in range(8):
            # qidx[f] - f for the mask generator: q token of em column f
            qoff = np.empty((2 * CH,), np.float32)
            qoff[:CH] = CH * i
            qoff[CH:] = CH * (15 - i) - CH
            xg[i, 0:O_QOFF] = (xsq[XS_SH_ROWS * i:XS_SH_ROWS * (i + 1), :]
                               .reshape(-1).view(np.uint8))
            xg[i, O_QOFF:O_XSC] = qoff.view(np.uint8)
            xg[i, O_XSC:XBLOB_BYTES] = xscf.view(np.uint8)
        xgs.append(xg.reshape(-1))
    return xgs[0], xgs[1], wg.reshape(-1)


def _unpack(parts, ans, b):
    """parts: 8 x [OUT_ROWS, N_OUT] int8 for batch b -> ans[b] float32."""
    for i in range(8):
        core = parts[i]
        scales = (core[2 * CH:].reshape(-1).view(np.float32)[:2 * CH]
                  .reshape(128, 2))              # [partition, chunk]
        vals = core[:2 * CH].astype(np.float32)  # [256 tokens, 1152]
        for tcn in range(2):
            vals[tcn * 128:(tcn + 1) * 128] *= scales[:, tcn:tcn + 1]
        ans[b, CH * i:CH * (i + 1)] = vals[:CH]
        ans[b, CH * (15 - i):CH * (16 - i)] = vals[CH:]


def kernel(x, side, Wq, bq, Wkv, bkv, Wproj, bproj, Wemb, bemb, **_unused):
    x = np.asarray(x, np.float32)
    side = np.asarray(side, np.float32)
    Wq = np.asarray(Wq, np.float32)
    bq = np.asarray(bq, np.float32)
    Wkv = np.asarray(Wkv, np.float32)
    bkv = np.asarray(bkv, np.float32)
    Wproj = np.asarray(Wproj, np.float32)
    bproj = np.asarray(bproj, np.float32)
    Wemb = np.asarray(Wemb, np.float32)
    bemb = np.asarray(bemb, np.float32)

    runner = _get_runner()
    xg0, xg1, wg = _prep_inputs(x, side, Wq, bq, Wkv, bkv, Wproj, bproj)
    parts0, parts1 = runner(xg0, xg1, wg)

    ans = np.empty((B, L, N_OUT), np.float32)
    _unpack(parts0, ans, 0)
    _unpack(parts1, ans, 1)
    # first token: replaced by learned embedding of side[:, 0] (exact, host-side)
    for b in range(B):
        first = side[b, 0].astype(np.float64) @ Wemb.astype(np.float64) + bemb
        ans[b, 0] = (first @ Wproj.astype(np.float64) + bproj).astype(np.float32)
    return ans


# revision 9
# speedup vs baseline: 1.4892x; 1.0103x over previous
"""Trainium2 Bass kernel: AutoregressiveSelfAttention (sparse_attention).

Sharding: 8 cores x 2 pipelined dispatches (one per batch). In each
dispatch all 8 cores work one batch, token-parallel with zigzag causal
load balancing: core j -> query chunks cA = j, cB = 15-j (128 tokens
each). Each core computes the full per-batch KV locally and the output
projection for its 256 query tokens. Host reassembles the 16 disjoint
output slices.

Wire format (dominates wall time through the ~50MB/s axon tunnel): x/side
and ALL weights travel int8 with per-channel f32 scales, dequantized to
bf16 on device; biases f32. Wq/Wk travel UNPADDED (192 cols) and are
scattered on device into the head-padded SBUF layout; biases and all
weight scales ride inside the sharded+gathered weight image instead of
being replicated 8x. The weight image is uploaded to the device mesh ONCE
per kernel call (jax.device_put) and shared by both per-batch dispatches;
each dispatch additionally carries that batch's xsT shard (1/8 per core).
Shards are AllGathered on device over the 8-core group with Shared
outputs, so replicated bytes never cross the host link.

Dispatch path: the relay serves requests FIFO, so batch 0's result-copy
request is issued BETWEEN the two executes — its D2H download then
streams while batch 1's H2D upload/execute proceeds (issuing both copies
after both executes measurably starves the first download until all
uploads drain). Donated output buffers are recycled device-side between
calls via a persistent spare pool (the stock run_bass_kernel_spmd path
re-jits per call and uploads fresh host zeros for donation — both
avoided with a cached jit; the kernel writes every output byte the host
reads, so zero content is unneeded). Output shards are fetched with a
thread pool so per-fetch fixed costs overlap.

Device layouts (per core):
  scores as sT[kv, q] (kv on partitions) so softmax needs no transpose; the
  denominator is folded into the AV matmul via an augmented V (97th channel);
  exp needs no max-subtraction (scores are O(1)).
  k^T/q^T are head-padded to 32-row strips so score matmuls address them in
  place via tile_position; the pad columns of the scattered weight tiles are
  uninitialized garbage, which only ever flows into kpad/qpad pad rows that
  no score matmul reads. Compute instructions here may carry only ONE
  semaphore wait, so DMA-loaded tiles get same-engine pre-touches before
  their consumers (with _legalize_waits as the generic backstop).
  Causal masks are generated on device (iota + a per-core q-offset row),
  the q token columns are sliced out of the gathered xs at runtime via
  partition-id, and the output is quantized per token to int8 (PE
  transpose + abs-max), with the f32 scales embedded in the output
  tensor's tail row.
"""

import sys
from concurrent.futures import ThreadPoolExecutor

sys.path.insert(0, "/opt/trn_rl_repo")

import numpy as np
import ml_dtypes

import jax
import jax.numpy as jnp
from jax.sharding import Mesh, PartitionSpec, NamedSharding
from jax.experimental.shard_map import shard_map

# Persistent XLA compilation cache: without it the whole BIR->NEFF pipeline
# reruns per process (~minutes); with it, repeat dispatches deserialize the
# cached executable.
jax.config.update("jax_compilation_cache_dir", "/tmp/jax_cc_cache")
jax.config.update("jax_persistent_cache_min_compile_time_secs", 0)
jax.config.update("jax_persistent_cache_min_entry_size_bytes", 0)

import concourse.bass as bass
import concourse.mybir as mybir
from concourse.tile import TileContext
from concourse.bass2jax import (
    _bass_exec_p,
    install_neuronx_cc_hook,
    partition_id_tensor,
)

BF16 = mybir.dt.bfloat16
F32 = mybir.dt.float32
NP_BF16 = ml_dtypes.bfloat16
I8 = mybir.dt.int8
AF = mybir.ActivationFunctionType

N_HEAD = 12
N_KQ = 192
N_OUT = 1152
HD_K = 16
HD_V = 96
HD_VA = 97             # v head channels + denominator column
N_VA = N_HEAD * HD_VA  # 1164
N_KP = N_HEAD * 32     # 384: head-padded k/q channel count
B, L = 2, 2048
CH = 128               # query chunk per core per zigzag leg

# ---- packed weight image (flat bytes, sharded 1/8 per core, gathered) ----
WQ_ELS = N_KQ * N_KQ * 2        # 73728: unpadded wq is [384, 192]
WK_ELS = N_OUT * N_KQ           # 221184: unpadded wk is [1152, 192]
WV_ELS = N_OUT * N_VA           # 1340928
O_FBIAS = WQ_ELS + WK_ELS + WV_ELS              # 1635840
N_BIAS = N_KP + N_KP + N_VA + N_OUT             # 3084 f32
O_FWSC = O_FBIAS + N_BIAS * 4                   # 1648176
N_WSC = N_KP + N_OUT + N_OUT                    # 2688 f32 w row scales
O_FWPSC = O_FWSC + N_WSC * 4                    # 1658928
W_BYTES_USED = O_FWPSC + N_OUT * 4              # 1663536
W_ROWS = 816                    # 1671168 bytes: W_BYTES_USED padded to 8*2048
W_SH_ROWS = W_ROWS // 8         # 102
WPH_SH_ROWS = N_OUT // 8        # 144

# ---- wblob layout: per-core weight shard, shared by both dispatches ----
O_WPH_B = W_SH_ROWS * 2048                      # 208896
WBLOB_BYTES = O_WPH_B + WPH_SH_ROWS * N_OUT     # 374784

# ---- xblob layout: per-core per-batch activation shard ----
XS_SH_ROWS = N_OUT // 8          # 144 rows of xsT per core (8-way gather)
O_QOFF = XS_SH_ROWS * 2048       # 294912 (int8 xs shard)
O_XSC = O_QOFF + 2 * CH * 4      # 295936
XBLOB_BYTES = O_XSC + N_OUT * 4  # 300544

MAGIC = 12582912.0  # 1.5 * 2**23: f32 add/sub rounds to nearest integer
OUT_ROWS = 2 * CH + 1  # 256 token rows int8 + 1 row carrying 256 f32 scales

_RUNNER = None

# padded column offset of head h inside the 384-col head-padded layout
_PC = [128 * (h // 4) + 32 * (h % 4) for h in range(N_HEAD)]


def _build_graph():
    nc = bass.Bass(num_devices=8)
    xblob = nc.declare_dram_parameter("xblob", [XBLOB_BYTES], mybir.dt.uint8,
                                      isOutput=False)
    wblob = nc.declare_dram_parameter("wblob", [WBLOB_BYTES], mybir.dt.uint8,
                                      isOutput=False)
    out_d = nc.declare_dram_parameter("out", [OUT_ROWS, N_OUT], mybir.dt.int8,
                                      isOutput=True)

    xbap = xblob.ap()
    xs_sh_ap = xbap[0:O_QOFF].bitcast(I8).rearrange("(p n) -> p n",
                                                    p=XS_SH_ROWS)
    qoff_ap = (xbap[O_QOFF:O_XSC].bitcast(F32)
               .rearrange("(o n) -> o n", o=1))
    xsc_ap = (xbap[O_XSC:XBLOB_BYTES].bitcast(F32)
              .rearrange("(e p) -> p e", e=9))

    wbap = wblob.ap()
    w_sh_ap = wbap[0:O_WPH_B].bitcast(I8).rearrange("(p n) -> p n",
                                                    p=W_SH_ROWS)
    wph_sh_ap = (wbap[O_WPH_B:WBLOB_BYTES].bitcast(I8)
                 .rearrange("(p n) -> p n", p=WPH_SH_ROWS))

    # gather staging + outputs (collectives cannot read IO tensors)
    xs_stage = nc.dram_tensor("xs_stage", [XS_SH_ROWS, 2048], I8)
    w_stage = nc.dram_tensor("w_stage", [W_SH_ROWS, 2048], I8)
    wph_stage = nc.dram_tensor("wph_stage", [WPH_SH_ROWS, N_OUT], I8)
    xs_full = nc.dram_tensor("xs_full", [N_OUT, 2048], I8,
                             addr_space="Shared")
    w_full = nc.dram_tensor("w_full", [W_ROWS, 2048], I8, addr_space="Shared")
    wph_full = nc.dram_tensor("wph_full", [N_OUT, N_OUT], I8,
                              addr_space="Shared")

    # biases and weight scales live in the gathered weight image
    wfl = w_full.ap().flatten()
    bias_ap = wfl[O_FBIAS:O_FWSC].bitcast(F32)
    wsc_ap = wfl[O_FWSC:O_FWPSC].bitcast(F32)
    wpsc_ap = (wfl[O_FWPSC:W_BYTES_USED].bitcast(F32)
               .rearrange("(h p) -> p h", p=96))
    wqsc_ap = wsc_ap[0:N_KP].rearrange("(m p) -> p m", p=128)
    wksc_ap = wsc_ap[N_KP:N_KP + N_OUT].rearrange("(e p) -> p e", p=128)
    wvsc_ap = (wsc_ap[N_KP + N_OUT:N_WSC]
               .rearrange("(e p) -> p e", p=128))
    bq_ap = bias_ap[0:N_KP].rearrange("(m p) -> p m", p=128)
    bk_ap = bias_ap[N_KP:2 * N_KP].rearrange("(m p) -> p m", p=128)
    bv_ap = bias_ap[2 * N_KP:2 * N_KP + N_VA].rearrange("(o n) -> o n", o=1)
    bp_ap = (bias_ap[2 * N_KP + N_VA:N_BIAS]
             .rearrange("(m p) -> p m", p=128))

    with TileContext(nc) as tc, tc.tile_pool(name="resident", bufs=1) as pr:
        # ---- resident tiles ----
        kpad = pr.tile([128, 3, L], BF16)        # k^T head-padded (32 rows/head)
        qpad = pr.tile([128, 3, 2 * CH], BF16)
        v_t = pr.tile([128, L // 128, N_VA], BF16)   # augmented v, token-major
        mC_t = pr.tile([128, 8, 2 * CH], BF16)
        mD_t = pr.tile([128, 8, CH], BF16)
        wph_t = pr.tile([96, 12, N_OUT], BF16)
        bp_t = pr.tile([128, 9], F32)
        id_t = pr.tile([128, 128], F32)      # identity for PE transposes
        yts = [pr.tile([HD_V, 2 * CH], BF16, name=f"yt{h}", tag=f"yt{h}")
               for h in range(N_HEAD)]

        with (
            tc.tile_pool(name="loads", bufs=1) as pw,
            tc.tile_pool(name="xsp", bufs=1) as pxs,
            tc.tile_pool(name="xstage", bufs=2) as pst,
            tc.tile_pool(name="scratch", bufs=1) as psc,
            tc.tile_pool(name="ps_small", bufs=2, space="PSUM") as psp,
            tc.tile_pool(name="ps_v", bufs=2, space="PSUM") as psv,
        ):
            # ---- stage shards, all-gather on device ----
            pid = nc.partition_id()
            nc.sync.dma_start(out=xs_stage.ap(), in_=xs_sh_ap)
            nc.sync.dma_start(out=w_stage.ap(), in_=w_sh_ap)
            nc.sync.dma_start(out=wph_stage.ap(), in_=wph_sh_ap)
            nc.gpsimd.collective_compute(
                "AllGather", mybir.AluOpType.bypass,
                replica_groups=[[0, 1, 2, 3, 4, 5, 6, 7]],
                ins=[xs_stage.ap()], outs=[xs_full.ap()],
            )
            nc.gpsimd.collective_compute(
                "AllGather", mybir.AluOpType.bypass,
                replica_groups=[[0, 1, 2, 3, 4, 5, 6, 7]],
                ins=[w_stage.ap()], outs=[w_full.ap()],
            )
            nc.gpsimd.collective_compute(
                "AllGather", mybir.AluOpType.bypass,
                replica_groups=[[0, 1, 2, 3, 4, 5, 6, 7]],
                ins=[wph_stage.ap()], outs=[wph_full.ap()],
            )

            # ---- SBUF loads; xs dequantized per 128-channel slab ----
            xs_t = pxs.tile([128, 9, L], BF16)
            xsc_t = pw.tile([128, 9], F32)
            nc.sync.dma_start(out=xsc_t, in_=xsc_ap)
            scv0 = pxs.tile([1, 16], F32, tag="scv0")
            nc.vector.tensor_copy(scv0[0:1, 0:1], xsc_t[0:1, 0:1])  # pre-touch
            xsf_r = xs_full.ap().rearrange("(e p) n -> p e n", p=128)
            for e in range(9):
                st8 = pst.tile([128, L], I8, tag="st8")
                nc.sync.dma_start(out=st8, in_=xsf_r[:, e, :])
                nc.vector.tensor_scalar(xs_t[:, e, :], st8,
                                        xsc_t[:, e:e + 1], None,
                                        mybir.AluOpType.mult)
            wqsc_t = pw.tile([128, 3], F32, tag="wqsc")
            nc.sync.dma_start(out=wqsc_t, in_=wqsc_ap)
            wksc_t = pw.tile([128, 9], F32, tag="wksc")
            nc.sync.dma_start(out=wksc_t, in_=wksc_ap)
            wvsc_t = pw.tile([128, 9], F32, tag="wvsc")
            nc.sync.dma_start(out=wvsc_t, in_=wvsc_ap)
            # wq/wk arrive unpadded (192 cols); dequantize then scatter the
            # 16-col head blocks into the head-padded strip layout. The pad
            # columns stay uninitialized — they only feed kpad/qpad pad rows
            # that no score matmul ever reads.
            wq_t = pw.tile([128, 3, N_KP], BF16)
            wq8_t = pw.tile([128, 3, N_KQ], I8, tag="wq8")
            nc.sync.dma_start(
                out=wq8_t,
                in_=wfl[0:WQ_ELS].rearrange("(m p n) -> p m n", m=3, p=128))
            wqf_t = pw.tile([128, 3, N_KQ], BF16, tag="wqf")
            for m in range(3):
                nc.vector.tensor_scalar(wqf_t[:, m, :], wq8_t[:, m, :],
                                        wqsc_t[:, m:m + 1], None,
                                        mybir.AluOpType.mult)
            for h in range(N_HEAD):
                nc.vector.tensor_copy(
                    wq_t[:, :, _PC[h]:_PC[h] + HD_K],
                    wqf_t[:, :, h * HD_K:(h + 1) * HD_K])
            wk_t = pw.tile([128, 9, N_KP], BF16)
            wkf_t = pw.tile([128, 9, N_KQ], BF16, tag="wkf")
            wv_t = pw.tile([128, 9, N_VA], BF16)
            for e in range(9):
                st = pst.tile([128, N_KQ], I8, tag="wk8")
                nc.sync.dma_start(
                    out=st,
                    in_=wfl[WQ_ELS + e * 128 * N_KQ:
                            WQ_ELS + (e + 1) * 128 * N_KQ]
                    .rearrange("(p n) -> p n", p=128))
                nc.vector.tensor_scalar(wkf_t[:, e, :], st,
                                        wksc_t[:, e:e + 1], None,
                                        mybir.AluOpType.mult)
            for h in range(N_HEAD):
                nc.vector.tensor_copy(
                    wk_t[:, :, _PC[h]:_PC[h] + HD_K],
                    wkf_t[:, :, h * HD_K:(h + 1) * HD_K])
            for e in range(9):
                st = pst.tile([128, N_VA], I8, tag="wv8")
                nc.sync.dma_start(
                    out=st,
                    in_=wfl[WQ_ELS + WK_ELS + e * 128 * N_VA:
                            WQ_ELS + WK_ELS + (e + 1) * 128 * N_VA]
                    .rearrange("(p n) -> p n", p=128))
                nc.vector.tensor_scalar(wv_t[:, e, :], st,
                                        wvsc_t[:, e:e + 1], None,
                                        mybir.AluOpType.mult)
            wpsc_t = pw.tile([96, 12], F32, tag="wpsc")
            nc.sync.dma_start(out=wpsc_t, in_=wpsc_ap)
            wphf_r = wph_full.ap().rearrange("(h p) n -> p h n", p=96)
            for h in range(N_HEAD):
                st = pst.tile([96, N_OUT], I8, tag="wph8")
                nc.sync.dma_start(out=st, in_=wphf_r[:, h, :])
                nc.vector.tensor_scalar(wph_t[:, h, :], st,
                                        wpsc_t[:, h:h + 1], None,
                                        mybir.AluOpType.mult)
            qb_t = pw.tile([128, 2 * CH], F32)
            nc.sync.dma_start(out=qb_t, in_=qoff_ap.to_broadcast([128, 2 * CH]))
            bq_t = pw.tile([128, 3], F32)
            nc.sync.dma_start(out=bq_t, in_=bq_ap)
            bk_t = pw.tile([128, 3], F32)
            nc.sync.dma_start(out=bk_t, in_=bk_ap)
            bv_t = pw.tile([128, N_VA], F32)
            nc.sync.dma_start(out=bv_t, in_=bv_ap.to_broadcast([128, N_VA]))
            nc.sync.dma_start(out=bp_t, in_=bp_ap)

            # ---- pre-touches: give each engine 1-wait visibility of loads ----
            dps = psp.tile([128, 512], F32, tag="ps")
            for i, t in enumerate(
                [xs_t[0:1, 0, 0:1], wq_t[0:1, 0, 0:1],
                 wk_t[0:1, 0, 0:1], wv_t[0:1, 0, 0:1], wph_t[0:1, 0, 0:1]]
            ):
                nc.tensor.matmul(dps[0:1, i:i + 1], lhsT=t, rhs=t,
                                 start=True, stop=True)
            sc = psc.tile([1, 16], F32)
            nc.scalar.activation(sc[0:1, 0:1], bq_t[0:1, 0:1], AF.Copy)
            nc.scalar.activation(sc[0:1, 1:2], bk_t[0:1, 0:1], AF.Copy)
            nc.scalar.activation(sc[0:1, 2:3], bp_t[0:1, 0:1], AF.Copy)
            scv = psc.tile([1, 16], F32, tag="scv")
            nc.vector.tensor_copy(scv[0:1, 0:1], bv_t[0:1, 0:1])
            nc.vector.tensor_copy(scv[0:1, 1:2], qb_t[0:1, 0:1])
            # ACT warm-up of Exp's implicit const-bias AP
            sce = psc.tile([1, 16], F32, tag="sce")
            nc.scalar.activation(sce[0:1, 0:1], scv[0:1, 0:1], AF.Exp)

            # ---- mask gen: m[p, f] = (qidx[f] - (128*kt + p) > 0) ----
            ci_t = pw.tile([128, 2 * CH], mybir.dt.int32)
            nc.gpsimd.iota(ci_t, pattern=[[1, 2 * CH]], base=0,
                           channel_multiplier=-1)
            cif_t = pw.tile([128, 2 * CH], F32)
            nc.vector.tensor_copy(cif_t, ci_t)
            mb_t = pw.tile([128, 2 * CH], F32)
            nc.vector.tensor_add(mb_t, cif_t, qb_t)
            for kt in range(8):
                nc.vector.tensor_scalar(
                    mC_t[:, kt, :], mb_t, float(128 * kt), None,
                    mybir.AluOpType.is_gt)
            for kt in range(8, 16):
                nc.vector.tensor_scalar(
                    mD_t[:, kt - 8, :], mb_t[:, CH:], float(128 * kt), None,
                    mybir.AluOpType.is_gt)
            nc.vector.tensor_scalar(id_t, cif_t[:, :128], 0.0, None,
                                    mybir.AluOpType.is_equal)

            # ---- q projection: qpad[384, 256]; q-token columns are sliced
            #      out of the gathered xs at runtime via partition-id ----
            offA = nc.s_assert_within(pid * CH, 0, L,
                                      skip_runtime_assert=True)
            offB = nc.s_assert_within((15 - pid) * CH, 0, L,
                                      skip_runtime_assert=True)
            sq_t = pw.tile([128, 3, 2 * CH], BF16)
            for e in range(3):
                nc.vector.tensor_copy(sq_t[:, e, :CH],
                                      xs_t[:, 6 + e, bass.ds(offA, CH)])
                nc.vector.tensor_copy(sq_t[:, e, CH:],
                                      xs_t[:, 6 + e, bass.ds(offB, CH)])
            for m in range(3):
                ps = psp.tile([128, 2 * CH], F32, tag="ps")
                for e in range(3):
                    nc.tensor.matmul(
                        ps, lhsT=wq_t[:, e, m * 128:(m + 1) * 128],
                        rhs=sq_t[:, e, :],
                        start=(e == 0), stop=(e == 2),
                    )
                nc.scalar.activation(qpad[:, m, :], ps, AF.Identity,
                                     bias=bq_t[:, m:m + 1])

            # ---- k projection: kpad[384, 2048], 512-token slabs ----
            for m in range(3):
                for nt in range(L // 512):
                    ps = psp.tile([128, 512], F32, tag="ps")
                    for e in range(9):
                        nc.tensor.matmul(
                            ps,
                            lhsT=wk_t[:, e, m * 128:(m + 1) * 128],
                            rhs=xs_t[:, e, nt * 512:(nt + 1) * 512],
                            start=(e == 0), stop=(e == 8),
                        )
                    nc.scalar.activation(
                        kpad[:, m, nt * 512:(nt + 1) * 512], ps, AF.Identity,
                        bias=bk_t[:, m:m + 1],
                    )

            # ---- v projection: v[2048, 1164] (token-major, augmented) ----
            for c in range(L // 128):
                ps = psv.tile([128, N_VA], F32, tag="vps")
                for e in range(9):
                    for n0, nn in [(0, 512), (512, 512), (1024, N_VA - 1024)]:
                        nc.tensor.matmul(
                            ps[:, n0:n0 + nn],
                            lhsT=xs_t[:, e, c * 128:(c + 1) * 128],
                            rhs=wv_t[:, e, n0:n0 + nn],
                            start=(e == 0), stop=(e == 8),
                        )
                nc.vector.tensor_add(v_t[:, c, :], ps, bv_t)

        # ---- attention ----
        with (
            tc.tile_pool(name="ps_s", bufs=4, space="PSUM") as pss,
            tc.tile_pool(name="ps_y", bufs=3, space="PSUM") as psy,
            tc.tile_pool(name="exps", bufs=20) as pe,
            tc.tile_pool(name="norm", bufs=2) as pn,
            tc.tile_pool(name="rdram", bufs=6, space="DRAM") as pdram,
        ):
            for h in range(N_HEAD):
                t, a = h // 4, 32 * (h % 4)
                ems = []
                for kt in range(8):
                    s_ps = pss.tile([128, 2 * CH], F32, tag="sps")
                    nc.tensor.matmul(
                        s_ps,
                        lhsT=kpad[a:a + HD_K, t, kt * 128:(kt + 1) * 128],
                        rhs=qpad[a:a + HD_K, t, :],
                        start=True, stop=True,
                        tile_position=(a, 0),
                    )
                    e_sb = pe.tile([128, 2 * CH], BF16, tag="esb")
                    nc.scalar.activation(e_sb, s_ps, AF.Exp, scale=0.25)
                    em_sb = pe.tile([128, 2 * CH], BF16, tag="emsb")
                    nc.vector.tensor_mul(em_sb, e_sb, mC_t[:, kt, :])
                    ems.append(em_sb)
                for kt in range(8, 16):
                    s_ps = pss.tile([128, 2 * CH], F32, tag="sps")
                    nc.tensor.matmul(
                        s_ps[:, :CH],
                        lhsT=kpad[a:a + HD_K, t, kt * 128:(kt + 1) * 128],
                        rhs=qpad[a:a + HD_K, t, CH:],
                        start=True, stop=True,
                        tile_position=(a, 0),
                    )
                    e_sb = pe.tile([128, 2 * CH], BF16, tag="esb")
                    nc.scalar.activation(e_sb[:, :CH], s_ps[:, :CH], AF.Exp,
                                         scale=0.25)
                    em_sb = pe.tile([128, 2 * CH], BF16, tag="emsb")
                    nc.vector.tensor_mul(em_sb[:, :CH], e_sb[:, :CH],
                                         mD_t[:, kt - 8, :])
                    ems.append(em_sb)
                y_ps = psy.tile([HD_VA, 2 * CH], F32, tag="yps")
                for kt in range(8):
                    nc.tensor.matmul(
                        y_ps,
                        lhsT=v_t[:, kt, h * HD_VA:(h + 1) * HD_VA],
                        rhs=ems[kt],
                        start=(kt == 0), stop=False,
                    )
                for kt in range(8, 16):
                    nc.tensor.matmul(
                        y_ps[:, CH:],
                        lhsT=v_t[:, kt, h * HD_VA:(h + 1) * HD_VA],
                        rhs=ems[kt][:, :CH],
                        start=False, stop=(kt == 15),
                    )
                # normalize: row 96 of y_ps is the softmax denominator
                # (clamped away from 0 so the dead q=0 column yields 0, not NaN)
                r_sb = pn.tile([128, 2 * CH], F32, tag="rsb")
                rmx = pn.tile([128, 2 * CH], F32, tag="rmx")
                nc.vector.tensor_scalar_max(rmx[96:97, :], y_ps[96:97, :],
                                            1e-30)
                nc.vector.reciprocal(r_sb[96:97, :], rmx[96:97, :])
                rd = pdram.tile([1, 2 * CH], F32, tag="rd")
                nc.sync.dma_start(out=rd, in_=r_sb[96:97, :])
                rb_t = pn.tile([HD_V, 2 * CH], F32, tag="rbt")
                nc.sync.dma_start(
                    out=rb_t, in_=rd[0:1, :].to_broadcast([HD_V, 2 * CH])
                )
                rtc = pn.tile([1, 1], F32, tag="rtc")
                nc.vector.tensor_copy(rtc, rb_t[0:1, 0:1])  # pre-touch
                nc.vector.tensor_mul(yts[h], y_ps[:HD_V, :], rb_t)

        # ---- output projection: outT[1152, 256] = sum_h Wp_h^T @ y_h,
        #      then per-token int8 quantization: transpose, abs-max, scale ----
        with (
            tc.tile_pool(name="ps_o", bufs=2, space="PSUM") as pso,
            tc.tile_pool(name="ps_q", bufs=2, space="PSUM") as psq,
            tc.tile_pool(name="qsb", bufs=3) as pq,
            tc.tile_pool(name="qsc", bufs=1) as pqs,
        ):
            outb = pqs.tile([128, 9, 2 * CH], F32, tag="outb")
            for mo in range(9):
                ps = pso.tile([128, 2 * CH], F32)
                for h in range(N_HEAD):
                    nc.tensor.matmul(
                        ps,
                        lhsT=wph_t[:, h, mo * 128:(mo + 1) * 128],
                        rhs=yts[h],
                        start=(h == 0), stop=(h == N_HEAD - 1),
                    )
                nc.scalar.activation(outb[:, mo, :], ps, AF.Identity,
                                     bias=bp_t[:, mo:mo + 1])
            sc_all = pqs.tile([128, 2], F32)
            rcp = pqs.tile([128, 2], F32, tag="rcp")
            mxs = pqs.tile([128, 2], F32, tag="mxs")
            for tcn in range(2):
                psT = psq.tile([128, N_OUT], F32, tag="psT")
                for mo in range(9):
                    nc.tensor.matmul(
                        psT[:, mo * 128:(mo + 1) * 128],
                        lhsT=outb[:, mo, tcn * 128:(tcn + 1) * 128],
                        rhs=id_t, is_transpose=True,
                        start=True, stop=True,
                    )
                nc.vector.tensor_reduce(
                    mxs[:, tcn:tcn + 1], psT, axis=mybir.AxisListType.X,
                    op=mybir.AluOpType.max, apply_absolute_value=True)
                nc.vector.tensor_scalar_mul(sc_all[:, tcn:tcn + 1],
                                            mxs[:, tcn:tcn + 1], 1.0 / 127.0)
                nc.vector.reciprocal(rcp[:, tcn:tcn + 1],
                                     sc_all[:, tcn:tcn + 1])
                qf = pq.tile([128, N_OUT], F32, tag="qf")
                nc.vector.tensor_scalar(qf, psT, rcp[:, tcn:tcn + 1], MAGIC,
                                        mybir.AluOpType.mult,
                                        mybir.AluOpType.add)
                qg = pq.tile([128, N_OUT], F32, tag="qg")
                nc.vector.tensor_scalar(qg, qf, MAGIC, None,
                                        mybir.AluOpType.subtract)
                qi = pq.tile([128, N_OUT], mybir.dt.int8, tag="qi")
                nc.vector.tensor_copy(qi, qg)
                nc.sync.dma_start(
                    out=out_d.ap()[tcn * 128:(tcn + 1) * 128, :], in_=qi)
            sc_dst = (out_d.ap()[2 * CH:OUT_ROWS, :].flatten()[0:2 * CH * 4]
                      .bitcast(F32).rearrange("(p n) -> p n", p=128))
            nc.sync.dma_start(out=sc_dst, in_=sc_all)
    return nc


def _legalize_waits(nc):
    """This walrus build accepts only ONE sync-wait per regular instruction;
    move overflow waits onto injected same-engine NoOps (like raw-bass
    wait_ge)."""
    keep = ("InstEventSemaphore",)
    cnt = 0
    for bbh in nc.bb_map.values():
        bb = bbh.bb
        new_list = []
        for inst in bb.instructions:
            si = inst.sync_info
            if (si is not None and len(si.on_wait) > 1
                    and type(inst).__name__ not in keep):
                waits = list(si.on_wait)
                for w in waits[:-1]:
                    cnt += 1
                    n = mybir.InstNoOp(name=f"legwait_{cnt}", ins=[], outs=[])
                    n.engine = inst.engine
                    n.sync_info = mybir.SyncInfo(on_wait=[w], on_update=[])
                    try:
                        nc.register_instruction(n)
                    except Exception:
                        pass
                    new_list.append(n)
                inst.sync_info = mybir.SyncInfo(
                    on_wait=[waits[-1]], on_update=list(si.on_update))
            new_list.append(inst)
        bb.instructions = new_list
    return cnt


class _Runner:
    """Cached 2-dispatch pipeline. Replicates run_bass_via_pjrt's
    _bass_exec_p lowering with a cached jit; uploads the weight image once
    per call (shared device array); issues batch 0's result-copy request
    BETWEEN the two executes so its download streams during batch 1's
    upload/execute (the relay serves requests FIFO); recycles previous
    outputs as donated buffers; fetches output shards with a thread pool."""

    def __init__(self, nc):
        install_neuronx_cc_hook()
        self.nc = nc
        partition_name = (nc.partition_id_tensor.name
                          if nc.partition_id_tensor else None)
        in_names, out_names, out_avals = [], [], []
        for alloc in nc.m.functions[0].allocations:
            if not isinstance(alloc, mybir.MemoryLocationSet):
                continue
            name = alloc.memorylocations[0].name
            if alloc.kind == "ExternalInput":
                if name != partition_name:
                    in_names.append(name)
            elif alloc.kind == "ExternalOutput":
                out_names.append(name)
                out_avals.append(jax.core.ShapedArray(
                    tuple(alloc.tensor_shape), mybir.dt.np(alloc.dtype)))
        n_params, n_outs = len(in_names), len(out_avals)
        assert in_names == ["xblob", "wblob"] and out_names == ["out"]
        in_names_all = in_names + out_names + (
            [partition_name] if partition_name else [])

        def _body(*args):
            operands = list(args)
            if partition_name is not None:
                operands.append(partition_id_tensor())
            return tuple(_bass_exec_p.bind(
                *operands, out_avals=tuple(out_avals),
                in_names=tuple(in_names_all), out_names=tuple(out_names),
                lowering_input_output_aliases=(), sim_require_finite=True,
                sim_require_nnan=True, nc=nc))

        devices = jax.devices()[:8]
        assert len(devices) == 8
        self.mesh = Mesh(np.asarray(devices), ("core",))
        self.sharding = NamedSharding(self.mesh, PartitionSpec("core"))
        donate = tuple(range(n_params, n_params + n_outs))
        self.sharded = jax.jit(
            shard_map(_body, mesh=self.mesh,
                      in_specs=(PartitionSpec("core"),) * (n_params + n_outs),
                      out_specs=(PartitionSpec("core"),) * n_outs,
                      check_rep=False),
            donate_argnums=donate, keep_unused=True)
        self.zmaker = jax.jit(
            lambda: jnp.zeros((8 * OUT_ROWS, N_OUT), jnp.int8),
            out_shardings=self.sharding)
        self.spares = None
        self.pool = ThreadPoolExecutor(8)

    def warm(self):
        """Seed the donated-buffer pool on device (no host traffic)."""
        if self.spares is None:
            s0, s1 = self.zmaker(), self.zmaker()
            jax.block_until_ready((s0, s1))
            self.spares = (s0, s1)

    def _fetch(self, o):
        shards = sorted(o.addressable_shards,
                        key=lambda sh: sh.index[0].start
                        if sh.index[0].start else 0)
        assert len(shards) == 8
        return list(self.pool.map(lambda sh: np.asarray(sh.data), shards))

    def __call__(self, xg0, xg1, wg):
        """Full device computation: returns per-core raw outputs for each
        batch, 8 x np.int8 [OUT_ROWS, N_OUT] per batch. One retry on
        transient relay failure ("worker hung up"): the donated spares are
        consumed/undefined after a failed dispatch, so the pool is reseeded
        (spares were already cleared before the dispatch; warm() reseeds)."""
        try:
            return self._run(xg0, xg1, wg)
        except Exception:
            self.spares = None
            return self._run(xg0, xg1, wg)

    def _run(self, xg0, xg1, wg):
        self.warm()
        s0, s1 = self.spares
        self.spares = None
        wdev = jax.device_put(wg, self.sharding)
        (o0,) = self.sharded(xg0, wdev, s0)
        o0.copy_to_host_async()  # MUST precede call 2: relay serves FIFO
        (o1,) = self.sharded(xg1, wdev, s1)
        o1.copy_to_host_async()
        parts0 = self._fetch(o0)
        parts1 = self._fetch(o1)
        self.spares = (o0, o1)  # recycle device buffers for next donation
        return parts0, parts1


def _get_runner():
    global _RUNNER
    if _RUNNER is None:
        nc = _build_graph()
        _legalize_waits(nc)
        # The pjrt lowering re-serializes the (frozen, never-mutated) graph
        # on every trace (~25ms for this BIR); memoize the identical bytes.
        raw = nc.to_json_bytes()
        nc.to_json_bytes = lambda: raw
        _RUNNER = _Runner(nc)
    return _RUNNER


def _head_pad_bias(b):
    """[192] -> [384] with head h vals at 128*(h//4)+32*(h%4)."""
    bp = np.zeros((N_KP,), np.float32)
    for h in range(N_HEAD):
        bp[_PC[h]:_PC[h] + HD_K] = b[h * HD_K:(h + 1) * HD_K]
    return bp


def _prep_inputs(x, side, Wq, bq, Wkv, bkv, Wproj, bproj):
    """Quantize + pack the wire blobs: returns (xg0, xg1, wg) global uint8
    arrays (concat of the 8 per-core shards along axis 0)."""
    Wk = Wkv[:, :N_KQ]
    Wv = Wkv[:, N_KQ:]
    bk = bkv[:N_KQ]
    bv = bkv[N_KQ:]
    bq_p = _head_pad_bias(bq)
    bk_p = _head_pad_bias(bk)
    # augmented V: per head 96 channels + a zero-weight/one-bias denom channel
    Wv_a = np.zeros((N_OUT, N_VA), np.float32)
    bv_a = np.zeros((N_VA,), np.float32)
    for h in range(N_HEAD):
        Wv_a[:, h * HD_VA:h * HD_VA + HD_V] = Wv[:, h * HD_V:(h + 1) * HD_V]
        bv_a[h * HD_VA:h * HD_VA + HD_V] = bv[h * HD_V:(h + 1) * HD_V]
        bv_a[h * HD_VA + HD_V] = 1.0

    # packed int8 q/k/v weights (per input-channel-row scales), unpadded
    def q8_rows(W):
        sc = np.maximum(np.abs(W).max(axis=1), 1e-30) / 127.0
        q = np.clip(np.round(W / sc[:, None]), -127, 127).astype(np.int8)
        return q, sc.astype(np.float32)

    wq8, wqsc = q8_rows(Wq)          # [384, 192]
    wk8, wksc = q8_rows(Wk)          # [1152, 192]
    wv8, wvsc = q8_rows(Wv_a)        # [1152, 1164]
    wscales = np.concatenate([wqsc, wksc, wvsc]).astype(np.float32)
    biases = np.concatenate([bq_p, bk_p, bv_a, bproj]).astype(np.float32)

    # Wproj rows per head, int8 per-row [1152, 1152]
    wph_all, wphsc = q8_rows(Wproj.reshape(N_HEAD * HD_V, N_OUT))

    wbuf = np.zeros((W_ROWS * 2048,), np.uint8)
    wbuf[0:WQ_ELS] = wq8.reshape(-1).view(np.uint8)
    wbuf[WQ_ELS:WQ_ELS + WK_ELS] = wk8.reshape(-1).view(np.uint8)
    wbuf[WQ_ELS + WK_ELS:O_FBIAS] = wv8.reshape(-1).view(np.uint8)
    wbuf[O_FBIAS:O_FWSC] = biases.view(np.uint8)
    wbuf[O_FWSC:O_FWPSC] = wscales.view(np.uint8)
    wbuf[O_FWPSC:W_BYTES_USED] = wphsc.view(np.uint8)

    wg = np.empty((8, WBLOB_BYTES), np.uint8)
    for i in range(8):
        wg[i, 0:O_WPH_B] = wbuf[W_SH_ROWS * 2048 * i:
                                W_SH_ROWS * 2048 * (i + 1)]
        wg[i, O_WPH_B:WBLOB_BYTES] = (
            wph_all[WPH_SH_ROWS * i:WPH_SH_ROWS * (i + 1), :]
            .reshape(-1).view(np.uint8))

    # per-channel int8 scales for [x|side]^T, shared by all cores of a batch
    xgs = []
    for b in range(B):
        xsT = np.ascontiguousarray(np.concatenate([x[b], side[b]], axis=1).T)
        xsc = np.maximum(np.abs(xsT).max(axis=1), 1e-30) / 127.0
        xsq = np.clip(np.round(xsT / xsc[:, None]), -127, 127).astype(np.int8)
        xscf = xsc.astype(np.float32)
        xg = np.empty((8, XBLOB_BYTES), np.uint8)
        for i in range(8):
            # qidx[f] - f for the mask generator: q token of em column f
            qoff = np.empty((2 * CH,), np.float32)
            qoff[:CH] = CH * i
            qoff[CH:] = CH * (15 - i) - CH
            xg[i, 0:O_QOFF] = (xsq[XS_SH_ROWS * i:XS_SH_ROWS * (i + 1), :]
                               .reshape(-1).view(np.uint8))
            xg[i, O_QOFF:O_XSC] = qoff.view(np.uint8)
            xg[i, O_XSC:XBLOB_BYTES] = xscf.view(np.uint8)
        xgs.append(xg.reshape(-1))
    return xgs[0], xgs[1], wg.reshape(-1)


def _unpack(parts, ans, b):
    """parts: 8 x [OUT_ROWS, N_OUT] int8 for batch b -> ans[b] float32."""
    for i in range(8):
        core = parts[i]
        scales = (core[2 * CH:].reshape(-1).view(np.float32)[:2 * CH]
                  .reshape(128, 2))              # [partition, chunk]
        vals = core[:2 * CH].astype(np.float32)  # [256 tokens, 1152]
        for tcn in range(2):
            vals[tcn * 128:(tcn + 1) * 128] *= scales[:, tcn:tcn + 1]
        ans[b, CH * i:CH * (i + 1)] = vals[:CH]
        ans[b, CH * (15 - i):CH * (16 - i)] = vals[CH:]


def kernel(x, side, Wq, bq, Wkv, bkv, Wproj, bproj, Wemb, bemb, **_unused):
    x = np.asarray(x, np.float32)
    side = np.asarray(side, np.float32)
    Wq = np.asarray(Wq, np.float32)
    bq = np.asarray(bq, np.float32)
    Wkv = np.asarray(Wkv, np.float32)
    bkv = np.asarray(bkv, np.float32)
    Wproj = np.asarray(Wproj, np.float32)
    bproj = np.asarray(bproj, np.float32)
    Wemb = np.asarray(Wemb, np.float32)
    bemb = np.asarray(bemb, np.float32)

    runner = _get_runner()
    xg0, xg1, wg = _prep_inputs(x, side, Wq, bq, Wkv, bkv, Wproj, bproj)
    parts0, parts1 = runner(xg0, xg1, wg)

    ans = np.empty((B, L, N_OUT), np.float32)
    _unpack(parts0, ans, 0)
    _unpack(parts1, ans, 1)
    # first token: replaced by learned embedding of side[:, 0] (exact, host-side)
    for b in range(B):
        first = side[b, 0].astype(np.float64) @ Wemb.astype(np.float64) + bemb
        ans[b, 0] = (first @ Wproj.astype(np.float64) + bproj).astype(np.float32)
    return ans
